# revision 25
# baseline (speedup 1.0000x reference)
"""DualBranchMoENet on Trainium2 — 8-core data-parallel (16 samples/core).

Channels live on SBUF partitions, (batch, time) on the free dim. Heavy
matmuls run fp32r (1 cyc/row at N>=256). Convolutions contract (cin, tap)
on the PE partition axis via shifted access patterns; only expert conv1
(129 ch x 5 taps) materialises an im2col stack. The LSTM keeps its hidden
state transposed ([256c, 16b]) so h @ whh^T needs no input transpose;
h is re-transposed once per step on the PE. The backward LSTM output
hb[T-1] equals ONE cell evaluated at t=32 from the zero state.
"""
import sys
sys.path.insert(0, '/opt/trn_rl_repo')
import numpy as np
import ml_dtypes

N_CORES = 8
B = 128
BC = B // N_CORES
L = 4096
NFFT = 256
NF = 129
T = 33
NE = 8

_cache = {}

# LSTM gate reorder (torch i,f,g,o) -> (i,f,o,g) so the three sigmoid gates
# are contiguous and fuse into one activation instruction.
GPERM = np.concatenate([np.arange(0, 512), np.arange(768, 1024), np.arange(512, 768)])


def _interp_tables():
    coords = np.clip((np.arange(T) + 0.5) * (64.0 / T) - 0.5, 0.0, 63.0)
    lo = np.floor(coords).astype(np.int64)
    w = coords - lo
    runs = []
    a = 0
    while a < T:
        b = a + 1
        if b < T:
            step = lo[a + 1] - lo[a]
            while b < T and lo[b] - lo[b - 1] == step:
                b += 1
        runs.append((a, b - a, int(lo[a]), int(lo[a + 1] - lo[a]) if b - a >= 2 else 1))
        a = b
    return runs, lo, w


def _build(ffn_b2_val):
    from concourse import bass, tile, mybir
    from concourse.mybir import AluOpType as alu
    from concourse.mybir import ActivationFunctionType as actf

    f32 = mybir.dt.float32
    f32r = mybir.dt.float32r
    bf16 = mybir.dt.bfloat16
    X = mybir.AxisListType.X

    BF16_IN = {'w1p', 'w2p', 'x7b', 'w14r', 'w2Lr', 'w3L', 'wihft', 'wihbt'}
    nc = bass.Bass()
    inp = lambda name, shape: nc.declare_dram_parameter(
        name, list(shape), bf16 if name in BF16_IN else f32, isOutput=False)

    d = {}
    for name, shape in [
        ('framesT', [NFFT, BC * T]), ('x7b', [3, 96, 8, 520]),
        ('crw', [NFFT, NF]), ('ciw', [NFFT, NF]),
        ('gw1ta', [128, 128]), ('gw1tb', [1, 128]), ('gb1c', [128, 1]),
        ('gw2t', [128, NE]), ('gb2c', [NE, 1]), ('iota8', [BC, NE]), ('ones1', [1, 128]), ('sel8', [NE, NE * 128]), ('zer', [128, 70]),
        ('w1p', [NE, 645, 256]), ('w1b', [256, NE]),
        ('w2p', [NE, 768, 256]), ('w2b', [256, NE]),
        ('w14r', [96, 128]), ('lb1c', [128, 1]),
        ('w2Lr', [128, 3, 128]), ('lb2c', [128, 1]),
        ('se2w1t', [128, 32]), ('se2w2t', [32, 128]),
        ('w3L', [3, 128, 256]), ('lb3c', [256, 1]),
        ('se3w1t', [256, 64]), ('se3w2t', [64, 256]),
        ('wlo', [1, T]),
        ('wihft', [512, 1024]), ('biasfT', [128, NE]), ('whhft', [256, 1024]),
        ('wihbt', [512, 1024]), ('biasbT', [128, NE]),
        ('i16', [16, 16]), ('i128', [128, 128]),
        ('ffn1t', [512, 256]), ('ffnb1', [256, 1]), ('ffn2t', [256, 1]),
    ]:
        d[name] = inp(name, shape)
    yout = nc.declare_dram_parameter('yout', [BC], f32, isOutput=True)

    runs, lo_t, w_t = _interp_tables()

    def mm(out, lhsT, rhs, start, stop):
        nc.tensor.matmul(out, lhsT, rhs, start=start, stop=stop)

    def mmr(out, lhsT, rhs, start, stop):
        nc.tensor.matmul(out, lhsT.bitcast(f32r), rhs.bitcast(f32r),
                         start=start, stop=stop)

    def mmf(out, lhsT, rhs, start, stop):
        nc.tensor.matmul(out, lhsT, rhs, start=start, stop=stop)

    with tile.TileContext(nc, num_cores=N_CORES) as tc:
        with (
            tc.tile_pool(name='const', bufs=1) as cp,
            tc.tile_pool(name='work', bufs=2) as wp,
            tc.tile_pool(name='one', bufs=1) as wp1,
            tc.tile_pool(name='big', bufs=1) as bp,
            tc.tile_pool(name='psA', bufs=2, space='PSUM') as psA,
            tc.tile_pool(name='psB', bufs=4, space='PSUM') as psB,
            tc.tile_pool(name='psC', bufs=2, space='PSUM') as psC,
        ):
            C = {}
            for name, shape, rr in [
                ('crw', [128, 2, NF], '(k p) m -> p k m'),
                ('ciw', [128, 2, NF], '(k p) m -> p k m'),
                ('gw1ta', [128, 128], None), ('gw1tb', [1, 128], None),
                ('gb1c', [128, 1], None),
                ('gw2t', [128, NE], None), ('gb2c', [NE, 1], None),
                ('iota8', [BC, NE], None),
                ('ones1', [1, 128], None),
                ('sel8', [NE, NE * 128], None),
                ('w1b', [128, 2, NE], '(k p) m -> p k m'),
                ('w2b', [128, 2, NE], '(k p) m -> p k m'),
                ('lb1c', [128, 1], None),
                ('lb2c', [128, 1], None),
                ('se2w1t', [128, 32], None), ('se2w2t', [32, 128], None),
                ('lb3c', [128, 2, 1], '(k p) m -> p k m'),
                ('se3w1t', [128, 2, 64], '(k p) m -> p k m'),
                ('se3w2t', [64, 256], None),
                ('wlo', [1, T], None),
                ('biasfT', [128, NE], None),
                ('whhft', [128, 2, 1024], '(k p) m -> p k m'),
                ('biasbT', [128, NE], None),
                ('i16', [16, 16], None), ('i128', [128, 128], None),
                ('ffn1t', [128, 4, 256], '(k p) m -> p k m'),
                ('ffnb1', [128, 2, 1], '(k p) m -> p k m'),
                ('ffn2t', [128, 2, 1], '(k p) m -> p k m'),
            ]:
                t = cp.tile(shape, f32, tag=name)
                src = d[name][:]
                if rr:
                    src = src.rearrange(rr, p=128)
                if name in ('crw', 'ciw', 'whhft', 'i128'):
                    nc.sync.dma_start(t[:].bitcast(f32r), src.bitcast(f32r))
                else:
                    nc.sync.dma_start(t[:], src)
                C[name] = t
            t = cp.tile([96, 128], bf16, tag='w14r')
            nc.sync.dma_start(t[:], d['w14r'][:])
            C['w14r'] = t
            t = cp.tile([128, 3, 128], bf16, tag='w2Lr')
            nc.sync.dma_start(t[:], d['w2Lr'][:])
            C['w2Lr'] = t
            t = cp.tile([128, 3, 256], bf16, tag='w3L')
            nc.sync.dma_start(t[:], d['w3L'][:].rearrange('d k m -> k d m'))
            C['w3L'] = t

            # ---------------- STFT magnitude ----------------
            lp_cm = tc.tile_pool(name='lp', bufs=2)
            lp = lp_cm.__enter__()
            ep_cm = tc.tile_pool(name='ep', bufs=2)
            ep = ep_cm.__enter__()
            c_fr = ep.tile([128, 2, BC * T], f32, tag='framesT', name='framesT', bufs=1)
            nc.sync.dma_start(c_fr[:].bitcast(f32r), d['framesT'][:].rearrange('(k p) m -> p k m', p=128).bitcast(f32r))
            C['framesT'] = c_fr
            magA = ep.tile([128, BC, T + 4], f32, tag='magA', name='magA', bufs=1)
            magB = ep.tile([1, BC, T + 4], f32, tag='magB', name='magB', bufs=1)
            nc.vector.memset(magA[:], 0.0)
            nc.vector.memset(magB[:], 0.0)
            NB2 = BC * T // 2
            for m0, mn, magX in [(0, 128, magA), (128, 1, magB)]:
                sqr = ep.tile([mn, BC * T], f32, tag=f'sqr{m0}', name=f'sqr{m0}', bufs=1)
                sqi = ep.tile([mn, BC * T], f32, tag=f'sqi{m0}', name=f'sqi{m0}', bufs=1)
                for ni in range(2):
                    pre = psA.tile([mn, NB2], f32, tag='pa', name='pa')
                    pim = psA.tile([mn, NB2], f32, tag='pa', name='pa')
                    for k in range(2):
                        co = slice(ni * NB2, (ni + 1) * NB2)
                        mmr(pre[:], C['crw'][:, k, m0:m0 + mn], C['framesT'][:, k, co], k == 0, k == 1)
                        mmr(pim[:], C['ciw'][:, k, m0:m0 + mn], C['framesT'][:, k, co], k == 0, k == 1)
                    nc.scalar.square(sqr[:, ni * NB2:(ni + 1) * NB2], pre[:])
                    nc.scalar.square(sqi[:, ni * NB2:(ni + 1) * NB2], pim[:])
                nc.vector.tensor_add(sqr[:], sqr[:], sqi[:])
                nc.scalar.sqrt(magX[0:mn, :, 2:2 + T],
                               sqr[:].rearrange('p (b t) -> p b t', b=BC))

            magAb = ep.tile([128, BC, T + 4], bf16, tag='magAb', name='magAb', bufs=1)
            magBb = ep.tile([1, BC, T + 4], bf16, tag='magBb', name='magBb', bufs=1)
            nc.scalar.activation(magAb[:], magA[:], actf.Copy)
            nc.scalar.activation(magBb[:], magB[:], actf.Copy)

            # ---------------- gating (fp32 matmuls) ----------------
            pooledA = ep.tile([128, BC], f32, tag='pooledA', name='pooledA')
            pooledB = ep.tile([1, BC], f32, tag='pooledB', name='pooledB')
            nc.vector.tensor_reduce(pooledA[:], magA[:, :, 2:2 + T], X, alu.add)
            nc.vector.tensor_reduce(pooledB[:], magB[:, :, 2:2 + T], X, alu.add)
            pg1 = psA.tile([128, BC], f32, tag='pa', name='pa')
            mmf(pg1[:], C['gw1ta'][:], pooledA[:], True, False)
            mmf(pg1[:], C['gw1tb'][:], pooledB[:], False, True)
            gh = ep.tile([128, BC], f32, tag='gh', name='gh')
            nc.scalar.activation(gh[:], pg1[:], actf.Relu, bias=C['gb1c'][:, 0:1])
            pg2 = psA.tile([NE, BC], f32, tag='pa', name='pa')
            mmf(pg2[:], C['gw2t'][:], gh[:], True, True)
            logitsT = ep.tile([NE, BC], f32, tag='logitsT', name='logitsT')
            nc.vector.tensor_tensor(logitsT[:], pg2[:],
                                    C['gb2c'][:, 0:1].to_broadcast([NE, BC]), alu.add)
            plg = psA.tile([BC, NE], f32, tag='pa', name='pa')
            nc.tensor.transpose(plg[:], logitsT[:], C['i16'][0:NE, 0:NE])
            lg = ep.tile([BC, NE], f32, tag='lg', name='lg')
            nc.vector.tensor_copy(lg[:], plg[:])
            iob = C['iota8'][:]

            def small(tag, shape=(BC, NE)):
                return ep.tile(list(shape), f32, tag=tag, name=tag)

            m1 = small('m1', (BC, 1))
            nc.vector.tensor_reduce(m1[:], lg[:], X, alu.max)
            eq1 = small('eq1')
            nc.vector.tensor_tensor(eq1[:], lg[:], m1[:].to_broadcast([BC, NE]), alu.is_equal)
            l2 = small('l2')
            nc.vector.scalar_tensor_tensor(l2[:], eq1[:], -1e30, lg[:], alu.mult, alu.add)
            m2 = small('m2', (BC, 1))
            nc.vector.tensor_reduce(m2[:], l2[:], X, alu.max)
            it1 = small('it1')
            nc.vector.tensor_tensor(it1[:], eq1[:], iob, alu.mult)
            idx1 = small('idx1', (BC, 1))
            nc.vector.tensor_reduce(idx1[:], it1[:], X, alu.max)
            eq2 = small('eq2')
            nc.vector.tensor_tensor(eq2[:], l2[:], m2[:].to_broadcast([BC, NE]), alu.is_equal)
            it2 = small('it2')
            nc.vector.tensor_tensor(it2[:], eq2[:], iob, alu.mult)
            idx2 = small('idx2', (BC, 1))
            nc.vector.tensor_reduce(idx2[:], it2[:], X, alu.max)
            dm = small('dm', (BC, 1))
            nc.vector.tensor_sub(dm[:], m1[:], m2[:])
            g1 = small('g1', (BC, 1))
            nc.scalar.activation(g1[:], dm[:], actf.Sigmoid)
            g2 = small('g2', (BC, 1))
            nc.vector.tensor_scalar(g2[:], g1[:], -1.0, 1.0, alu.mult, alu.add)
            eA = small('eA')
            nc.vector.tensor_tensor(eA[:], idx1[:].to_broadcast([BC, NE]), iob, alu.is_equal)
            eB = small('eB')
            nc.vector.tensor_tensor(eB[:], idx2[:].to_broadcast([BC, NE]), iob, alu.is_equal)
            tA = small('tA')
            nc.vector.tensor_tensor(tA[:], eA[:], g1[:].to_broadcast([BC, NE]), alu.mult)
            tB = small('tB')
            nc.vector.tensor_tensor(tB[:], eB[:], g2[:].to_broadcast([BC, NE]), alu.mult)
            W8 = small('W8')
            nc.vector.tensor_add(W8[:], tA[:], tB[:])
            pW8T = psA.tile([NE, BC], f32, tag='pa', name='pa')
            nc.tensor.transpose(pW8T[:], W8[:], C['i16'][:])
            W8T = ep.tile([NE, BC], f32, tag='W8T', name='W8T')
            nc.vector.tensor_copy(W8T[:], pW8T[:])

            # ---------------- line conv1 (emitted early: overlaps expert DMA) ----
            h1 = lp.tile([128, 8, 1028], bf16, tag='h1', name='h1', bufs=1)
            nc.gpsimd.memset(h1[:, :, 0:2], 0.0)
            nc.gpsimd.memset(h1[:, :, 1026:1028], 0.0)
            x7 = [ep.tile([96, 8, 520], bf16, tag=f'x7_{h}', name=f'x7_{h}', bufs=1)
                  for h in range(3)]
            for h in range(3):
                nc.sync.dma_start(x7[h][:], d['x7b'][h])
            for u in range(8):
                ub = (u % 3) * 32
                for gg in range(8):
                    pl1 = psC.tile([128, 512], f32, tag='pc', name='pc')
                    mm(pl1[:], C['w14r'][ub:ub + 32, :],
                       x7[u // 3][ub:ub + 32, gg, 0:512], True, True)
                    o0 = 2 + u * 128
                    nc.vector.tensor_reduce(h1[:, gg, o0:o0 + 128],
                                            pl1[:].rearrange('p (t q) -> p t q', q=4),
                                            X, alu.max)
            nc.scalar.activation(h1[:, :, 2:1026], h1[:, :, 2:1026], actf.Relu,
                                 bias=C['lb1c'][:, 0:1])

            # ---------------- experts (dense, weighted sum) ----------------
            imt = [ep.tile([128 if k < 5 else 5, BC, T], bf16, tag=f'im1_{k}', name=f'im1_{k}', bufs=1)
                   for k in range(6)]
            for dt in range(5):
                pos = dt * NF
                done = 0
                while done < NF:
                    k, r = divmod(pos + done, 128)
                    if done < 128:
                        n = min(128 - r, NF - done, 128 - done)
                        nc.sync.dma_start(imt[k][r:r + n],
                                          magAb[done:done + n, :, dt:dt + T])
                    else:
                        n = 1
                        nc.sync.dma_start(imt[k][r:r + 1], magBb[0:1, :, dt:dt + T])
                    done += n
            accF = [bp.tile([128, BC, T], f32, tag=f'accF{i}', name=f'accF{i}') for i in range(2)]
            H = BC // 2
            for e in range(NE):
                w1s = ep.tile([128, 6, 256], bf16, tag='w1s', name='w1s')
                nc.sync.dma_start(w1s[:, 0:5, :],
                                  d['w1p'][e, 0:640, :].rearrange('(k p) m -> p k m', p=128))
                nc.sync.dma_start(w1s[0:5, 5, :], d['w1p'][e, 640:645, :])
                he = [ep.tile([128, BC * (T + 2) + 2], bf16, tag=f'he{i}', name=f'he{i}') for i in range(2)]
                for i in range(2):
                    hv = he[i][:, 0:BC * (T + 2)].rearrange('p (b t) -> p b t', t=T + 2)
                    nc.gpsimd.memset(hv[:, :, 0:1], 0.0)
                    nc.gpsimd.memset(hv[:, :, T + 1:T + 2], 0.0)
                    nc.gpsimd.memset(he[i][:, BC * (T + 2):], 0.0)
                for mi in range(2):
                    for ni in range(2):
                        pe1 = psB.tile([128, H * T], f32, tag='pb', name='pb')
                        for k in range(6):
                            kn = 128 if k < 5 else 5
                            mm(pe1[:], w1s[0:kn, k, mi * 128:(mi + 1) * 128],
                               imt[k][:].rearrange('p b t -> p (b t)')[:, ni * H * T:(ni + 1) * H * T],
                               k == 0, k == 5)
                        nc.scalar.activation(he[mi][:, 0:BC * (T + 2)].rearrange('p (b t) -> p b t', t=T + 2)[:, ni * H:(ni + 1) * H, 1:1 + T],
                                             pe1[:].rearrange('p (b t) -> p b t', t=T),
                                             actf.Relu, bias=C['w1b'][:, mi, e:e + 1])
                w2s = ep.tile([128, 6, 256], bf16, tag='w2s', name='w2s')
                nc.sync.dma_start(w2s[:], d['w2p'][e].rearrange('(k p) m -> p k m', p=128))
                eo = [ep.tile([128, BC, T], f32, tag=f'eo{i}', name=f'eo{i}', bufs=1) for i in range(2)]
                W2 = T + 2
                for mi in range(2):
                    for bi in range(2):
                        pe2 = psB.tile([128, H * W2], f32, tag='pb', name='pb')
                        for k in range(6):
                            dt, ch = divmod(k, 2)
                            mm(pe2[:], w2s[:, k, mi * 128:(mi + 1) * 128],
                               he[ch][:, bi * H * W2 + dt:bi * H * W2 + dt + H * W2],
                               k == 0, k == 5)
                        nc.scalar.activation(eo[mi][:, bi * H:(bi + 1) * H, :],
                                             pe2[:].rearrange('p (b t) -> p b t', t=W2)[:, :, 0:T],
                                             actf.Relu, bias=C['w2b'][:, mi, e:e + 1])
                pwe = psA.tile([128, BC], f32, tag='pa', name='pwe')
                mmf(pwe[:], C['sel8'][:, e * 128:(e + 1) * 128], W8T[:], True, True)
                wE = ep.tile([128, BC], f32, tag='wE', name='wE')
                nc.vector.tensor_copy(wE[:], pwe[:])
                wbc = wE[:].unsqueeze(2).to_broadcast([128, BC, T])
                for mi in range(2):
                    if e == 0:
                        nc.vector.tensor_tensor(accF[mi][:].bitcast(f32r), eo[mi][:], wbc, alu.mult)
                    else:
                        eow = ep.tile([128, BC, T], f32, tag='eow', name='eow')
                        nc.vector.tensor_tensor(eow[:], eo[mi][:], wbc, alu.mult)
                        nc.vector.tensor_add(accF[mi][:].bitcast(f32r), accF[mi][:], eow[:])

            ep_cm.__exit__(None, None, None)

            # ---------------- line branch ----------------

            hp2 = lp.tile([128, BC, 258], bf16, tag='hp2', name='hp2', bufs=1)
            nc.gpsimd.memset(hp2[:, :, 0:1], 0.0)
            nc.gpsimd.memset(hp2[:, :, 257:258], 0.0)
            hp2v = hp2[:, :, 1:257].rearrange('p (g s) t -> p s g t', s=2)
            seacc = lp.tile([128, 2, 8, 2], f32, tag='seacc', name='seacc', bufs=1)
            hs = [lp.tile([128, 8, 1028], bf16, tag=f'hs{s}', name=f'hs{s}', bufs=1)
                  for s in range(2)]
            for s in range(2):
                nc.sync.dma_start(hs[s][0:64], h1[s * 64:(s + 1) * 64])
                nc.sync.dma_start(hs[s][64:128, :, 0:1027], h1[s * 64:(s + 1) * 64, :, 1:1028])
            for s in range(2):
                for gg in range(8):
                    for uh in range(2):
                        pl2 = psB.tile([128, 512], f32, tag='pb', name='pb')
                        base = uh * 512
                        mm(pl2[:], C['w2Lr'][:, 0, :], hs[s][:, gg, base:base + 512],
                           True, False)
                        mm(pl2[:], C['w2Lr'][:, 1, :], hs[s][:, gg, base + 2:base + 514],
                           False, False)
                        mm(pl2[:], C['w2Lr'][0:64, 2, :], hs[s][0:64, gg, base + 4:base + 516],
                           False, True)
                        r2 = lp.tile([128, 512], f32, tag='r2', name='r2')
                        nc.scalar.activation(r2[:], pl2[:], actf.Relu, bias=C['lb2c'][:, 0:1],
                                             accum_out=seacc[:, s, gg, uh:uh + 1])
                        nc.vector.tensor_reduce(hp2v[:, s, gg, uh * 128:uh * 128 + 128],
                                                r2[:].rearrange('p (w q) -> p w q', q=4),
                                                X, alu.max)
            seY = lp.tile([128, 2, 8], f32, tag='seY', name='seY')
            nc.vector.tensor_reduce(seY[:], seacc[:], X, alu.add)
            pse1 = psA.tile([32, 16], f32, tag='pa', name='pa')
            mmf(pse1[:], C['se2w1t'][:], seY[:].rearrange('p s g -> p (s g)'), True, True)
            z2 = lp.tile([32, 16], f32, tag='z2', name='z2')
            nc.scalar.activation(z2[:], pse1[:], actf.Relu)
            pse2 = psA.tile([128, 16], f32, tag='pa', name='pa')
            mmf(pse2[:], C['se2w2t'][:], z2[:], True, True)
            sc2 = lp.tile([128, 2, 8], f32, tag='sc2', name='sc2')
            nc.scalar.activation(sc2[:].rearrange('p s g -> p (s g)'), pse2[:], actf.Sigmoid)
            nc.vector.tensor_tensor(
                hp2[:, :, 1:257].rearrange('p (g s) t -> p g s t', s=2),
                hp2[:, :, 1:257].rearrange('p (g s) t -> p g s t', s=2),
                sc2[:].rearrange('p s g -> p g s').unsqueeze(3).to_broadcast([128, 8, 2, 256]),
                alu.mult)

            # conv3 + SE3 + pool, chunked over sample pairs
            y3 = lp.tile([128, 2, BC], f32, tag='y3', name='y3')
            lf = [lp.tile([128, BC, 64], bf16, tag=f'lf{i}', name=f'lf{i}', bufs=1) for i in range(2)]
            hp2f = hp2[:].rearrange('p b t -> p (b t)')
            for mi in range(2):
                for b0 in range(0, BC, 2):
                    pl3 = psB.tile([128, 2, 256], f32, tag='pb', name='pb')
                    for bi in (b0, b0 + 1):
                        for dt in range(3):
                            mm(pl3[:, bi - b0, :], C['w3L'][:, dt, mi * 128:(mi + 1) * 128],
                               hp2f[:, bi * 258 + dt:bi * 258 + dt + 256], dt == 0, dt == 2)
                    r3 = lp.tile([128, 2, 256], f32, tag='r3', name='r3')
                    nc.scalar.activation(r3[:], pl3[:],
                                         actf.Relu, bias=C['lb3c'][:, mi, 0:1])
                    nc.vector.tensor_reduce(y3[:, mi, b0:b0 + 2], r3[:], X, alu.add)
                    nc.vector.tensor_reduce(lf[mi][:, b0:b0 + 2, :],
                                            r3[:].rearrange('p c (u q) -> p c u q', q=4),
                                            X, alu.max)
            pse3 = psA.tile([64, 16], f32, tag='pa', name='pa')
            for k in range(2):
                mmf(pse3[:], C['se3w1t'][:, k, :], y3[:, k, :], k == 0, k == 1)
            z3 = lp.tile([64, 16], f32, tag='z3', name='z3')
            nc.scalar.activation(z3[:], pse3[:], actf.Relu)
            sc3 = [lp.tile([128, BC], f32, tag=f'sc3_{i}', name=f'sc3_{i}') for i in range(2)]
            for mi in range(2):
                pse4 = psA.tile([128, 16], f32, tag='pa', name='pa')
                mmf(pse4[:], C['se3w2t'][:, mi * 128:(mi + 1) * 128], z3[:], True, True)
                nc.scalar.activation(sc3[mi][:], pse4[:], actf.Sigmoid)
            for mi in range(2):
                nc.gpsimd.tensor_tensor(lf[mi][:], lf[mi][:],
                                        sc3[mi][:].unsqueeze(2).to_broadcast([128, BC, 64]),
                                        alu.mult)
            # interp 64 -> 33
            li = [bp.tile([128, BC, T], f32, tag=f'li{i}', name=f'li{i}') for i in range(2)]
            pwl = psA.tile([128, T], f32, tag='pa', name='pwl')
            mmf(pwl[:], C['ones1'][:], C['wlo'][:], True, True)
            wlo128 = lp.tile([128, T], f32, tag='wlo128', name='wlo128', bufs=1)
            nc.vector.tensor_copy(wlo128[:], pwl[:])
            wbc_all = wlo128[:]
            for mi in range(2):
                for (a, n, lo0, st) in runs:
                    end = lo0 + (n - 1) * st + 1
                    lov = lf[mi][:, :, lo0:end:st]
                    hiv = lf[mi][:, :, lo0 + 1:end + 1:st]
                    dd = lp.tile([128, BC, T], f32, tag='dd', name='dd')
                    eng = nc.gpsimd if mi == 0 else nc.vector
                    eng.tensor_sub(dd[:, :, a:a + n], hiv, lov)
                    eng.tensor_tensor(dd[:, :, a:a + n], dd[:, :, a:a + n],
                                      wbc_all[:, a:a + n].unsqueeze(1).to_broadcast([128, BC, n]),
                                      alu.mult)
                    eng.tensor_add(li[mi][:, :, a:a + n].bitcast(f32r), dd[:, :, a:a + n], lov)

            lp_cm.__exit__(None, None, None)

            # ---------------- LSTM input precompute (transposed) ----------------
            # XsT[p, jj, t, b] = (x_t @ wih^T + bias)[b, jj*128+p], gate order (i,f,o,g)
            ct = [accF[0], accF[1], li[0], li[1]]
            xp_cm = tc.tile_pool(name='xp', bufs=2)
            xpp = xp_cm.__enter__()
            XsT = bp.tile([128, 8, T, 16], f32, tag='XsT', name='XsT')
            wft = [xpp.tile([128, 1024], bf16, tag=f'wft{k}', name=f'wft{k}', bufs=1)
                   for k in range(4)]
            xtT = [xpp.tile([128, T, 16], bf16, tag=f'xtT{k}', name=f'xtT{k}', bufs=1)
                   for k in range(4)]
            for k in range(4):
                nc.sync.dma_start(wft[k][:], d['wihft'][k * 128:(k + 1) * 128, :])
                nc.vector.tensor_copy(xtT[k][:], ct[k][:, :, :].transpose([0, 2, 1]))
            for jj in range(8):
                for (t0, tl) in ((0, 16), (16, 17)):
                    ps = psB.tile([128, tl * 16], f32, tag='pb', name='pb')
                    for k in range(4):
                        mm(ps[:], wft[k][:, jj * 128:(jj + 1) * 128],
                           xtT[k][:, t0:t0 + tl, :].rearrange('p t b -> p (t b)'),
                           k == 0, k == 3)
                    nc.vector.tensor_tensor(
                        XsT[:, jj, t0:t0 + tl, :].rearrange('p t b -> p (t b)').bitcast(f32r),
                        ps[:],
                        C['biasfT'][:, jj:jj + 1].to_broadcast([128, tl * 16]),
                        alu.add)

            # ---------------- backward cell (t=32, transposed) ----------------
            wbt = [xpp.tile([128, 1024], bf16, tag=f'wft{k}', name=f'wbt{k}', bufs=1)
                   for k in range(4)]
            for k in range(4):
                nc.sync.dma_start(wbt[k][:], d['wihbt'][k * 128:(k + 1) * 128, :])
            psb = psA.tile([128, 8, 16], f32, tag='pa', name='psb')
            for jj in range(8):
                for k in range(4):
                    mm(psb[:, jj, :], wbt[k][:, jj * 128:(jj + 1) * 128],
                       xtT[k][:, 32, :], k == 0, k == 3)
            gbT = wp1.tile([128, 8, 16], f32, tag='gbT', name='gbT')
            nc.vector.tensor_tensor(gbT[:], psb[:],
                                    C['biasbT'][:].unsqueeze(2).to_broadcast([128, 8, 16]),
                                    alu.add)
            sgb = wp1.tile([128, 8, 16], f32, tag='sgb', name='sgb')
            nc.scalar.activation(sgb[:, 0:6, :], gbT[:, 0:6, :], actf.Sigmoid)
            nc.scalar.activation(sgb[:, 6:8, :], gbT[:, 6:8, :], actf.Tanh)
            cbT = wp1.tile([128, 2, 16], f32, tag='cbT', name='cbT')
            nc.vector.tensor_tensor(cbT[:], sgb[:, 0:2, :], sgb[:, 6:8, :], alu.mult)
            tcb = wp1.tile([128, 2, 16], f32, tag='tcb', name='tcb')
            nc.scalar.activation(tcb[:], cbT[:], actf.Tanh)
            hbT = bp.tile([128, 2, 16], f32, tag='hbT', name='hbT')
            nc.vector.tensor_tensor(hbT[:], sgb[:, 4:6, :], tcb[:], alu.mult)
            xp_cm.__exit__(None, None, None)

            # ---------------- forward LSTM (33 steps, transposed) ----------------
            # gates live as [128 = j-chunk, jj, 16 = batch]; no per-step transposes.
            hT = None
            cT = None
            for t in range(T):
                psg = psB.tile([128, 8, 16], f32, tag='pb', name='psg')
                for jj in (6, 7, 0, 1, 2, 3, 4, 5):
                    mmr(psg[:, jj, :], C['i128'][:], XsT[:, jj, t, :], True, t == 0)
                    if t > 0:
                        for k in range(2):
                            mmr(psg[:, jj, :], C['whhft'][:, k, jj * 128:(jj + 1) * 128],
                                hT[:, k, :], False, k == 1)
                sg = wp.tile([128, 8, 16], f32, tag='lstm_sg', name='lstm_sg')
                nc.scalar.activation(sg[:, 0:6, :], psg[:, 0:6, :], actf.Sigmoid)
                nc.scalar.activation(sg[:, 6:8, :], psg[:, 6:8, :], actf.Tanh)
                t2 = wp.tile([128, 2, 16], f32, tag='lstm_t2', name='lstm_t2')
                nc.gpsimd.tensor_tensor(t2[:], sg[:, 0:2, :], sg[:, 6:8, :], alu.mult)
                cT_new = wp.tile([128, 2, 16], f32, tag='lstm_c', name='lstm_c')
                if t == 0:
                    nc.vector.tensor_copy(cT_new[:], t2[:])
                else:
                    t1 = wp.tile([128, 2, 16], f32, tag='lstm_t1', name='lstm_t1')
                    nc.vector.tensor_tensor(t1[:], sg[:, 2:4, :], cT[:], alu.mult)
                    nc.vector.tensor_tensor(cT_new[:], t1[:], t2[:], alu.add)
                cT = cT_new
                tct = wp.tile([128, 2, 16], f32, tag='lstm_tc', name='lstm_tc')
                nc.scalar.activation(tct[:], cT[:], actf.Tanh)
                hT_new = wp.tile([128, 2, 16], f32, tag='lstm_h', name='lstm_h')
                nc.vector.tensor_tensor(hT_new[:].bitcast(f32r), sg[:, 4:6, :], tct[:],
                                        alu.mult)
                hT = hT_new

            # ---------------- FFN head ----------------
            lastT = [hT[:, 0, :], hT[:, 1, :], hbT[:, 0, :], hbT[:, 1, :]]
            z = [wp1.tile([128, 16], f32, tag=f'z_{i}', name=f'z_{i}') for i in range(2)]
            for mi in range(2):
                pz = psA.tile([128, 16], f32, tag='pa', name='pa')
                for k in range(4):
                    mmf(pz[:], C['ffn1t'][:, k, mi * 128:(mi + 1) * 128], lastT[k],
                        k == 0, k == 3)
                nc.scalar.activation(z[mi][:], pz[:], actf.Relu,
                                     bias=C['ffnb1'][:, mi, 0:1])
            py = psA.tile([1, 16], f32, tag='pa', name='pa')
            for k in range(2):
                mmf(py[:], C['ffn2t'][:, k, :], z[k][:], k == 0, k == 1)
            yo = wp1.tile([1, 16], f32, tag='yo', name='yo')
            nc.scalar.activation(yo[:], py[:], actf.Copy, bias=float(ffn_b2_val))
            nc.sync.dma_start(yout[:].unsqueeze(0), yo[:])

    _split_tpb_waits(nc)
    return nc


def _split_tpb_waits(nc, max_waits=1):
    """This walrus build caps sync-waits per TPB instruction; hoist extras
    onto same-engine NoOps placed immediately before the instruction."""
    from concourse import mybir
    dma_ops = ('DMACopy', 'DMATranspose', 'TensorLoad', 'TensorSave')
    cnt = 0
    for f in nc.m.functions:
        for bb in f.blocks:
            out = []
            changed = False
            for inst in bb.instructions:
                si = inst.sync_info
                opc = getattr(inst, 'opcode', '') or type(inst).__name__
                if (si is not None and len(si.on_wait) > max_waits
                        and getattr(inst, 'engine', None) is not None):
                    waits = list(si.on_wait)
                    for w in waits[:-max_waits]:
                        nop = mybir.InstNoOp(name=f'{inst.name}-sw{cnt}', ins=[], outs=[])
                        cnt += 1
                        nop.engine = inst.engine
                        nop.sync_info = mybir.SyncInfo(on_wait=[w], on_update=[])
                        out.append(nop)
                    inst.sync_info = mybir.SyncInfo(on_wait=waits[-max_waits:],
                                                    on_update=list(si.on_update))
                    changed = True
                out.append(inst)
            if changed:
                bb.instructions = out
    return nc


def _host_prep(inputs):
    f = lambda x: np.ascontiguousarray(x, dtype=np.float32)
    n = np.arange(NFFT)
    win = 0.5 * (1.0 - np.cos(2.0 * np.pi * n / NFFT))
    k = np.arange(NF)
    ang = 2.0 * np.pi * np.outer(n, k) / NFFT
    gw1t = inputs['gate_w1'].T / T
    runs, lo_t, w_t = _interp_tables()
    w14 = np.zeros((14, 128), np.float32)
    for s in range(2):
        for jj in range(7):
            w14[s * 7 + jj, s * 64:(s + 1) * 64] = inputs['lw1'][:, 0, jj]
    w14r = np.concatenate([np.concatenate([w14, np.zeros((18, 128), np.float32)])] * 3)
    wt = np.transpose(inputs['lw2'], (1, 2, 0))  # [64ch, 5dt, 128oc]
    w2Lr = np.zeros((128, 3, 128), np.float32)
    for c in range(3):
        w2Lr[0:64, c, :] = wt[:, 2 * c, :]
        if c < 2:
            w2Lr[64:128, c, :] = wt[:, 2 * c + 1, :]
    shared = {
        'crw': f(win[:, None] * np.cos(ang)),
        'ciw': f(win[:, None] * np.sin(ang)),
        'gw1ta': f(gw1t[0:128]), 'gw1tb': f(gw1t[128:129]),
        'gb1c': f(inputs['gate_b1'][:, None]),
        'gw2t': f(inputs['gate_w2'].T), 'gb2c': f(inputs['gate_b2'][:, None]),
        'iota8': f(np.tile(np.arange(NE)[None, :], (BC, 1))),
        'ones1': f(np.ones((1, 128))),
        'zer': f(np.zeros((128, 70))),
        'sel8': f(np.concatenate([np.tile(v[:, None], (1, 128)) for v in np.eye(NE)], axis=1)),
        'w1p': np.asarray(np.transpose(inputs['exp_w1'], (0, 3, 2, 1)).reshape(NE, 645, 256), dtype=ml_dtypes.bfloat16),
        'w1b': f(inputs['exp_b1'].T),
        'w2p': np.asarray(np.transpose(inputs['exp_w2'], (0, 3, 2, 1)).reshape(NE, 768, 256), dtype=ml_dtypes.bfloat16),
        'w2b': f(inputs['exp_b2'].T),
        'w14r': np.asarray(w14r, dtype=ml_dtypes.bfloat16), 'lb1c': f(np.tile(inputs['lb1'], 2)[:, None]),
        'w2Lr': np.asarray(w2Lr, dtype=ml_dtypes.bfloat16),
        'lb2c': f(inputs['lb2'][:, None]),
        'se2w1t': f(inputs['se2_w1'].T / 1024.0), 'se2w2t': f(inputs['se2_w2'].T),
        'w3L': np.asarray(np.transpose(inputs['lw3'], (2, 1, 0)), dtype=ml_dtypes.bfloat16),
        'lb3c': f(inputs['lb3'][:, None]),
        'se3w1t': f(inputs['se3_w1'].T / 256.0), 'se3w2t': f(inputs['se3_w2'].T),
        'wlo': f(w_t[None, :]),
        'wihft': np.asarray(inputs['wih_f'].T[:, GPERM], dtype=ml_dtypes.bfloat16),
        'biasfT': f((inputs['bih_f'] + inputs['bhh_f'])[GPERM].reshape(NE, 128).T),
        'whhft': f(inputs['whh_f'].T[:, GPERM]),
        'wihbt': np.asarray(inputs['wih_b'].T[:, GPERM], dtype=ml_dtypes.bfloat16),
        'biasbT': f((inputs['bih_b'] + inputs['bhh_b'])[GPERM].reshape(NE, 128).T),
        'i16': f(np.eye(16)), 'i128': f(np.eye(128)),
        'ffn1t': f(inputs['ffn_w1'].T), 'ffnb1': f(inputs['ffn_b1'][:, None]),
        'ffn2t': f(inputs['ffn_w2'].T),
    }
    xp = np.pad(inputs['x_continuum'], ((0, 0), (NFFT // 2, NFFT // 2)), mode='reflect')
    s0, s1 = xp.strides
    frames = np.lib.stride_tricks.as_strided(xp, (B, T, NFFT), (s0, 128 * s1, s1))
    xnp = np.pad(inputs['x_normalized'], ((0, 0), (3, 3 + 10)))
    in_maps = []
    for c in range(N_CORES):
        m = dict(shared)
        fr = frames[c * BC:(c + 1) * BC]
        m['framesT'] = f(np.transpose(fr, (2, 0, 1)).reshape(NFFT, BC * T))
        xc = xnp[c * BC:(c + 1) * BC]  # [16, 4112]
        x7b = np.zeros((3, 96, 8, 520), np.float32)
        for u in range(8):
            for s in range(2):
                for jj in range(7):
                    r = (u % 3) * 32 + s * 7 + jj
                    for gg in range(8):
                        x7b[u // 3, r, gg, :] = xc[gg * 2 + s, u * 512 + jj:u * 512 + jj + 520]
        m['x7b'] = np.asarray(x7b, dtype=ml_dtypes.bfloat16)
        in_maps.append(m)
    return in_maps


def _apply_tile_patch():
    from concourse import tile, mybir
    from concourse.vector_clock import ScopedClock

    def _drain_split(self, tick_clock, wait_clock):
        nc2 = self.nc
        di = nc2.sync.drain()
        wait_clock.add_sem_waits(di.ins, ScopedClock({None: tick_clock.global_clock}))
        si = di.ins.sync_info
        if si is not None and len(si.on_wait) > 1:
            waits = list(si.on_wait)
            ups = list(si.on_update)
            di.ins.sync_info = mybir.SyncInfo(on_wait=waits[:1], on_update=[])
            for kk, w in enumerate(waits[1:]):
                extra = nc2.sync.drain()
                extra.ins.sync_info = mybir.SyncInfo(
                    on_wait=[w], on_update=ups if kk == len(waits) - 2 else [])
        nc2.all_engine_barrier()
        assert self.sems is not None
        popped = nc2._tile_sem_poison_stack.pop()
        assert popped is self._sem_poison
        nc2.clear_and_free_semaphores(list(self.sems.allocated().values()))
        nc2.all_engine_barrier()

    tile.TileContext._drain_and_barrier = _drain_split


def kernel(**inputs):
    global _cache
    if 'nc' not in _cache:
        _apply_tile_patch()
        _cache['nc'] = _build(float(np.asarray(inputs['ffn_b2']).reshape(-1)[0]))
    from concourse.bass_utils import run_bass_kernel_spmd
    in_maps = _host_prep(inputs)
    res = run_bass_kernel_spmd(_cache['nc'], in_maps, list(range(N_CORES)))
    out = np.concatenate([res.results[c]['yout'] for c in range(N_CORES)])
    return out[:, None].astype(np.float32)



# revision 26
# speedup vs baseline: 1.0001x; 1.0001x over previous
"""DualBranchMoENet on Trainium2 — 8-core data-parallel (16 samples/core).

Channels live on SBUF partitions, (batch, time) on the free dim. Heavy
matmuls run fp32r (1 cyc/row at N>=256). Convolutions contract (cin, tap)
on the PE partition axis via shifted access patterns; only expert conv1
(129 ch x 5 taps) materialises an im2col stack. The LSTM keeps its hidden
state transposed ([256c, 16b]) so h @ whh^T needs no input transpose;
h is re-transposed once per step on the PE. The backward LSTM output
hb[T-1] equals ONE cell evaluated at t=32 from the zero state.
"""
import sys
sys.path.insert(0, '/opt/trn_rl_repo')
import numpy as np
import ml_dtypes

N_CORES = 8
B = 128
BC = B // N_CORES
L = 4096
NFFT = 256
NF = 129
T = 33
NE = 8

_cache = {}

# LSTM gate reorder (torch i,f,g,o) -> (i,f,o,g) so the three sigmoid gates
# are contiguous and fuse into one activation instruction.
GPERM = np.concatenate([np.arange(0, 512), np.arange(768, 1024), np.arange(512, 768)])


def _interp_tables():
    coords = np.clip((np.arange(T) + 0.5) * (64.0 / T) - 0.5, 0.0, 63.0)
    lo = np.floor(coords).astype(np.int64)
    w = coords - lo
    runs = []
    a = 0
    while a < T:
        b = a + 1
        if b < T:
            step = lo[a + 1] - lo[a]
            while b < T and lo[b] - lo[b - 1] == step:
                b += 1
        runs.append((a, b - a, int(lo[a]), int(lo[a + 1] - lo[a]) if b - a >= 2 else 1))
        a = b
    return runs, lo, w


def _build(ffn_b2_val):
    from concourse import bass, tile, mybir
    from concourse.mybir import AluOpType as alu
    from concourse.mybir import ActivationFunctionType as actf

    f32 = mybir.dt.float32
    f32r = mybir.dt.float32r
    bf16 = mybir.dt.bfloat16
    X = mybir.AxisListType.X

    BF16_IN = {'w1p', 'w2p', 'x7b', 'w14r', 'w2Lr', 'w3L', 'wihft', 'wihbt'}
    nc = bass.Bass()
    inp = lambda name, shape: nc.declare_dram_parameter(
        name, list(shape), bf16 if name in BF16_IN else f32, isOutput=False)

    d = {}
    for name, shape in [
        ('framesT', [NFFT, BC * T]), ('x7b', [3, 96, 8, 520]),
        ('crw', [NFFT, NF]), ('ciw', [NFFT, NF]),
        ('gw1ta', [128, 128]), ('gw1tb', [1, 128]), ('gb1c', [128, 1]),
        ('gw2t', [128, NE]), ('gb2c', [NE, 1]), ('iota8', [BC, NE]), ('ones1', [1, 128]), ('sel8', [NE, NE * 128]), ('zer', [128, 70]),
        ('w1p', [NE, 645, 256]), ('w1b', [256, NE]),
        ('w2p', [NE, 768, 256]), ('w2b', [256, NE]),
        ('w14r', [96, 128]), ('lb1c', [128, 1]),
        ('w2Lr', [128, 3, 128]), ('lb2c', [128, 1]),
        ('se2w1t', [128, 32]), ('se2w2t', [32, 128]),
        ('w3L', [3, 128, 256]), ('lb3c', [256, 1]),
        ('se3w1t', [256, 64]), ('se3w2t', [64, 256]),
        ('wlo', [1, T]),
        ('wihft', [512, 1024]), ('biasfT', [128, NE]), ('whhft', [256, 1024]),
        ('wihbt', [512, 1024]), ('biasbT', [128, NE]),
        ('i16', [16, 16]), ('i128', [128, 128]),
        ('ffn1t', [512, 256]), ('ffnb1', [256, 1]), ('ffn2t', [256, 1]),
    ]:
        d[name] = inp(name, shape)
    yout = nc.declare_dram_parameter('yout', [BC], f32, isOutput=True)

    runs, lo_t, w_t = _interp_tables()

    def mm(out, lhsT, rhs, start, stop):
        nc.tensor.matmul(out, lhsT, rhs, start=start, stop=stop)

    def mmr(out, lhsT, rhs, start, stop):
        nc.tensor.matmul(out, lhsT.bitcast(f32r), rhs.bitcast(f32r),
                         start=start, stop=stop)

    def mmf(out, lhsT, rhs, start, stop):
        nc.tensor.matmul(out, lhsT, rhs, start=start, stop=stop)

    with tile.TileContext(nc, num_cores=N_CORES) as tc:
        with (
            tc.tile_pool(name='const', bufs=1) as cp,
            tc.tile_pool(name='work', bufs=2) as wp,
            tc.tile_pool(name='one', bufs=1) as wp1,
            tc.tile_pool(name='big', bufs=1) as bp,
            tc.tile_pool(name='psA', bufs=2, space='PSUM') as psA,
            tc.tile_pool(name='psB', bufs=4, space='PSUM') as psB,
            tc.tile_pool(name='psC', bufs=2, space='PSUM') as psC,
        ):
            C = {}
            for name, shape, rr in [
                ('crw', [128, 2, NF], '(k p) m -> p k m'),
                ('ciw', [128, 2, NF], '(k p) m -> p k m'),
                ('gw1ta', [128, 128], None), ('gw1tb', [1, 128], None),
                ('gb1c', [128, 1], None),
                ('gw2t', [128, NE], None), ('gb2c', [NE, 1], None),
                ('iota8', [BC, NE], None),
                ('ones1', [1, 128], None),
                ('sel8', [NE, NE * 128], None),
                ('w1b', [128, 2, NE], '(k p) m -> p k m'),
                ('w2b', [128, 2, NE], '(k p) m -> p k m'),
                ('lb1c', [128, 1], None),
                ('lb2c', [128, 1], None),
                ('se2w1t', [128, 32], None), ('se2w2t', [32, 128], None),
                ('lb3c', [128, 2, 1], '(k p) m -> p k m'),
                ('se3w1t', [128, 2, 64], '(k p) m -> p k m'),
                ('se3w2t', [64, 256], None),
                ('wlo', [1, T], None),
                ('biasfT', [128, NE], None),
                ('whhft', [128, 2, 1024], '(k p) m -> p k m'),
                ('biasbT', [128, NE], None),
                ('i16', [16, 16], None), ('i128', [128, 128], None),
                ('ffn1t', [128, 4, 256], '(k p) m -> p k m'),
                ('ffnb1', [128, 2, 1], '(k p) m -> p k m'),
                ('ffn2t', [128, 2, 1], '(k p) m -> p k m'),
            ]:
                t = cp.tile(shape, f32, tag=name)
                src = d[name][:]
                if rr:
                    src = src.rearrange(rr, p=128)
                if name in ('crw', 'ciw', 'whhft', 'i128'):
                    nc.sync.dma_start(t[:].bitcast(f32r), src.bitcast(f32r))
                else:
                    nc.sync.dma_start(t[:], src)
                C[name] = t
            t = cp.tile([96, 128], bf16, tag='w14r')
            nc.sync.dma_start(t[:], d['w14r'][:])
            C['w14r'] = t
            t = cp.tile([128, 3, 128], bf16, tag='w2Lr')
            nc.sync.dma_start(t[:], d['w2Lr'][:])
            C['w2Lr'] = t
            t = cp.tile([128, 3, 256], bf16, tag='w3L')
            nc.sync.dma_start(t[:], d['w3L'][:].rearrange('d k m -> k d m'))
            C['w3L'] = t

            # ---------------- STFT magnitude ----------------
            lp_cm = tc.tile_pool(name='lp', bufs=2)
            lp = lp_cm.__enter__()
            ep_cm = tc.tile_pool(name='ep', bufs=2)
            ep = ep_cm.__enter__()
            c_fr = ep.tile([128, 2, BC * T], f32, tag='framesT', name='framesT', bufs=1)
            nc.sync.dma_start(c_fr[:].bitcast(f32r), d['framesT'][:].rearrange('(k p) m -> p k m', p=128).bitcast(f32r))
            C['framesT'] = c_fr
            magA = ep.tile([128, BC, T + 4], f32, tag='magA', name='magA', bufs=1)
            magB = ep.tile([1, BC, T + 4], f32, tag='magB', name='magB', bufs=1)
            nc.vector.memset(magA[:], 0.0)
            nc.vector.memset(magB[:], 0.0)
            NB2 = BC * T // 2
            for m0, mn, magX in [(0, 128, magA), (128, 1, magB)]:
                sqr = ep.tile([mn, BC * T], f32, tag=f'sqr{m0}', name=f'sqr{m0}', bufs=1)
                sqi = ep.tile([mn, BC * T], f32, tag=f'sqi{m0}', name=f'sqi{m0}', bufs=1)
                for ni in range(2):
                    pre = psA.tile([mn, NB2], f32, tag='pa', name='pa')
                    pim = psA.tile([mn, NB2], f32, tag='pa', name='pa')
                    for k in range(2):
                        co = slice(ni * NB2, (ni + 1) * NB2)
                        mmr(pre[:], C['crw'][:, k, m0:m0 + mn], C['framesT'][:, k, co], k == 0, k == 1)
                        mmr(pim[:], C['ciw'][:, k, m0:m0 + mn], C['framesT'][:, k, co], k == 0, k == 1)
                    nc.scalar.square(sqr[:, ni * NB2:(ni + 1) * NB2], pre[:])
                    nc.scalar.square(sqi[:, ni * NB2:(ni + 1) * NB2], pim[:])
                nc.vector.tensor_add(sqr[:], sqr[:], sqi[:])
                nc.scalar.sqrt(magX[0:mn, :, 2:2 + T],
                               sqr[:].rearrange('p (b t) -> p b t', b=BC))

            magAb = ep.tile([128, BC, T + 4], bf16, tag='magAb', name='magAb', bufs=1)
            magBb = ep.tile([1, BC, T + 4], bf16, tag='magBb', name='magBb', bufs=1)
            nc.scalar.activation(magAb[:], magA[:], actf.Copy)
            nc.scalar.activation(magBb[:], magB[:], actf.Copy)

            # ---------------- gating (fp32 matmuls) ----------------
            pooledA = ep.tile([128, BC], f32, tag='pooledA', name='pooledA')
            pooledB = ep.tile([1, BC], f32, tag='pooledB', name='pooledB')
            nc.vector.tensor_reduce(pooledA[:], magA[:, :, 2:2 + T], X, alu.add)
            nc.vector.tensor_reduce(pooledB[:], magB[:, :, 2:2 + T], X, alu.add)
            pg1 = psA.tile([128, BC], f32, tag='pa', name='pa')
            mmf(pg1[:], C['gw1ta'][:], pooledA[:], True, False)
            mmf(pg1[:], C['gw1tb'][:], pooledB[:], False, True)
            gh = ep.tile([128, BC], f32, tag='gh', name='gh')
            nc.scalar.activation(gh[:], pg1[:], actf.Relu, bias=C['gb1c'][:, 0:1])
            pg2 = psA.tile([NE, BC], f32, tag='pa', name='pa')
            mmf(pg2[:], C['gw2t'][:], gh[:], True, True)
            logitsT = ep.tile([NE, BC], f32, tag='logitsT', name='logitsT')
            nc.vector.tensor_tensor(logitsT[:], pg2[:],
                                    C['gb2c'][:, 0:1].to_broadcast([NE, BC]), alu.add)
            plg = psA.tile([BC, NE], f32, tag='pa', name='pa')
            nc.tensor.transpose(plg[:], logitsT[:], C['i16'][0:NE, 0:NE])
            lg = ep.tile([BC, NE], f32, tag='lg', name='lg')
            nc.vector.tensor_copy(lg[:], plg[:])
            iob = C['iota8'][:]

            def small(tag, shape=(BC, NE)):
                return ep.tile(list(shape), f32, tag=tag, name=tag)

            m1 = small('m1', (BC, 1))
            nc.vector.tensor_reduce(m1[:], lg[:], X, alu.max)
            eq1 = small('eq1')
            nc.vector.tensor_tensor(eq1[:], lg[:], m1[:].to_broadcast([BC, NE]), alu.is_equal)
            l2 = small('l2')
            nc.vector.scalar_tensor_tensor(l2[:], eq1[:], -1e30, lg[:], alu.mult, alu.add)
            m2 = small('m2', (BC, 1))
            nc.vector.tensor_reduce(m2[:], l2[:], X, alu.max)
            it1 = small('it1')
            nc.vector.tensor_tensor(it1[:], eq1[:], iob, alu.mult)
            idx1 = small('idx1', (BC, 1))
            nc.vector.tensor_reduce(idx1[:], it1[:], X, alu.max)
            eq2 = small('eq2')
            nc.vector.tensor_tensor(eq2[:], l2[:], m2[:].to_broadcast([BC, NE]), alu.is_equal)
            it2 = small('it2')
            nc.vector.tensor_tensor(it2[:], eq2[:], iob, alu.mult)
            idx2 = small('idx2', (BC, 1))
            nc.vector.tensor_reduce(idx2[:], it2[:], X, alu.max)
            dm = small('dm', (BC, 1))
            nc.vector.tensor_sub(dm[:], m1[:], m2[:])
            g1 = small('g1', (BC, 1))
            nc.scalar.activation(g1[:], dm[:], actf.Sigmoid)
            g2 = small('g2', (BC, 1))
            nc.vector.tensor_scalar(g2[:], g1[:], -1.0, 1.0, alu.mult, alu.add)
            eA = small('eA')
            nc.vector.tensor_tensor(eA[:], idx1[:].to_broadcast([BC, NE]), iob, alu.is_equal)
            eB = small('eB')
            nc.vector.tensor_tensor(eB[:], idx2[:].to_broadcast([BC, NE]), iob, alu.is_equal)
            tA = small('tA')
            nc.vector.tensor_tensor(tA[:], eA[:], g1[:].to_broadcast([BC, NE]), alu.mult)
            tB = small('tB')
            nc.vector.tensor_tensor(tB[:], eB[:], g2[:].to_broadcast([BC, NE]), alu.mult)
            W8 = small('W8')
            nc.vector.tensor_add(W8[:], tA[:], tB[:])
            pW8T = psA.tile([NE, BC], f32, tag='pa', name='pa')
            nc.tensor.transpose(pW8T[:], W8[:], C['i16'][:])
            W8T = ep.tile([NE, BC], f32, tag='W8T', name='W8T')
            nc.vector.tensor_copy(W8T[:], pW8T[:])

            # ---------------- line conv1 (emitted early: overlaps expert DMA) ----
            h1 = lp.tile([128, 8, 1028], bf16, tag='h1', name='h1', bufs=1)
            nc.gpsimd.memset(h1[:, :, 0:2], 0.0)
            nc.gpsimd.memset(h1[:, :, 1026:1028], 0.0)
            x7 = [ep.tile([96, 8, 520], bf16, tag=f'x7_{h}', name=f'x7_{h}', bufs=1)
                  for h in range(3)]
            for h in range(3):
                nc.sync.dma_start(x7[h][:], d['x7b'][h])
            for u in range(8):
                ub = (u % 3) * 32
                for gg in range(8):
                    pl1 = psC.tile([128, 512], f32, tag='pc', name='pc')
                    mm(pl1[:], C['w14r'][ub:ub + 32, :],
                       x7[u // 3][ub:ub + 32, gg, 0:512], True, True)
                    o0 = 2 + u * 128
                    nc.vector.tensor_reduce(h1[:, gg, o0:o0 + 128],
                                            pl1[:].rearrange('p (t q) -> p t q', q=4),
                                            X, alu.max)
            nc.scalar.activation(h1[:, :, 2:1026], h1[:, :, 2:1026], actf.Relu,
                                 bias=C['lb1c'][:, 0:1])

            # ---------------- experts (dense, weighted sum) ----------------
            imt = [ep.tile([128 if k < 5 else 5, BC, T], bf16, tag=f'im1_{k}', name=f'im1_{k}', bufs=1)
                   for k in range(6)]
            for dt in range(5):
                pos = dt * NF
                done = 0
                while done < NF:
                    k, r = divmod(pos + done, 128)
                    if done < 128:
                        n = min(128 - r, NF - done, 128 - done)
                        nc.sync.dma_start(imt[k][r:r + n],
                                          magAb[done:done + n, :, dt:dt + T])
                    else:
                        n = 1
                        nc.sync.dma_start(imt[k][r:r + 1], magBb[0:1, :, dt:dt + T])
                    done += n
            accF = [bp.tile([128, BC, T], f32, tag=f'accF{i}', name=f'accF{i}') for i in range(2)]
            H = BC // 2
            for e in range(NE):
                w1s = ep.tile([128, 6, 256], bf16, tag='w1s', name='w1s')
                nc.sync.dma_start(w1s[:, 0:5, :],
                                  d['w1p'][e, 0:640, :].rearrange('(k p) m -> p k m', p=128))
                nc.sync.dma_start(w1s[0:5, 5, :], d['w1p'][e, 640:645, :])
                he = [ep.tile([128, BC * (T + 2) + 2], bf16, tag=f'he{i}', name=f'he{i}') for i in range(2)]
                for i in range(2):
                    hv = he[i][:, 0:BC * (T + 2)].rearrange('p (b t) -> p b t', t=T + 2)
                    nc.gpsimd.memset(hv[:, :, 0:1], 0.0)
                    nc.gpsimd.memset(hv[:, :, T + 1:T + 2], 0.0)
                    nc.gpsimd.memset(he[i][:, BC * (T + 2):], 0.0)
                for mi in range(2):
                    for ni in range(2):
                        pe1 = psB.tile([128, H * T], f32, tag='pb', name='pb')
                        for k in range(6):
                            kn = 128 if k < 5 else 5
                            mm(pe1[:], w1s[0:kn, k, mi * 128:(mi + 1) * 128],
                               imt[k][:].rearrange('p b t -> p (b t)')[:, ni * H * T:(ni + 1) * H * T],
                               k == 0, k == 5)
                        nc.scalar.activation(he[mi][:, 0:BC * (T + 2)].rearrange('p (b t) -> p b t', t=T + 2)[:, ni * H:(ni + 1) * H, 1:1 + T],
                                             pe1[:].rearrange('p (b t) -> p b t', t=T),
                                             actf.Relu, bias=C['w1b'][:, mi, e:e + 1])
                w2s = ep.tile([128, 6, 256], bf16, tag='w2s', name='w2s')
                nc.sync.dma_start(w2s[:], d['w2p'][e].rearrange('(k p) m -> p k m', p=128))
                eo = [ep.tile([128, BC, T], f32, tag=f'eo{i}', name=f'eo{i}', bufs=1) for i in range(2)]
                W2 = T + 2
                for mi in range(2):
                    for bi in range(2):
                        pe2 = psB.tile([128, H * W2], f32, tag='pb', name='pb')
                        for k in range(6):
                            dt, ch = divmod(k, 2)
                            mm(pe2[:], w2s[:, k, mi * 128:(mi + 1) * 128],
                               he[ch][:, bi * H * W2 + dt:bi * H * W2 + dt + H * W2],
                               k == 0, k == 5)
                        nc.scalar.activation(eo[mi][:, bi * H:(bi + 1) * H, :],
                                             pe2[:].rearrange('p (b t) -> p b t', t=W2)[:, :, 0:T],
                                             actf.Relu, bias=C['w2b'][:, mi, e:e + 1])
                pwe = psA.tile([128, BC], f32, tag='pa', name='pwe')
                mmf(pwe[:], C['sel8'][:, e * 128:(e + 1) * 128], W8T[:], True, True)
                wE = ep.tile([128, BC], f32, tag='wE', name='wE')
                nc.vector.tensor_copy(wE[:], pwe[:])
                wbc = wE[:].unsqueeze(2).to_broadcast([128, BC, T])
                for mi in range(2):
                    if e == 0:
                        nc.vector.tensor_tensor(accF[mi][:].bitcast(f32r), eo[mi][:], wbc, alu.mult)
                    else:
                        eow = ep.tile([128, BC, T], f32, tag='eow', name='eow')
                        nc.vector.tensor_tensor(eow[:], eo[mi][:], wbc, alu.mult)
                        nc.vector.tensor_add(accF[mi][:].bitcast(f32r), accF[mi][:], eow[:])

            ep_cm.__exit__(None, None, None)

            # ---------------- line branch ----------------

            hp2 = lp.tile([128, BC, 258], bf16, tag='hp2', name='hp2', bufs=1)
            nc.gpsimd.memset(hp2[:, :, 0:1], 0.0)
            nc.gpsimd.memset(hp2[:, :, 257:258], 0.0)
            hp2v = hp2[:, :, 1:257].rearrange('p (g s) t -> p s g t', s=2)
            seacc = lp.tile([128, 2, 8, 2], f32, tag='seacc', name='seacc', bufs=1)
            hs = [lp.tile([128, 8, 1028], bf16, tag=f'hs{s}', name=f'hs{s}', bufs=1)
                  for s in range(2)]
            for s in range(2):
                nc.sync.dma_start(hs[s][0:64], h1[s * 64:(s + 1) * 64])
                nc.sync.dma_start(hs[s][64:128, :, 0:1027], h1[s * 64:(s + 1) * 64, :, 1:1028])
            for s in range(2):
                for gg in range(8):
                    for uh in range(2):
                        pl2 = psB.tile([128, 512], f32, tag='pb', name='pb')
                        base = uh * 512
                        mm(pl2[:], C['w2Lr'][:, 0, :], hs[s][:, gg, base:base + 512],
                           True, False)
                        mm(pl2[:], C['w2Lr'][:, 1, :], hs[s][:, gg, base + 2:base + 514],
                           False, False)
                        mm(pl2[:], C['w2Lr'][0:64, 2, :], hs[s][0:64, gg, base + 4:base + 516],
                           False, True)
                        r2 = lp.tile([128, 512], f32, tag='r2', name='r2')
                        nc.scalar.activation(r2[:], pl2[:], actf.Relu, bias=C['lb2c'][:, 0:1],
                                             accum_out=seacc[:, s, gg, uh:uh + 1])
                        nc.vector.tensor_reduce(hp2v[:, s, gg, uh * 128:uh * 128 + 128],
                                                r2[:].rearrange('p (w q) -> p w q', q=4),
                                                X, alu.max)
            seY = lp.tile([128, 2, 8], f32, tag='seY', name='seY')
            nc.vector.tensor_reduce(seY[:], seacc[:], X, alu.add)
            pse1 = psA.tile([32, 16], f32, tag='pa', name='pa')
            mmf(pse1[:], C['se2w1t'][:], seY[:].rearrange('p s g -> p (s g)'), True, True)
            z2 = lp.tile([32, 16], f32, tag='z2', name='z2')
            nc.scalar.activation(z2[:], pse1[:], actf.Relu)
            pse2 = psA.tile([128, 16], f32, tag='pa', name='pa')
            mmf(pse2[:], C['se2w2t'][:], z2[:], True, True)
            sc2 = lp.tile([128, 2, 8], f32, tag='sc2', name='sc2')
            nc.scalar.activation(sc2[:].rearrange('p s g -> p (s g)'), pse2[:], actf.Sigmoid)
            nc.vector.tensor_tensor(
                hp2[:, :, 1:257].rearrange('p (g s) t -> p g s t', s=2),
                hp2[:, :, 1:257].rearrange('p (g s) t -> p g s t', s=2),
                sc2[:].rearrange('p s g -> p g s').unsqueeze(3).to_broadcast([128, 8, 2, 256]),
                alu.mult)

            # conv3 + SE3 + pool, chunked over sample pairs
            y3 = lp.tile([128, 2, BC], f32, tag='y3', name='y3')
            lf = [lp.tile([128, BC, 64], bf16, tag=f'lf{i}', name=f'lf{i}', bufs=1) for i in range(2)]
            hp2f = hp2[:].rearrange('p b t -> p (b t)')
            for mi in range(2):
                for b0 in range(0, BC, 2):
                    pl3 = psB.tile([128, 2, 256], f32, tag='pb', name='pb')
                    for bi in (b0, b0 + 1):
                        for dt in range(3):
                            mm(pl3[:, bi - b0, :], C['w3L'][:, dt, mi * 128:(mi + 1) * 128],
                               hp2f[:, bi * 258 + dt:bi * 258 + dt + 256], dt == 0, dt == 2)
                    r3 = lp.tile([128, 2, 256], f32, tag='r3', name='r3')
                    nc.scalar.activation(r3[:], pl3[:],
                                         actf.Relu, bias=C['lb3c'][:, mi, 0:1])
                    nc.vector.tensor_reduce(y3[:, mi, b0:b0 + 2], r3[:], X, alu.add)
                    nc.vector.tensor_reduce(lf[mi][:, b0:b0 + 2, :],
                                            r3[:].rearrange('p c (u q) -> p c u q', q=4),
                                            X, alu.max)
            pse3 = psA.tile([64, 16], f32, tag='pa', name='pa')
            for k in range(2):
                mmf(pse3[:], C['se3w1t'][:, k, :], y3[:, k, :], k == 0, k == 1)
            z3 = lp.tile([64, 16], f32, tag='z3', name='z3')
            nc.scalar.activation(z3[:], pse3[:], actf.Relu)
            sc3 = [lp.tile([128, BC], f32, tag=f'sc3_{i}', name=f'sc3_{i}') for i in range(2)]
            for mi in range(2):
                pse4 = psA.tile([128, 16], f32, tag='pa', name='pa')
                mmf(pse4[:], C['se3w2t'][:, mi * 128:(mi + 1) * 128], z3[:], True, True)
                nc.scalar.activation(sc3[mi][:], pse4[:], actf.Sigmoid)
            for mi in range(2):
                nc.gpsimd.tensor_tensor(lf[mi][:], lf[mi][:],
                                        sc3[mi][:].unsqueeze(2).to_broadcast([128, BC, 64]),
                                        alu.mult)
            # interp 64 -> 33
            li = [bp.tile([128, BC, T], f32, tag=f'li{i}', name=f'li{i}') for i in range(2)]
            pwl = psA.tile([128, T], f32, tag='pa', name='pwl')
            mmf(pwl[:], C['ones1'][:], C['wlo'][:], True, True)
            wlo128 = lp.tile([128, T], f32, tag='wlo128', name='wlo128', bufs=1)
            nc.vector.tensor_copy(wlo128[:], pwl[:])
            wbc_all = wlo128[:]
            for mi in range(2):
                for (a, n, lo0, st) in runs:
                    end = lo0 + (n - 1) * st + 1
                    lov = lf[mi][:, :, lo0:end:st]
                    hiv = lf[mi][:, :, lo0 + 1:end + 1:st]
                    dd = lp.tile([128, BC, T], f32, tag='dd', name='dd')
                    eng = nc.gpsimd if mi == 0 else nc.vector
                    eng.tensor_sub(dd[:, :, a:a + n], hiv, lov)
                    eng.tensor_tensor(dd[:, :, a:a + n], dd[:, :, a:a + n],
                                      wbc_all[:, a:a + n].unsqueeze(1).to_broadcast([128, BC, n]),
                                      alu.mult)
                    eng.tensor_add(li[mi][:, :, a:a + n].bitcast(f32r), dd[:, :, a:a + n], lov)

            lp_cm.__exit__(None, None, None)

            # ---------------- LSTM input precompute (transposed) ----------------
            # XsT[p, jj, t, b] = (x_t @ wih^T + bias)[b, jj*128+p], gate order (i,f,o,g)
            ct = [accF[0], accF[1], li[0], li[1]]
            xp_cm = tc.tile_pool(name='xp', bufs=2)
            xpp = xp_cm.__enter__()
            XsT = bp.tile([128, 8, T, 16], f32, tag='XsT', name='XsT')
            wft = [xpp.tile([128, 1024], bf16, tag=f'wft{k}', name=f'wft{k}', bufs=1)
                   for k in range(4)]
            xtT = [xpp.tile([128, T, 16], bf16, tag=f'xtT{k}', name=f'xtT{k}', bufs=1)
                   for k in range(4)]
            for k in range(4):
                nc.sync.dma_start(wft[k][:], d['wihft'][k * 128:(k + 1) * 128, :])
                nc.vector.tensor_copy(xtT[k][:], ct[k][:, :, :].transpose([0, 2, 1]))
            for jj in range(8):
                for (t0, tl) in ((0, 16), (16, 17)):
                    ps = psB.tile([128, tl * 16], f32, tag='pb', name='pb')
                    for k in range(4):
                        mm(ps[:], wft[k][:, jj * 128:(jj + 1) * 128],
                           xtT[k][:, t0:t0 + tl, :].rearrange('p t b -> p (t b)'),
                           k == 0, k == 3)
                    nc.vector.tensor_tensor(
                        XsT[:, jj, t0:t0 + tl, :].rearrange('p t b -> p (t b)').bitcast(f32r),
                        ps[:],
                        C['biasfT'][:, jj:jj + 1].to_broadcast([128, tl * 16]),
                        alu.add)

            # ---------------- backward cell (t=32, transposed) ----------------
            wbt = [xpp.tile([128, 1024], bf16, tag=f'wft{k}', name=f'wbt{k}', bufs=1)
                   for k in range(4)]
            for k in range(4):
                nc.sync.dma_start(wbt[k][:], d['wihbt'][k * 128:(k + 1) * 128, :])
            psb = psA.tile([128, 8, 16], f32, tag='pa', name='psb')
            for jj in range(8):
                for k in range(4):
                    mm(psb[:, jj, :], wbt[k][:, jj * 128:(jj + 1) * 128],
                       xtT[k][:, 32, :], k == 0, k == 3)
            gbT = wp1.tile([128, 8, 16], f32, tag='gbT', name='gbT')
            nc.vector.tensor_tensor(gbT[:], psb[:],
                                    C['biasbT'][:].unsqueeze(2).to_broadcast([128, 8, 16]),
                                    alu.add)
            sgb = wp1.tile([128, 8, 16], f32, tag='sgb', name='sgb')
            nc.scalar.activation(sgb[:, 0:6, :], gbT[:, 0:6, :], actf.Sigmoid)
            nc.scalar.activation(sgb[:, 6:8, :], gbT[:, 6:8, :], actf.Tanh)
            cbT = wp1.tile([128, 2, 16], f32, tag='cbT', name='cbT')
            nc.vector.tensor_tensor(cbT[:], sgb[:, 0:2, :], sgb[:, 6:8, :], alu.mult)
            tcb = wp1.tile([128, 2, 16], f32, tag='tcb', name='tcb')
            nc.scalar.activation(tcb[:], cbT[:], actf.Tanh)
            hbT = bp.tile([128, 2, 16], f32, tag='hbT', name='hbT')
            nc.vector.tensor_tensor(hbT[:], sgb[:, 4:6, :], tcb[:], alu.mult)
            xp_cm.__exit__(None, None, None)

            # ---------------- forward LSTM (33 steps, transposed) ----------------
            # gates live as [128 = j-chunk, jj, 16 = batch]; no per-step transposes.
            hT = None
            cT = None
            for t in range(T):
                psg = psB.tile([128, 8, 16], f32, tag='pb', name='psg')
                for jj in range(8):
                    mmr(psg[:, jj, :], C['i128'][:], XsT[:, jj, t, :], True, t == 0)
                    if t > 0:
                        for k in range(2):
                            mmr(psg[:, jj, :], C['whhft'][:, k, jj * 128:(jj + 1) * 128],
                                hT[:, k, :], False, k == 1)
                sg = wp.tile([128, 8, 16], f32, tag='lstm_sg', name='lstm_sg')
                nc.scalar.activation(sg[:, 0:6, :], psg[:, 0:6, :], actf.Sigmoid)
                nc.scalar.activation(sg[:, 6:8, :], psg[:, 6:8, :], actf.Tanh)
                t2 = wp.tile([128, 2, 16], f32, tag='lstm_t2', name='lstm_t2')
                nc.gpsimd.tensor_tensor(t2[:], sg[:, 0:2, :], sg[:, 6:8, :], alu.mult)
                cT_new = wp.tile([128, 2, 16], f32, tag='lstm_c', name='lstm_c')
                if t == 0:
                    nc.vector.tensor_copy(cT_new[:], t2[:])
                else:
                    t1 = wp.tile([128, 2, 16], f32, tag='lstm_t1', name='lstm_t1')
                    nc.vector.tensor_tensor(t1[:], sg[:, 2:4, :], cT[:], alu.mult)
                    nc.vector.tensor_tensor(cT_new[:], t1[:], t2[:], alu.add)
                cT = cT_new
                tct = wp.tile([128, 2, 16], f32, tag='lstm_tc', name='lstm_tc')
                nc.scalar.activation(tct[:], cT[:], actf.Tanh)
                hT_new = wp.tile([128, 2, 16], f32, tag='lstm_h', name='lstm_h')
                nc.vector.tensor_tensor(hT_new[:].bitcast(f32r), sg[:, 4:6, :], tct[:],
                                        alu.mult)
                hT = hT_new

            # ---------------- FFN head ----------------
            lastT = [hT[:, 0, :], hT[:, 1, :], hbT[:, 0, :], hbT[:, 1, :]]
            z = [wp1.tile([128, 16], f32, tag=f'z_{i}', name=f'z_{i}') for i in range(2)]
            for mi in range(2):
                pz = psA.tile([128, 16], f32, tag='pa', name='pa')
                for k in range(4):
                    mmf(pz[:], C['ffn1t'][:, k, mi * 128:(mi + 1) * 128], lastT[k],
                        k == 0, k == 3)
                nc.scalar.activation(z[mi][:], pz[:], actf.Relu,
                                     bias=C['ffnb1'][:, mi, 0:1])
            py = psA.tile([1, 16], f32, tag='pa', name='pa')
            for k in range(2):
                mmf(py[:], C['ffn2t'][:, k, :], z[k][:], k == 0, k == 1)
            yo = wp1.tile([1, 16], f32, tag='yo', name='yo')
            nc.scalar.activation(yo[:], py[:], actf.Copy, bias=float(ffn_b2_val))
            nc.sync.dma_start(yout[:].unsqueeze(0), yo[:])

    _split_tpb_waits(nc)
    return nc


def _split_tpb_waits(nc, max_waits=1):
    """This walrus build caps sync-waits per TPB instruction; hoist extras
    onto same-engine NoOps placed immediately before the instruction."""
    from concourse import mybir
    dma_ops = ('DMACopy', 'DMATranspose', 'TensorLoad', 'TensorSave')
    cnt = 0
    for f in nc.m.functions:
        for bb in f.blocks:
            out = []
            changed = False
            for inst in bb.instructions:
                si = inst.sync_info
                opc = getattr(inst, 'opcode', '') or type(inst).__name__
                if (si is not None and len(si.on_wait) > max_waits
                        and getattr(inst, 'engine', None) is not None):
                    waits = list(si.on_wait)
                    for w in waits[:-max_waits]:
                        nop = mybir.InstNoOp(name=f'{inst.name}-sw{cnt}', ins=[], outs=[])
                        cnt += 1
                        nop.engine = inst.engine
                        nop.sync_info = mybir.SyncInfo(on_wait=[w], on_update=[])
                        out.append(nop)
                    inst.sync_info = mybir.SyncInfo(on_wait=waits[-max_waits:],
                                                    on_update=list(si.on_update))
                    changed = True
                out.append(inst)
            if changed:
                bb.instructions = out
    return nc


def _host_prep(inputs):
    f = lambda x: np.ascontiguousarray(x, dtype=np.float32)
    n = np.arange(NFFT)
    win = 0.5 * (1.0 - np.cos(2.0 * np.pi * n / NFFT))
    k = np.arange(NF)
    ang = 2.0 * np.pi * np.outer(n, k) / NFFT
    gw1t = inputs['gate_w1'].T / T
    runs, lo_t, w_t = _interp_tables()
    w14 = np.zeros((14, 128), np.float32)
    for s in range(2):
        for jj in range(7):
            w14[s * 7 + jj, s * 64:(s + 1) * 64] = inputs['lw1'][:, 0, jj]
    w14r = np.concatenate([np.concatenate([w14, np.zeros((18, 128), np.float32)])] * 3)
    wt = np.transpose(inputs['lw2'], (1, 2, 0))  # [64ch, 5dt, 128oc]
    w2Lr = np.zeros((128, 3, 128), np.float32)
    for c in range(3):
        w2Lr[0:64, c, :] = wt[:, 2 * c, :]
        if c < 2:
            w2Lr[64:128, c, :] = wt[:, 2 * c + 1, :]
    shared = {
        'crw': f(win[:, None] * np.cos(ang)),
        'ciw': f(win[:, None] * np.sin(ang)),
        'gw1ta': f(gw1t[0:128]), 'gw1tb': f(gw1t[128:129]),
        'gb1c': f(inputs['gate_b1'][:, None]),
        'gw2t': f(inputs['gate_w2'].T), 'gb2c': f(inputs['gate_b2'][:, None]),
        'iota8': f(np.tile(np.arange(NE)[None, :], (BC, 1))),
        'ones1': f(np.ones((1, 128))),
        'zer': f(np.zeros((128, 70))),
        'sel8': f(np.concatenate([np.tile(v[:, None], (1, 128)) for v in np.eye(NE)], axis=1)),
        'w1p': np.asarray(np.transpose(inputs['exp_w1'], (0, 3, 2, 1)).reshape(NE, 645, 256), dtype=ml_dtypes.bfloat16),
        'w1b': f(inputs['exp_b1'].T),
        'w2p': np.asarray(np.transpose(inputs['exp_w2'], (0, 3, 2, 1)).reshape(NE, 768, 256), dtype=ml_dtypes.bfloat16),
        'w2b': f(inputs['exp_b2'].T),
        'w14r': np.asarray(w14r, dtype=ml_dtypes.bfloat16), 'lb1c': f(np.tile(inputs['lb1'], 2)[:, None]),
        'w2Lr': np.asarray(w2Lr, dtype=ml_dtypes.bfloat16),
        'lb2c': f(inputs['lb2'][:, None]),
        'se2w1t': f(inputs['se2_w1'].T / 1024.0), 'se2w2t': f(inputs['se2_w2'].T),
        'w3L': np.asarray(np.transpose(inputs['lw3'], (2, 1, 0)), dtype=ml_dtypes.bfloat16),
        'lb3c': f(inputs['lb3'][:, None]),
        'se3w1t': f(inputs['se3_w1'].T / 256.0), 'se3w2t': f(inputs['se3_w2'].T),
        'wlo': f(w_t[None, :]),
        'wihft': np.asarray(inputs['wih_f'].T[:, GPERM], dtype=ml_dtypes.bfloat16),
        'biasfT': f((inputs['bih_f'] + inputs['bhh_f'])[GPERM].reshape(NE, 128).T),
        'whhft': f(inputs['whh_f'].T[:, GPERM]),
        'wihbt': np.asarray(inputs['wih_b'].T[:, GPERM], dtype=ml_dtypes.bfloat16),
        'biasbT': f((inputs['bih_b'] + inputs['bhh_b'])[GPERM].reshape(NE, 128).T),
        'i16': f(np.eye(16)), 'i128': f(np.eye(128)),
        'ffn1t': f(inputs['ffn_w1'].T), 'ffnb1': f(inputs['ffn_b1'][:, None]),
        'ffn2t': f(inputs['ffn_w2'].T),
    }
    xp = np.pad(inputs['x_continuum'], ((0, 0), (NFFT // 2, NFFT // 2)), mode='reflect')
    s0, s1 = xp.strides
    frames = np.lib.stride_tricks.as_strided(xp, (B, T, NFFT), (s0, 128 * s1, s1))
    xnp = np.pad(inputs['x_normalized'], ((0, 0), (3, 3 + 10)))
    in_maps = []
    for c in range(N_CORES):
        m = dict(shared)
        fr = frames[c * BC:(c + 1) * BC]
        m['framesT'] = f(np.transpose(fr, (2, 0, 1)).reshape(NFFT, BC * T))
        xc = xnp[c * BC:(c + 1) * BC]  # [16, 4112]
        x7b = np.zeros((3, 96, 8, 520), np.float32)
        for u in range(8):
            for s in range(2):
                for jj in range(7):
                    r = (u % 3) * 32 + s * 7 + jj
                    for gg in range(8):
                        x7b[u // 3, r, gg, :] = xc[gg * 2 + s, u * 512 + jj:u * 512 + jj + 520]
        m['x7b'] = np.asarray(x7b, dtype=ml_dtypes.bfloat16)
        in_maps.append(m)
    return in_maps


def _apply_tile_patch():
    from concourse import tile, mybir
    from concourse.vector_clock import ScopedClock

    def _drain_split(self, tick_clock, wait_clock):
        nc2 = self.nc
        di = nc2.sync.drain()
        wait_clock.add_sem_waits(di.ins, ScopedClock({None: tick_clock.global_clock}))
        si = di.ins.sync_info
        if si is not None and len(si.on_wait) > 1:
            waits = list(si.on_wait)
            ups = list(si.on_update)
            di.ins.sync_info = mybir.SyncInfo(on_wait=waits[:1], on_update=[])
            for kk, w in enumerate(waits[1:]):
                extra = nc2.sync.drain()
                extra.ins.sync_info = mybir.SyncInfo(
                    on_wait=[w], on_update=ups if kk == len(waits) - 2 else [])
        nc2.all_engine_barrier()
        assert self.sems is not None
        popped = nc2._tile_sem_poison_stack.pop()
        assert popped is self._sem_poison
        nc2.clear_and_free_semaphores(list(self.sems.allocated().values()))
        nc2.all_engine_barrier()

    tile.TileContext._drain_and_barrier = _drain_split


def kernel(**inputs):
    global _cache
    if 'nc' not in _cache:
        _apply_tile_patch()
        _cache['nc'] = _build(float(np.asarray(inputs['ffn_b2']).reshape(-1)[0]))
    from concourse.bass_utils import run_bass_kernel_spmd
    in_maps = _host_prep(inputs)
    res = run_bass_kernel_spmd(_cache['nc'], in_maps, list(range(N_CORES)))
    out = np.concatenate([res.results[c]['yout'] for c in range(N_CORES)])
    return out[:, None].astype(np.float32)



# revision 27
# speedup vs baseline: 1.0163x; 1.0162x over previous
"""DualBranchMoENet on Trainium2 — 8-core data-parallel (16 samples/core).

Channels live on SBUF partitions, (batch, time) on the free dim. Heavy
matmuls run fp32r (1 cyc/row at N>=256). Convolutions contract (cin, tap)
on the PE partition axis via shifted access patterns; only expert conv1
(129 ch x 5 taps) materialises an im2col stack. The LSTM keeps its hidden
state transposed ([256c, 16b]) so h @ whh^T needs no input transpose;
h is re-transposed once per step on the PE. The backward LSTM output
hb[T-1] equals ONE cell evaluated at t=32 from the zero state.
"""
import sys
sys.path.insert(0, '/opt/trn_rl_repo')
import numpy as np
import ml_dtypes

N_CORES = 8
B = 128
BC = B // N_CORES
L = 4096
NFFT = 256
NF = 129
T = 33
NE = 8

_cache = {}

# LSTM gate reorder (torch i,f,g,o) -> (i,f,o,g) so the three sigmoid gates
# are contiguous and fuse into one activation instruction.
GPERM = np.concatenate([np.arange(0, 512), np.arange(768, 1024), np.arange(512, 768)])


def _interp_tables():
    coords = np.clip((np.arange(T) + 0.5) * (64.0 / T) - 0.5, 0.0, 63.0)
    lo = np.floor(coords).astype(np.int64)
    w = coords - lo
    runs = []
    a = 0
    while a < T:
        b = a + 1
        if b < T:
            step = lo[a + 1] - lo[a]
            while b < T and lo[b] - lo[b - 1] == step:
                b += 1
        runs.append((a, b - a, int(lo[a]), int(lo[a + 1] - lo[a]) if b - a >= 2 else 1))
        a = b
    return runs, lo, w


def _build(ffn_b2_val):
    from concourse import bass, tile, mybir
    from concourse.mybir import AluOpType as alu
    from concourse.mybir import ActivationFunctionType as actf

    f32 = mybir.dt.float32
    f32r = mybir.dt.float32r
    bf16 = mybir.dt.bfloat16
    X = mybir.AxisListType.X

    BF16_IN = {'w1p', 'w2p', 'x7b', 'w14r', 'w2Lr', 'w3L', 'wihft', 'wihbt'}
    nc = bass.Bass()
    inp = lambda name, shape: nc.declare_dram_parameter(
        name, list(shape), bf16 if name in BF16_IN else f32, isOutput=False)

    d = {}
    for name, shape in [
        ('framesT', [NFFT, BC * T]), ('x7b', [3, 96, 8, 520]),
        ('crw', [NFFT, NF]), ('ciw', [NFFT, NF]),
        ('gw1ta', [128, 128]), ('gw1tb', [1, 128]), ('gb1c', [128, 1]),
        ('gw2t', [128, NE]), ('gb2c', [NE, 1]), ('iota8', [BC, NE]), ('ones1', [1, 128]), ('sel8', [NE, NE * 128]), ('zer', [128, 70]),
        ('w1p', [NE, 645, 256]), ('w1b', [256, NE]),
        ('w2p', [NE, 768, 256]), ('w2b', [256, NE]),
        ('w14r', [96, 128]), ('lb1c', [128, 1]),
        ('w2Lr', [128, 3, 128]), ('lb2c', [128, 1]),
        ('se2w1t', [128, 32]), ('se2w2t', [32, 128]),
        ('w3L', [3, 128, 256]), ('lb3c', [256, 1]),
        ('se3w1t', [256, 64]), ('se3w2t', [64, 256]),
        ('wlo', [1, T]),
        ('wihft', [512, 1024]), ('biasfT', [128, NE]), ('whhft', [256, 1024]),
        ('wihbt', [512, 1024]), ('biasbT', [128, NE]),
        ('i16', [16, 16]), ('i128', [128, 128]),
        ('ffn1t', [512, 256]), ('ffnb1', [256, 1]), ('ffn2t', [256, 1]),
    ]:
        d[name] = inp(name, shape)
    yout = nc.declare_dram_parameter('yout', [BC], f32, isOutput=True)

    runs, lo_t, w_t = _interp_tables()

    def mm(out, lhsT, rhs, start, stop):
        nc.tensor.matmul(out, lhsT, rhs, start=start, stop=stop)

    def mmr(out, lhsT, rhs, start, stop):
        nc.tensor.matmul(out, lhsT.bitcast(f32r), rhs.bitcast(f32r),
                         start=start, stop=stop)

    def mmf(out, lhsT, rhs, start, stop):
        nc.tensor.matmul(out, lhsT, rhs, start=start, stop=stop)

    with tile.TileContext(nc, num_cores=N_CORES) as tc:
        with (
            tc.tile_pool(name='const', bufs=1) as cp,
            tc.tile_pool(name='work', bufs=2) as wp,
            tc.tile_pool(name='one', bufs=1) as wp1,
            tc.tile_pool(name='big', bufs=1) as bp,
            tc.tile_pool(name='psA', bufs=2, space='PSUM') as psA,
            tc.tile_pool(name='psB', bufs=4, space='PSUM') as psB,
            tc.tile_pool(name='psC', bufs=2, space='PSUM') as psC,
        ):
            C = {}
            for name, shape, rr in [
                ('crw', [128, 2, NF], '(k p) m -> p k m'),
                ('ciw', [128, 2, NF], '(k p) m -> p k m'),
                ('gw1ta', [128, 128], None), ('gw1tb', [1, 128], None),
                ('gb1c', [128, 1], None),
                ('gw2t', [128, NE], None), ('gb2c', [NE, 1], None),
                ('iota8', [BC, NE], None),
                ('ones1', [1, 128], None),
                ('sel8', [NE, NE * 128], None),
                ('w1b', [128, 2, NE], '(k p) m -> p k m'),
                ('w2b', [128, 2, NE], '(k p) m -> p k m'),
                ('lb1c', [128, 1], None),
                ('lb2c', [128, 1], None),
                ('se2w1t', [128, 32], None), ('se2w2t', [32, 128], None),
                ('lb3c', [128, 2, 1], '(k p) m -> p k m'),
                ('se3w1t', [128, 2, 64], '(k p) m -> p k m'),
                ('se3w2t', [64, 256], None),
                ('wlo', [1, T], None),
                ('biasfT', [128, NE], None),
                ('whhft', [128, 2, 1024], '(k p) m -> p k m'),
                ('biasbT', [128, NE], None),
                ('i16', [16, 16], None), ('i128', [128, 128], None),
                ('ffn1t', [128, 4, 256], '(k p) m -> p k m'),
                ('ffnb1', [128, 2, 1], '(k p) m -> p k m'),
                ('ffn2t', [128, 2, 1], '(k p) m -> p k m'),
            ]:
                t = cp.tile(shape, f32, tag=name)
                src = d[name][:]
                if rr:
                    src = src.rearrange(rr, p=128)
                if name in ('crw', 'ciw', 'whhft', 'i128'):
                    nc.sync.dma_start(t[:].bitcast(f32r), src.bitcast(f32r))
                else:
                    nc.sync.dma_start(t[:], src)
                C[name] = t
            t = cp.tile([96, 128], bf16, tag='w14r')
            nc.sync.dma_start(t[:], d['w14r'][:])
            C['w14r'] = t
            t = cp.tile([128, 3, 128], bf16, tag='w2Lr')
            nc.sync.dma_start(t[:], d['w2Lr'][:])
            C['w2Lr'] = t
            t = cp.tile([128, 3, 256], bf16, tag='w3L')
            nc.sync.dma_start(t[:], d['w3L'][:].rearrange('d k m -> k d m'))
            C['w3L'] = t

            # ---------------- STFT magnitude ----------------
            lp_cm = tc.tile_pool(name='lp', bufs=2)
            lp = lp_cm.__enter__()
            ep_cm = tc.tile_pool(name='ep', bufs=2)
            ep = ep_cm.__enter__()
            c_fr = ep.tile([128, 2, BC * T], f32, tag='framesT', name='framesT', bufs=1)
            nc.sync.dma_start(c_fr[:].bitcast(f32r), d['framesT'][:].rearrange('(k p) m -> p k m', p=128).bitcast(f32r))
            C['framesT'] = c_fr
            magA = ep.tile([128, BC, T + 4], f32, tag='magA', name='magA', bufs=1)
            magB = ep.tile([1, BC, T + 4], f32, tag='magB', name='magB', bufs=1)
            nc.vector.memset(magA[:], 0.0)
            nc.vector.memset(magB[:], 0.0)
            NB2 = BC * T // 2
            for m0, mn, magX in [(0, 128, magA), (128, 1, magB)]:
                sqr = ep.tile([mn, BC * T], f32, tag=f'sqr{m0}', name=f'sqr{m0}', bufs=1)
                sqi = ep.tile([mn, BC * T], f32, tag=f'sqi{m0}', name=f'sqi{m0}', bufs=1)
                for ni in range(2):
                    pre = psA.tile([mn, NB2], f32, tag='pa', name='pa')
                    pim = psA.tile([mn, NB2], f32, tag='pa', name='pa')
                    for k in range(2):
                        co = slice(ni * NB2, (ni + 1) * NB2)
                        mmr(pre[:], C['crw'][:, k, m0:m0 + mn], C['framesT'][:, k, co], k == 0, k == 1)
                        mmr(pim[:], C['ciw'][:, k, m0:m0 + mn], C['framesT'][:, k, co], k == 0, k == 1)
                    nc.scalar.square(sqr[:, ni * NB2:(ni + 1) * NB2], pre[:])
                    nc.scalar.square(sqi[:, ni * NB2:(ni + 1) * NB2], pim[:])
                nc.vector.tensor_add(sqr[:], sqr[:], sqi[:])
                nc.scalar.sqrt(magX[0:mn, :, 2:2 + T],
                               sqr[:].rearrange('p (b t) -> p b t', b=BC))

            magAb = ep.tile([128, BC, T + 4], bf16, tag='magAb', name='magAb', bufs=1)
            magBb = ep.tile([1, BC, T + 4], bf16, tag='magBb', name='magBb', bufs=1)
            nc.scalar.activation(magAb[:], magA[:], actf.Copy)
            nc.scalar.activation(magBb[:], magB[:], actf.Copy)

            # ---------------- gating (fp32 matmuls) ----------------
            pooledA = ep.tile([128, BC], f32, tag='pooledA', name='pooledA')
            pooledB = ep.tile([1, BC], f32, tag='pooledB', name='pooledB')
            nc.vector.tensor_reduce(pooledA[:], magA[:, :, 2:2 + T], X, alu.add)
            nc.vector.tensor_reduce(pooledB[:], magB[:, :, 2:2 + T], X, alu.add)
            pg1 = psA.tile([128, BC], f32, tag='pa', name='pa')
            mmf(pg1[:], C['gw1ta'][:], pooledA[:], True, False)
            mmf(pg1[:], C['gw1tb'][:], pooledB[:], False, True)
            gh = ep.tile([128, BC], f32, tag='gh', name='gh')
            nc.scalar.activation(gh[:], pg1[:], actf.Relu, bias=C['gb1c'][:, 0:1])
            pg2 = psA.tile([NE, BC], f32, tag='pa', name='pa')
            mmf(pg2[:], C['gw2t'][:], gh[:], True, True)
            logitsT = ep.tile([NE, BC], f32, tag='logitsT', name='logitsT')
            nc.vector.tensor_tensor(logitsT[:], pg2[:],
                                    C['gb2c'][:, 0:1].to_broadcast([NE, BC]), alu.add)
            plg = psA.tile([BC, NE], f32, tag='pa', name='pa')
            nc.tensor.transpose(plg[:], logitsT[:], C['i16'][0:NE, 0:NE])
            lg = ep.tile([BC, NE], f32, tag='lg', name='lg')
            nc.vector.tensor_copy(lg[:], plg[:])
            iob = C['iota8'][:]

            def small(tag, shape=(BC, NE)):
                return ep.tile(list(shape), f32, tag=tag, name=tag)

            m1 = small('m1', (BC, 1))
            nc.vector.tensor_reduce(m1[:], lg[:], X, alu.max)
            eq1 = small('eq1')
            nc.vector.tensor_tensor(eq1[:], lg[:], m1[:].to_broadcast([BC, NE]), alu.is_equal)
            l2 = small('l2')
            nc.vector.scalar_tensor_tensor(l2[:], eq1[:], -1e30, lg[:], alu.mult, alu.add)
            m2 = small('m2', (BC, 1))
            nc.vector.tensor_reduce(m2[:], l2[:], X, alu.max)
            it1 = small('it1')
            nc.vector.tensor_tensor(it1[:], eq1[:], iob, alu.mult)
            idx1 = small('idx1', (BC, 1))
            nc.vector.tensor_reduce(idx1[:], it1[:], X, alu.max)
            eq2 = small('eq2')
            nc.vector.tensor_tensor(eq2[:], l2[:], m2[:].to_broadcast([BC, NE]), alu.is_equal)
            it2 = small('it2')
            nc.vector.tensor_tensor(it2[:], eq2[:], iob, alu.mult)
            idx2 = small('idx2', (BC, 1))
            nc.vector.tensor_reduce(idx2[:], it2[:], X, alu.max)
            dm = small('dm', (BC, 1))
            nc.vector.tensor_sub(dm[:], m1[:], m2[:])
            g1 = small('g1', (BC, 1))
            nc.scalar.activation(g1[:], dm[:], actf.Sigmoid)
            g2 = small('g2', (BC, 1))
            nc.vector.tensor_scalar(g2[:], g1[:], -1.0, 1.0, alu.mult, alu.add)
            eA = small('eA')
            nc.vector.tensor_tensor(eA[:], idx1[:].to_broadcast([BC, NE]), iob, alu.is_equal)
            eB = small('eB')
            nc.vector.tensor_tensor(eB[:], idx2[:].to_broadcast([BC, NE]), iob, alu.is_equal)
            tA = small('tA')
            nc.vector.tensor_tensor(tA[:], eA[:], g1[:].to_broadcast([BC, NE]), alu.mult)
            tB = small('tB')
            nc.vector.tensor_tensor(tB[:], eB[:], g2[:].to_broadcast([BC, NE]), alu.mult)
            W8 = small('W8')
            nc.vector.tensor_add(W8[:], tA[:], tB[:])
            pW8T = psA.tile([NE, BC], f32, tag='pa', name='pa')
            nc.tensor.transpose(pW8T[:], W8[:], C['i16'][:])
            W8T = ep.tile([NE, BC], f32, tag='W8T', name='W8T')
            nc.vector.tensor_copy(W8T[:], pW8T[:])

            # ---------------- line conv1 (emitted early: overlaps expert DMA) ----
            h1 = lp.tile([128, 8, 1028], bf16, tag='h1', name='h1', bufs=1)
            nc.gpsimd.memset(h1[:, :, 0:2], 0.0)
            nc.gpsimd.memset(h1[:, :, 1026:1028], 0.0)
            x7 = [ep.tile([96, 8, 520], bf16, tag=f'x7_{h}', name=f'x7_{h}', bufs=1)
                  for h in range(3)]
            for h in range(3):
                nc.sync.dma_start(x7[h][:], d['x7b'][h])
            for u in range(8):
                ub = (u % 3) * 32
                for gg in range(8):
                    pl1 = psC.tile([128, 512], f32, tag='pc', name='pc')
                    mm(pl1[:], C['w14r'][ub:ub + 32, :],
                       x7[u // 3][ub:ub + 32, gg, 0:512], True, True)
                    o0 = 2 + u * 128
                    nc.vector.tensor_reduce(h1[:, gg, o0:o0 + 128],
                                            pl1[:].rearrange('p (t q) -> p t q', q=4),
                                            X, alu.max)
            nc.scalar.activation(h1[:, :, 2:1026], h1[:, :, 2:1026], actf.Relu,
                                 bias=C['lb1c'][:, 0:1])

            # ---------------- experts (dense, weighted sum) ----------------
            imt = [ep.tile([128 if k < 5 else 5, BC, T], bf16, tag=f'im1_{k}', name=f'im1_{k}', bufs=1)
                   for k in range(6)]
            for dt in range(5):
                pos = dt * NF
                done = 0
                while done < NF:
                    k, r = divmod(pos + done, 128)
                    if done < 128:
                        n = min(128 - r, NF - done, 128 - done)
                        nc.sync.dma_start(imt[k][r:r + n],
                                          magAb[done:done + n, :, dt:dt + T])
                    else:
                        n = 1
                        nc.sync.dma_start(imt[k][r:r + 1], magBb[0:1, :, dt:dt + T])
                    done += n
            accF = [bp.tile([128, BC, T], f32, tag=f'accF{i}', name=f'accF{i}') for i in range(2)]
            H = BC // 2
            for e in range(NE):
                w1s = ep.tile([128, 6, 256], bf16, tag='w1s', name='w1s')
                nc.sync.dma_start(w1s[:, 0:5, :],
                                  d['w1p'][e, 0:640, :].rearrange('(k p) m -> p k m', p=128))
                nc.sync.dma_start(w1s[0:5, 5, :], d['w1p'][e, 640:645, :])
                he = [ep.tile([128, BC * (T + 2) + 2], bf16, tag=f'he{i}', name=f'he{i}') for i in range(2)]
                for i in range(2):
                    hv = he[i][:, 0:BC * (T + 2)].rearrange('p (b t) -> p b t', t=T + 2)
                    nc.gpsimd.memset(hv[:, :, 0:1], 0.0)
                    nc.gpsimd.memset(hv[:, :, T + 1:T + 2], 0.0)
                    nc.gpsimd.memset(he[i][:, BC * (T + 2):], 0.0)
                for mi in range(2):
                    for ni in range(2):
                        pe1 = psB.tile([128, H * T], f32, tag='pb', name='pb')
                        for k in range(6):
                            kn = 128 if k < 5 else 5
                            mm(pe1[:], w1s[0:kn, k, mi * 128:(mi + 1) * 128],
                               imt[k][:].rearrange('p b t -> p (b t)')[:, ni * H * T:(ni + 1) * H * T],
                               k == 0, k == 5)
                        nc.scalar.activation(he[mi][:, 0:BC * (T + 2)].rearrange('p (b t) -> p b t', t=T + 2)[:, ni * H:(ni + 1) * H, 1:1 + T],
                                             pe1[:].rearrange('p (b t) -> p b t', t=T),
                                             actf.Relu, bias=C['w1b'][:, mi, e:e + 1])
                w2s = ep.tile([128, 6, 256], bf16, tag='w2s', name='w2s')
                nc.sync.dma_start(w2s[:], d['w2p'][e].rearrange('(k p) m -> p k m', p=128))
                eo = [ep.tile([128, BC, T], f32, tag=f'eo{i}', name=f'eo{i}', bufs=1) for i in range(2)]
                W2 = T + 2
                for mi in range(2):
                    for bi in range(2):
                        pe2 = psB.tile([128, H * W2], f32, tag='pb', name='pb')
                        for k in range(6):
                            dt, ch = divmod(k, 2)
                            mm(pe2[:], w2s[:, k, mi * 128:(mi + 1) * 128],
                               he[ch][:, bi * H * W2 + dt:bi * H * W2 + dt + H * W2],
                               k == 0, k == 5)
                        nc.scalar.activation(eo[mi][:, bi * H:(bi + 1) * H, :],
                                             pe2[:].rearrange('p (b t) -> p b t', t=W2)[:, :, 0:T],
                                             actf.Relu, bias=C['w2b'][:, mi, e:e + 1])
                pwe = psA.tile([128, BC], f32, tag='pa', name='pwe')
                mmf(pwe[:], C['sel8'][:, e * 128:(e + 1) * 128], W8T[:], True, True)
                wE = ep.tile([128, BC], f32, tag='wE', name='wE')
                nc.vector.tensor_copy(wE[:], pwe[:])
                wbc = wE[:].unsqueeze(2).to_broadcast([128, BC, T])
                for mi in range(2):
                    if e == 0:
                        nc.vector.tensor_tensor(accF[mi][:].bitcast(f32r), eo[mi][:], wbc, alu.mult)
                    else:
                        eow = ep.tile([128, BC, T], f32, tag='eow', name='eow')
                        nc.vector.tensor_tensor(eow[:], eo[mi][:], wbc, alu.mult)
                        nc.vector.tensor_add(accF[mi][:].bitcast(f32r), accF[mi][:], eow[:])

            ep_cm.__exit__(None, None, None)

            # ---------------- line branch ----------------

            hp2 = lp.tile([128, BC, 258], bf16, tag='hp2', name='hp2', bufs=1)
            nc.gpsimd.memset(hp2[:, :, 0:1], 0.0)
            nc.gpsimd.memset(hp2[:, :, 257:258], 0.0)
            hp2v = hp2[:, :, 1:257].rearrange('p (g s) t -> p s g t', s=2)
            seacc = lp.tile([128, 2, 8, 2], f32, tag='seacc', name='seacc', bufs=1)
            hs = [lp.tile([128, 8, 1028], bf16, tag=f'hs{s}', name=f'hs{s}', bufs=1)
                  for s in range(2)]
            for s in range(2):
                nc.sync.dma_start(hs[s][0:64], h1[s * 64:(s + 1) * 64])
                nc.sync.dma_start(hs[s][64:128, :, 0:1027], h1[s * 64:(s + 1) * 64, :, 1:1028])
            for s in range(2):
                for gg in range(8):
                    for uh in range(2):
                        pl2 = psB.tile([128, 512], f32, tag='pb', name='pb')
                        base = uh * 512
                        mm(pl2[:], C['w2Lr'][:, 0, :], hs[s][:, gg, base:base + 512],
                           True, False)
                        mm(pl2[:], C['w2Lr'][:, 1, :], hs[s][:, gg, base + 2:base + 514],
                           False, False)
                        mm(pl2[:], C['w2Lr'][0:64, 2, :], hs[s][0:64, gg, base + 4:base + 516],
                           False, True)
                        r2 = lp.tile([128, 512], f32, tag='r2', name='r2')
                        nc.scalar.activation(r2[:], pl2[:], actf.Relu, bias=C['lb2c'][:, 0:1],
                                             accum_out=seacc[:, s, gg, uh:uh + 1])
                        nc.vector.tensor_reduce(hp2v[:, s, gg, uh * 128:uh * 128 + 128],
                                                r2[:].rearrange('p (w q) -> p w q', q=4),
                                                X, alu.max)
            seY = lp.tile([128, 2, 8], f32, tag='seY', name='seY')
            nc.vector.tensor_reduce(seY[:], seacc[:], X, alu.add)
            pse1 = psA.tile([32, 16], f32, tag='pa', name='pa')
            mmf(pse1[:], C['se2w1t'][:], seY[:].rearrange('p s g -> p (s g)'), True, True)
            z2 = lp.tile([32, 16], f32, tag='z2', name='z2')
            nc.scalar.activation(z2[:], pse1[:], actf.Relu)
            pse2 = psA.tile([128, 16], f32, tag='pa', name='pa')
            mmf(pse2[:], C['se2w2t'][:], z2[:], True, True)
            sc2 = lp.tile([128, 2, 8], f32, tag='sc2', name='sc2')
            nc.scalar.activation(sc2[:].rearrange('p s g -> p (s g)'), pse2[:], actf.Sigmoid)
            nc.vector.tensor_tensor(
                hp2[:, :, 1:257].rearrange('p (g s) t -> p g s t', s=2),
                hp2[:, :, 1:257].rearrange('p (g s) t -> p g s t', s=2),
                sc2[:].rearrange('p s g -> p g s').unsqueeze(3).to_broadcast([128, 8, 2, 256]),
                alu.mult)

            # conv3 + SE3 + pool, chunked over sample pairs
            y3 = lp.tile([128, 2, BC], f32, tag='y3', name='y3')
            lf = [lp.tile([128, BC, 64], bf16, tag=f'lf{i}', name=f'lf{i}', bufs=1) for i in range(2)]
            hp2f = hp2[:].rearrange('p b t -> p (b t)')
            for mi in range(2):
                for b0 in range(0, BC, 2):
                    pl3 = psB.tile([128, 2, 256], f32, tag='pb', name='pb')
                    for bi in (b0, b0 + 1):
                        for dt in range(3):
                            mm(pl3[:, bi - b0, :], C['w3L'][:, dt, mi * 128:(mi + 1) * 128],
                               hp2f[:, bi * 258 + dt:bi * 258 + dt + 256], dt == 0, dt == 2)
                    r3 = lp.tile([128, 2, 256], f32, tag='r3', name='r3')
                    nc.scalar.activation(r3[:], pl3[:],
                                         actf.Relu, bias=C['lb3c'][:, mi, 0:1])
                    nc.vector.tensor_reduce(y3[:, mi, b0:b0 + 2], r3[:], X, alu.add)
                    nc.vector.tensor_reduce(lf[mi][:, b0:b0 + 2, :],
                                            r3[:].rearrange('p c (u q) -> p c u q', q=4),
                                            X, alu.max)
            pse3 = psA.tile([64, 16], f32, tag='pa', name='pa')
            for k in range(2):
                mmf(pse3[:], C['se3w1t'][:, k, :], y3[:, k, :], k == 0, k == 1)
            z3 = lp.tile([64, 16], f32, tag='z3', name='z3')
            nc.scalar.activation(z3[:], pse3[:], actf.Relu)
            sc3 = [lp.tile([128, BC], f32, tag=f'sc3_{i}', name=f'sc3_{i}') for i in range(2)]
            for mi in range(2):
                pse4 = psA.tile([128, 16], f32, tag='pa', name='pa')
                mmf(pse4[:], C['se3w2t'][:, mi * 128:(mi + 1) * 128], z3[:], True, True)
                nc.scalar.activation(sc3[mi][:], pse4[:], actf.Sigmoid)
            for mi in range(2):
                nc.vector.tensor_tensor(lf[mi][:], lf[mi][:],
                                        sc3[mi][:].unsqueeze(2).to_broadcast([128, BC, 64]),
                                        alu.mult)
            # interp 64 -> 33
            li = [bp.tile([128, BC, T], f32, tag=f'li{i}', name=f'li{i}') for i in range(2)]
            pwl = psA.tile([128, T], f32, tag='pa', name='pwl')
            mmf(pwl[:], C['ones1'][:], C['wlo'][:], True, True)
            wlo128 = lp.tile([128, T], f32, tag='wlo128', name='wlo128', bufs=1)
            nc.vector.tensor_copy(wlo128[:], pwl[:])
            wbc_all = wlo128[:]
            for mi in range(2):
                for (a, n, lo0, st) in runs:
                    end = lo0 + (n - 1) * st + 1
                    lov = lf[mi][:, :, lo0:end:st]
                    hiv = lf[mi][:, :, lo0 + 1:end + 1:st]
                    dd = lp.tile([128, BC, T], f32, tag='dd', name='dd')
                    nc.vector.tensor_sub(dd[:, :, a:a + n], hiv, lov)
                    nc.vector.tensor_tensor(dd[:, :, a:a + n], dd[:, :, a:a + n],
                                            wbc_all[:, a:a + n].unsqueeze(1).to_broadcast([128, BC, n]),
                                            alu.mult)
                    nc.vector.tensor_add(li[mi][:, :, a:a + n].bitcast(f32r), dd[:, :, a:a + n], lov)

            lp_cm.__exit__(None, None, None)

            # ---------------- LSTM input precompute (transposed) ----------------
            # XsT[p, jj, t, b] = (x_t @ wih^T + bias)[b, jj*128+p], gate order (i,f,o,g)
            ct = [accF[0], accF[1], li[0], li[1]]
            xp_cm = tc.tile_pool(name='xp', bufs=2)
            xpp = xp_cm.__enter__()
            XsT = bp.tile([128, 8, T, 16], f32, tag='XsT', name='XsT')
            wft = [xpp.tile([128, 1024], bf16, tag=f'wft{k}', name=f'wft{k}', bufs=1)
                   for k in range(4)]
            xtT = [xpp.tile([128, T, 16], bf16, tag=f'xtT{k}', name=f'xtT{k}', bufs=1)
                   for k in range(4)]
            for k in range(4):
                nc.sync.dma_start(wft[k][:], d['wihft'][k * 128:(k + 1) * 128, :])
                nc.vector.tensor_copy(xtT[k][:], ct[k][:, :, :].transpose([0, 2, 1]))
            for jj in range(8):
                for (t0, tl) in ((0, 16), (16, 17)):
                    ps = psB.tile([128, tl * 16], f32, tag='pb', name='pb')
                    for k in range(4):
                        mm(ps[:], wft[k][:, jj * 128:(jj + 1) * 128],
                           xtT[k][:, t0:t0 + tl, :].rearrange('p t b -> p (t b)'),
                           k == 0, k == 3)
                    nc.vector.tensor_tensor(
                        XsT[:, jj, t0:t0 + tl, :].rearrange('p t b -> p (t b)').bitcast(f32r),
                        ps[:],
                        C['biasfT'][:, jj:jj + 1].to_broadcast([128, tl * 16]),
                        alu.add)

            # ---------------- backward cell (t=32, transposed) ----------------
            wbt = [xpp.tile([128, 1024], bf16, tag=f'wft{k}', name=f'wbt{k}', bufs=1)
                   for k in range(4)]
            for k in range(4):
                nc.sync.dma_start(wbt[k][:], d['wihbt'][k * 128:(k + 1) * 128, :])
            psb = psA.tile([128, 8, 16], f32, tag='pa', name='psb')
            for jj in range(8):
                for k in range(4):
                    mm(psb[:, jj, :], wbt[k][:, jj * 128:(jj + 1) * 128],
                       xtT[k][:, 32, :], k == 0, k == 3)
            gbT = wp1.tile([128, 8, 16], f32, tag='gbT', name='gbT')
            nc.vector.tensor_tensor(gbT[:], psb[:],
                                    C['biasbT'][:].unsqueeze(2).to_broadcast([128, 8, 16]),
                                    alu.add)
            sgb = wp1.tile([128, 8, 16], f32, tag='sgb', name='sgb')
            nc.scalar.activation(sgb[:, 0:6, :], gbT[:, 0:6, :], actf.Sigmoid)
            nc.scalar.activation(sgb[:, 6:8, :], gbT[:, 6:8, :], actf.Tanh)
            cbT = wp1.tile([128, 2, 16], f32, tag='cbT', name='cbT')
            nc.vector.tensor_tensor(cbT[:], sgb[:, 0:2, :], sgb[:, 6:8, :], alu.mult)
            tcb = wp1.tile([128, 2, 16], f32, tag='tcb', name='tcb')
            nc.scalar.activation(tcb[:], cbT[:], actf.Tanh)
            hbT = bp.tile([128, 2, 16], f32, tag='hbT', name='hbT')
            nc.vector.tensor_tensor(hbT[:], sgb[:, 4:6, :], tcb[:], alu.mult)
            xp_cm.__exit__(None, None, None)

            # ---------------- forward LSTM (33 steps, transposed) ----------------
            # gates live as [128 = j-chunk, jj, 16 = batch]; no per-step transposes.
            hT = None
            cT = None
            for t in range(T):
                psg = psB.tile([128, 8, 16], f32, tag='pb', name='psg')
                for jj in range(8):
                    mmr(psg[:, jj, :], C['i128'][:], XsT[:, jj, t, :], True, t == 0)
                    if t > 0:
                        for k in range(2):
                            mmr(psg[:, jj, :], C['whhft'][:, k, jj * 128:(jj + 1) * 128],
                                hT[:, k, :], False, k == 1)
                sg = wp.tile([128, 8, 16], f32, tag='lstm_sg', name='lstm_sg')
                nc.scalar.activation(sg[:, 0:6, :], psg[:, 0:6, :], actf.Sigmoid)
                nc.scalar.activation(sg[:, 6:8, :], psg[:, 6:8, :], actf.Tanh)
                t2 = wp.tile([128, 2, 16], f32, tag='lstm_t2', name='lstm_t2')
                nc.gpsimd.tensor_tensor(t2[:], sg[:, 0:2, :], sg[:, 6:8, :], alu.mult)
                cT_new = wp.tile([128, 2, 16], f32, tag='lstm_c', name='lstm_c')
                if t == 0:
                    nc.vector.tensor_copy(cT_new[:], t2[:])
                else:
                    t1 = wp.tile([128, 2, 16], f32, tag='lstm_t1', name='lstm_t1')
                    nc.vector.tensor_tensor(t1[:], sg[:, 2:4, :], cT[:], alu.mult)
                    nc.vector.tensor_tensor(cT_new[:], t1[:], t2[:], alu.add)
                cT = cT_new
                tct = wp.tile([128, 2, 16], f32, tag='lstm_tc', name='lstm_tc')
                nc.scalar.activation(tct[:], cT[:], actf.Tanh)
                hT_new = wp.tile([128, 2, 16], f32, tag='lstm_h', name='lstm_h')
                nc.vector.tensor_tensor(hT_new[:].bitcast(f32r), sg[:, 4:6, :], tct[:],
                                        alu.mult)
                hT = hT_new

            # ---------------- FFN head ----------------
            lastT = [hT[:, 0, :], hT[:, 1, :], hbT[:, 0, :], hbT[:, 1, :]]
            z = [wp1.tile([128, 16], f32, tag=f'z_{i}', name=f'z_{i}') for i in range(2)]
            for mi in range(2):
                pz = psA.tile([128, 16], f32, tag='pa', name='pa')
                for k in range(4):
                    mmf(pz[:], C['ffn1t'][:, k, mi * 128:(mi + 1) * 128], lastT[k],
                        k == 0, k == 3)
                nc.scalar.activation(z[mi][:], pz[:], actf.Relu,
                                     bias=C['ffnb1'][:, mi, 0:1])
            py = psA.tile([1, 16], f32, tag='pa', name='pa')
            for k in range(2):
                mmf(py[:], C['ffn2t'][:, k, :], z[k][:], k == 0, k == 1)
            yo = wp1.tile([1, 16], f32, tag='yo', name='yo')
            nc.scalar.activation(yo[:], py[:], actf.Copy, bias=float(ffn_b2_val))
            nc.sync.dma_start(yout[:].unsqueeze(0), yo[:])

    _split_tpb_waits(nc)
    return nc


def _split_tpb_waits(nc, max_waits=1):
    """This walrus build caps sync-waits per TPB instruction; hoist extras
    onto same-engine NoOps placed immediately before the instruction."""
    from concourse import mybir
    dma_ops = ('DMACopy', 'DMATranspose', 'TensorLoad', 'TensorSave')
    cnt = 0
    for f in nc.m.functions:
        for bb in f.blocks:
            out = []
            changed = False
            for inst in bb.instructions:
                si = inst.sync_info
                opc = getattr(inst, 'opcode', '') or type(inst).__name__
                if (si is not None and len(si.on_wait) > max_waits
                        and getattr(inst, 'engine', None) is not None):
                    waits = list(si.on_wait)
                    for w in waits[:-max_waits]:
                        nop = mybir.InstNoOp(name=f'{inst.name}-sw{cnt}', ins=[], outs=[])
                        cnt += 1
                        nop.engine = inst.engine
                        nop.sync_info = mybir.SyncInfo(on_wait=[w], on_update=[])
                        out.append(nop)
                    inst.sync_info = mybir.SyncInfo(on_wait=waits[-max_waits:],
                                                    on_update=list(si.on_update))
                    changed = True
                out.append(inst)
            if changed:
                bb.instructions = out
    return nc


def _host_prep(inputs):
    f = lambda x: np.ascontiguousarray(x, dtype=np.float32)
    n = np.arange(NFFT)
    win = 0.5 * (1.0 - np.cos(2.0 * np.pi * n / NFFT))
    k = np.arange(NF)
    ang = 2.0 * np.pi * np.outer(n, k) / NFFT
    gw1t = inputs['gate_w1'].T / T
    runs, lo_t, w_t = _interp_tables()
    w14 = np.zeros((14, 128), np.float32)
    for s in range(2):
        for jj in range(7):
            w14[s * 7 + jj, s * 64:(s + 1) * 64] = inputs['lw1'][:, 0, jj]
    w14r = np.concatenate([np.concatenate([w14, np.zeros((18, 128), np.float32)])] * 3)
    wt = np.transpose(inputs['lw2'], (1, 2, 0))  # [64ch, 5dt, 128oc]
    w2Lr = np.zeros((128, 3, 128), np.float32)
    for c in range(3):
        w2Lr[0:64, c, :] = wt[:, 2 * c, :]
        if c < 2:
            w2Lr[64:128, c, :] = wt[:, 2 * c + 1, :]
    shared = {
        'crw': f(win[:, None] * np.cos(ang)),
        'ciw': f(win[:, None] * np.sin(ang)),
        'gw1ta': f(gw1t[0:128]), 'gw1tb': f(gw1t[128:129]),
        'gb1c': f(inputs['gate_b1'][:, None]),
        'gw2t': f(inputs['gate_w2'].T), 'gb2c': f(inputs['gate_b2'][:, None]),
        'iota8': f(np.tile(np.arange(NE)[None, :], (BC, 1))),
        'ones1': f(np.ones((1, 128))),
        'zer': f(np.zeros((128, 70))),
        'sel8': f(np.concatenate([np.tile(v[:, None], (1, 128)) for v in np.eye(NE)], axis=1)),
        'w1p': np.asarray(np.transpose(inputs['exp_w1'], (0, 3, 2, 1)).reshape(NE, 645, 256), dtype=ml_dtypes.bfloat16),
        'w1b': f(inputs['exp_b1'].T),
        'w2p': np.asarray(np.transpose(inputs['exp_w2'], (0, 3, 2, 1)).reshape(NE, 768, 256), dtype=ml_dtypes.bfloat16),
        'w2b': f(inputs['exp_b2'].T),
        'w14r': np.asarray(w14r, dtype=ml_dtypes.bfloat16), 'lb1c': f(np.tile(inputs['lb1'], 2)[:, None]),
        'w2Lr': np.asarray(w2Lr, dtype=ml_dtypes.bfloat16),
        'lb2c': f(inputs['lb2'][:, None]),
        'se2w1t': f(inputs['se2_w1'].T / 1024.0), 'se2w2t': f(inputs['se2_w2'].T),
        'w3L': np.asarray(np.transpose(inputs['lw3'], (2, 1, 0)), dtype=ml_dtypes.bfloat16),
        'lb3c': f(inputs['lb3'][:, None]),
        'se3w1t': f(inputs['se3_w1'].T / 256.0), 'se3w2t': f(inputs['se3_w2'].T),
        'wlo': f(w_t[None, :]),
        'wihft': np.asarray(inputs['wih_f'].T[:, GPERM], dtype=ml_dtypes.bfloat16),
        'biasfT': f((inputs['bih_f'] + inputs['bhh_f'])[GPERM].reshape(NE, 128).T),
        'whhft': f(inputs['whh_f'].T[:, GPERM]),
        'wihbt': np.asarray(inputs['wih_b'].T[:, GPERM], dtype=ml_dtypes.bfloat16),
        'biasbT': f((inputs['bih_b'] + inputs['bhh_b'])[GPERM].reshape(NE, 128).T),
        'i16': f(np.eye(16)), 'i128': f(np.eye(128)),
        'ffn1t': f(inputs['ffn_w1'].T), 'ffnb1': f(inputs['ffn_b1'][:, None]),
        'ffn2t': f(inputs['ffn_w2'].T),
    }
    xp = np.pad(inputs['x_continuum'], ((0, 0), (NFFT // 2, NFFT // 2)), mode='reflect')
    s0, s1 = xp.strides
    frames = np.lib.stride_tricks.as_strided(xp, (B, T, NFFT), (s0, 128 * s1, s1))
    xnp = np.pad(inputs['x_normalized'], ((0, 0), (3, 3 + 10)))
    in_maps = []
    for c in range(N_CORES):
        m = dict(shared)
        fr = frames[c * BC:(c + 1) * BC]
        m['framesT'] = f(np.transpose(fr, (2, 0, 1)).reshape(NFFT, BC * T))
        xc = xnp[c * BC:(c + 1) * BC]  # [16, 4112]
        x7b = np.zeros((3, 96, 8, 520), np.float32)
        for u in range(8):
            for s in range(2):
                for jj in range(7):
                    r = (u % 3) * 32 + s * 7 + jj
                    for gg in range(8):
                        x7b[u // 3, r, gg, :] = xc[gg * 2 + s, u * 512 + jj:u * 512 + jj + 520]
        m['x7b'] = np.asarray(x7b, dtype=ml_dtypes.bfloat16)
        in_maps.append(m)
    return in_maps


def _apply_tile_patch():
    from concourse import tile, mybir
    from concourse.vector_clock import ScopedClock

    def _drain_split(self, tick_clock, wait_clock):
        nc2 = self.nc
        di = nc2.sync.drain()
        wait_clock.add_sem_waits(di.ins, ScopedClock({None: tick_clock.global_clock}))
        si = di.ins.sync_info
        if si is not None and len(si.on_wait) > 1:
            waits = list(si.on_wait)
            ups = list(si.on_update)
            di.ins.sync_info = mybir.SyncInfo(on_wait=waits[:1], on_update=[])
            for kk, w in enumerate(waits[1:]):
                extra = nc2.sync.drain()
                extra.ins.sync_info = mybir.SyncInfo(
                    on_wait=[w], on_update=ups if kk == len(waits) - 2 else [])
        nc2.all_engine_barrier()
        assert self.sems is not None
        popped = nc2._tile_sem_poison_stack.pop()
        assert popped is self._sem_poison
        nc2.clear_and_free_semaphores(list(self.sems.allocated().values()))
        nc2.all_engine_barrier()

    tile.TileContext._drain_and_barrier = _drain_split


def kernel(**inputs):
    global _cache
    if 'nc' not in _cache:
        _apply_tile_patch()
        _cache['nc'] = _build(float(np.asarray(inputs['ffn_b2']).reshape(-1)[0]))
    from concourse.bass_utils import run_bass_kernel_spmd
    in_maps = _host_prep(inputs)
    res = run_bass_kernel_spmd(_cache['nc'], in_maps, list(range(N_CORES)))
    out = np.concatenate([res.results[c]['yout'] for c in range(N_CORES)])
    return out[:, None].astype(np.float32)



# revision 28
# speedup vs baseline: 1.0358x; 1.0192x over previous
"""DualBranchMoENet on Trainium2 — 8-core data-parallel (16 samples/core).

Channels live on SBUF partitions, (batch, time) on the free dim. Heavy
matmuls run fp32r (1 cyc/row at N>=256). Convolutions contract (cin, tap)
on the PE partition axis via shifted access patterns; only expert conv1
(129 ch x 5 taps) materialises an im2col stack. The LSTM keeps its hidden
state transposed ([256c, 16b]) so h @ whh^T needs no input transpose;
h is re-transposed once per step on the PE. The backward LSTM output
hb[T-1] equals ONE cell evaluated at t=32 from the zero state.
"""
import sys
sys.path.insert(0, '/opt/trn_rl_repo')
import numpy as np
import ml_dtypes

N_CORES = 8
B = 128
BC = B // N_CORES
L = 4096
NFFT = 256
NF = 129
T = 33
NE = 8

_cache = {}

# LSTM gate reorder (torch i,f,g,o) -> (i,f,o,g) so the three sigmoid gates
# are contiguous and fuse into one activation instruction.
GPERM = np.concatenate([np.arange(0, 512), np.arange(768, 1024), np.arange(512, 768)])


def _interp_tables():
    coords = np.clip((np.arange(T) + 0.5) * (64.0 / T) - 0.5, 0.0, 63.0)
    lo = np.floor(coords).astype(np.int64)
    w = coords - lo
    runs = []
    a = 0
    while a < T:
        b = a + 1
        if b < T:
            step = lo[a + 1] - lo[a]
            while b < T and lo[b] - lo[b - 1] == step:
                b += 1
        runs.append((a, b - a, int(lo[a]), int(lo[a + 1] - lo[a]) if b - a >= 2 else 1))
        a = b
    return runs, lo, w


def _build(ffn_b2_val):
    from concourse import bass, tile, mybir
    from concourse.mybir import AluOpType as alu
    from concourse.mybir import ActivationFunctionType as actf

    f32 = mybir.dt.float32
    f32r = mybir.dt.float32r
    bf16 = mybir.dt.bfloat16
    X = mybir.AxisListType.X

    BF16_IN = {'w1p', 'w2p', 'x7b', 'w14r', 'w2Lr', 'w3L', 'wihft', 'wihbt'}
    nc = bass.Bass()
    inp = lambda name, shape: nc.declare_dram_parameter(
        name, list(shape), bf16 if name in BF16_IN else f32, isOutput=False)

    d = {}
    for name, shape in [
        ('framesT', [NFFT, BC * T]), ('x7b', [3, 96, 8, 520]),
        ('crw', [NFFT, NF]), ('ciw', [NFFT, NF]),
        ('gw1ta', [128, 128]), ('gw1tb', [1, 128]), ('gb1c', [128, 1]),
        ('gw2t', [128, NE]), ('gb2c', [NE, 1]), ('iota8', [BC, NE]), ('ones1', [1, 128]), ('sel8', [NE, NE * 128]), ('zer', [128, 70]),
        ('w1p', [NE, 645, 256]), ('w1b', [256, NE]),
        ('w2p', [NE, 768, 256]), ('w2b', [256, NE]),
        ('w14r', [96, 128]), ('lb1c', [128, 1]),
        ('w2Lr', [128, 3, 128]), ('lb2c', [128, 1]),
        ('se2w1t', [128, 32]), ('se2w2t', [32, 128]),
        ('w3L', [3, 128, 256]), ('lb3c', [256, 1]),
        ('se3w1t', [256, 64]), ('se3w2t', [64, 256]),
        ('wlo', [1, T]),
        ('wihft', [512, 1024]), ('biasfT', [128, NE]), ('whhft', [256, 1024]),
        ('wihbt', [512, 1024]), ('biasbT', [128, NE]),
        ('i16', [16, 16]), ('i128', [128, 128]),
        ('ffn1t', [512, 256]), ('ffnb1', [256, 1]), ('ffn2t', [256, 1]),
    ]:
        d[name] = inp(name, shape)
    yout = nc.declare_dram_parameter('yout', [BC], f32, isOutput=True)

    runs, lo_t, w_t = _interp_tables()

    def mm(out, lhsT, rhs, start, stop):
        nc.tensor.matmul(out, lhsT, rhs, start=start, stop=stop)

    def mmr(out, lhsT, rhs, start, stop):
        nc.tensor.matmul(out, lhsT.bitcast(f32r), rhs.bitcast(f32r),
                         start=start, stop=stop)

    def mmf(out, lhsT, rhs, start, stop):
        nc.tensor.matmul(out, lhsT, rhs, start=start, stop=stop)

    with tile.TileContext(nc, num_cores=N_CORES) as tc:
        with (
            tc.tile_pool(name='const', bufs=1) as cp,
            tc.tile_pool(name='work', bufs=2) as wp,
            tc.tile_pool(name='one', bufs=1) as wp1,
            tc.tile_pool(name='big', bufs=1) as bp,
            tc.tile_pool(name='psA', bufs=2, space='PSUM') as psA,
            tc.tile_pool(name='psB', bufs=4, space='PSUM') as psB,
            tc.tile_pool(name='psC', bufs=2, space='PSUM') as psC,
        ):
            C = {}
            for name, shape, rr in [
                ('crw', [128, 2, NF], '(k p) m -> p k m'),
                ('ciw', [128, 2, NF], '(k p) m -> p k m'),
                ('gw1ta', [128, 128], None), ('gw1tb', [1, 128], None),
                ('gb1c', [128, 1], None),
                ('gw2t', [128, NE], None), ('gb2c', [NE, 1], None),
                ('iota8', [BC, NE], None),
                ('ones1', [1, 128], None),
                ('sel8', [NE, NE * 128], None),
                ('w1b', [128, 2, NE], '(k p) m -> p k m'),
                ('w2b', [128, 2, NE], '(k p) m -> p k m'),
                ('lb1c', [128, 1], None),
                ('lb2c', [128, 1], None),
                ('se2w1t', [128, 32], None), ('se2w2t', [32, 128], None),
                ('lb3c', [128, 2, 1], '(k p) m -> p k m'),
                ('se3w1t', [128, 2, 64], '(k p) m -> p k m'),
                ('se3w2t', [64, 256], None),
                ('wlo', [1, T], None),
                ('biasfT', [128, NE], None),
                ('whhft', [128, 2, 1024], '(k p) m -> p k m'),
                ('biasbT', [128, NE], None),
                ('i16', [16, 16], None), ('i128', [128, 128], None),
                ('ffn1t', [128, 4, 256], '(k p) m -> p k m'),
                ('ffnb1', [128, 2, 1], '(k p) m -> p k m'),
                ('ffn2t', [128, 2, 1], '(k p) m -> p k m'),
            ]:
                t = cp.tile(shape, f32, tag=name)
                src = d[name][:]
                if rr:
                    src = src.rearrange(rr, p=128)
                if name in ('crw', 'ciw', 'whhft', 'i128'):
                    nc.sync.dma_start(t[:].bitcast(f32r), src.bitcast(f32r))
                else:
                    nc.sync.dma_start(t[:], src)
                C[name] = t
            t = cp.tile([96, 128], bf16, tag='w14r')
            nc.sync.dma_start(t[:], d['w14r'][:])
            C['w14r'] = t
            t = cp.tile([128, 3, 128], bf16, tag='w2Lr')
            nc.sync.dma_start(t[:], d['w2Lr'][:])
            C['w2Lr'] = t
            t = cp.tile([128, 3, 256], bf16, tag='w3L')
            nc.sync.dma_start(t[:], d['w3L'][:].rearrange('d k m -> k d m'))
            C['w3L'] = t

            # ---------------- STFT magnitude ----------------
            lp_cm = tc.tile_pool(name='lp', bufs=2)
            lp = lp_cm.__enter__()
            ep_cm = tc.tile_pool(name='ep', bufs=2)
            ep = ep_cm.__enter__()
            c_fr = ep.tile([128, 2, BC * T], f32, tag='framesT', name='framesT', bufs=1)
            nc.sync.dma_start(c_fr[:].bitcast(f32r), d['framesT'][:].rearrange('(k p) m -> p k m', p=128).bitcast(f32r))
            C['framesT'] = c_fr
            magA = ep.tile([128, BC, T + 4], f32, tag='magA', name='magA', bufs=1)
            magB = ep.tile([1, BC, T + 4], f32, tag='magB', name='magB', bufs=1)
            nc.vector.memset(magA[:], 0.0)
            nc.vector.memset(magB[:], 0.0)
            NB2 = BC * T // 2
            for m0, mn, magX in [(0, 128, magA), (128, 1, magB)]:
                sqr = ep.tile([mn, BC * T], f32, tag=f'sqr{m0}', name=f'sqr{m0}', bufs=1)
                sqi = ep.tile([mn, BC * T], f32, tag=f'sqi{m0}', name=f'sqi{m0}', bufs=1)
                for ni in range(2):
                    pre = psA.tile([mn, NB2], f32, tag='pa', name='pa')
                    pim = psA.tile([mn, NB2], f32, tag='pa', name='pa')
                    for k in range(2):
                        co = slice(ni * NB2, (ni + 1) * NB2)
                        mmr(pre[:], C['crw'][:, k, m0:m0 + mn], C['framesT'][:, k, co], k == 0, k == 1)
                        mmr(pim[:], C['ciw'][:, k, m0:m0 + mn], C['framesT'][:, k, co], k == 0, k == 1)
                    nc.scalar.square(sqr[:, ni * NB2:(ni + 1) * NB2], pre[:])
                    nc.scalar.square(sqi[:, ni * NB2:(ni + 1) * NB2], pim[:])
                nc.vector.tensor_add(sqr[:], sqr[:], sqi[:])
                nc.scalar.sqrt(magX[0:mn, :, 2:2 + T],
                               sqr[:].rearrange('p (b t) -> p b t', b=BC))

            magAb = ep.tile([128, BC, T + 4], bf16, tag='magAb', name='magAb', bufs=1)
            magBb = ep.tile([1, BC, T + 4], bf16, tag='magBb', name='magBb', bufs=1)
            nc.scalar.activation(magAb[:], magA[:], actf.Copy)
            nc.scalar.activation(magBb[:], magB[:], actf.Copy)

            # ---------------- gating (fp32 matmuls) ----------------
            pooledA = ep.tile([128, BC], f32, tag='pooledA', name='pooledA')
            pooledB = ep.tile([1, BC], f32, tag='pooledB', name='pooledB')
            nc.vector.tensor_reduce(pooledA[:], magA[:, :, 2:2 + T], X, alu.add)
            nc.vector.tensor_reduce(pooledB[:], magB[:, :, 2:2 + T], X, alu.add)
            pg1 = psA.tile([128, BC], f32, tag='pa', name='pa')
            mmf(pg1[:], C['gw1ta'][:], pooledA[:], True, False)
            mmf(pg1[:], C['gw1tb'][:], pooledB[:], False, True)
            gh = ep.tile([128, BC], f32, tag='gh', name='gh')
            nc.scalar.activation(gh[:], pg1[:], actf.Relu, bias=C['gb1c'][:, 0:1])
            pg2 = psA.tile([NE, BC], f32, tag='pa', name='pa')
            mmf(pg2[:], C['gw2t'][:], gh[:], True, True)
            logitsT = ep.tile([NE, BC], f32, tag='logitsT', name='logitsT')
            nc.vector.tensor_tensor(logitsT[:], pg2[:],
                                    C['gb2c'][:, 0:1].to_broadcast([NE, BC]), alu.add)
            plg = psA.tile([BC, NE], f32, tag='pa', name='pa')
            nc.tensor.transpose(plg[:], logitsT[:], C['i16'][0:NE, 0:NE])
            lg = ep.tile([BC, NE], f32, tag='lg', name='lg')
            nc.vector.tensor_copy(lg[:], plg[:])
            iob = C['iota8'][:]

            def small(tag, shape=(BC, NE)):
                return ep.tile(list(shape), f32, tag=tag, name=tag)

            m1 = small('m1', (BC, 1))
            nc.vector.tensor_reduce(m1[:], lg[:], X, alu.max)
            eq1 = small('eq1')
            nc.vector.tensor_tensor(eq1[:], lg[:], m1[:].to_broadcast([BC, NE]), alu.is_equal)
            l2 = small('l2')
            nc.vector.scalar_tensor_tensor(l2[:], eq1[:], -1e30, lg[:], alu.mult, alu.add)
            m2 = small('m2', (BC, 1))
            nc.vector.tensor_reduce(m2[:], l2[:], X, alu.max)
            it1 = small('it1')
            nc.vector.tensor_tensor(it1[:], eq1[:], iob, alu.mult)
            idx1 = small('idx1', (BC, 1))
            nc.vector.tensor_reduce(idx1[:], it1[:], X, alu.max)
            eq2 = small('eq2')
            nc.vector.tensor_tensor(eq2[:], l2[:], m2[:].to_broadcast([BC, NE]), alu.is_equal)
            it2 = small('it2')
            nc.vector.tensor_tensor(it2[:], eq2[:], iob, alu.mult)
            idx2 = small('idx2', (BC, 1))
            nc.vector.tensor_reduce(idx2[:], it2[:], X, alu.max)
            dm = small('dm', (BC, 1))
            nc.vector.tensor_sub(dm[:], m1[:], m2[:])
            g1 = small('g1', (BC, 1))
            nc.scalar.activation(g1[:], dm[:], actf.Sigmoid)
            g2 = small('g2', (BC, 1))
            nc.vector.tensor_scalar(g2[:], g1[:], -1.0, 1.0, alu.mult, alu.add)
            eA = small('eA')
            nc.vector.tensor_tensor(eA[:], idx1[:].to_broadcast([BC, NE]), iob, alu.is_equal)
            eB = small('eB')
            nc.vector.tensor_tensor(eB[:], idx2[:].to_broadcast([BC, NE]), iob, alu.is_equal)
            tA = small('tA')
            nc.vector.tensor_tensor(tA[:], eA[:], g1[:].to_broadcast([BC, NE]), alu.mult)
            tB = small('tB')
            nc.vector.tensor_tensor(tB[:], eB[:], g2[:].to_broadcast([BC, NE]), alu.mult)
            W8 = small('W8')
            nc.vector.tensor_add(W8[:], tA[:], tB[:])
            pW8T = psA.tile([NE, BC], f32, tag='pa', name='pa')
            nc.tensor.transpose(pW8T[:], W8[:], C['i16'][:])
            W8T = ep.tile([NE, BC], f32, tag='W8T', name='W8T')
            nc.vector.tensor_copy(W8T[:], pW8T[:])

            # ---------------- line conv1 (emitted early: overlaps expert DMA) ----
            h1 = lp.tile([128, 8, 1028], bf16, tag='h1', name='h1', bufs=1)
            nc.gpsimd.memset(h1[:, :, 0:2], 0.0)
            nc.gpsimd.memset(h1[:, :, 1026:1028], 0.0)
            x7 = [ep.tile([96, 8, 520], bf16, tag=f'x7_{h}', name=f'x7_{h}', bufs=1)
                  for h in range(3)]
            for h in range(3):
                nc.sync.dma_start(x7[h][:], d['x7b'][h])
            for u in range(8):
                ub = (u % 3) * 32
                for gg in range(8):
                    pl1 = psC.tile([128, 512], f32, tag='pc', name='pc')
                    mm(pl1[:], C['w14r'][ub:ub + 32, :],
                       x7[u // 3][ub:ub + 32, gg, 0:512], True, True)
                    o0 = 2 + u * 128
                    nc.vector.tensor_reduce(h1[:, gg, o0:o0 + 128],
                                            pl1[:].rearrange('p (t q) -> p t q', q=4),
                                            X, alu.max)
            nc.scalar.activation(h1[:, :, 2:1026], h1[:, :, 2:1026], actf.Relu,
                                 bias=C['lb1c'][:, 0:1])

            # ---------------- experts (dense, weighted sum) ----------------
            imt = [ep.tile([128 if k < 5 else 5, BC, T], bf16, tag=f'im1_{k}', name=f'im1_{k}', bufs=1)
                   for k in range(6)]
            for dt in range(5):
                pos = dt * NF
                done = 0
                while done < NF:
                    k, r = divmod(pos + done, 128)
                    if done < 128:
                        n = min(128 - r, NF - done, 128 - done)
                        nc.sync.dma_start(imt[k][r:r + n],
                                          magAb[done:done + n, :, dt:dt + T])
                    else:
                        n = 1
                        nc.sync.dma_start(imt[k][r:r + 1], magBb[0:1, :, dt:dt + T])
                    done += n
            accF = [bp.tile([128, BC, T], f32, tag=f'accF{i}', name=f'accF{i}') for i in range(2)]
            H = BC // 2
            for e in range(NE):
                w1s = ep.tile([128, 6, 256], bf16, tag='w1s', name='w1s')
                nc.sync.dma_start(w1s[:, 0:5, :],
                                  d['w1p'][e, 0:640, :].rearrange('(k p) m -> p k m', p=128))
                nc.sync.dma_start(w1s[0:5, 5, :], d['w1p'][e, 640:645, :])
                he = [ep.tile([128, BC * (T + 2) + 2], bf16, tag=f'he{i}', name=f'he{i}') for i in range(2)]
                for i in range(2):
                    hv = he[i][:, 0:BC * (T + 2)].rearrange('p (b t) -> p b t', t=T + 2)
                    nc.gpsimd.memset(hv[:, :, 0:1], 0.0)
                    nc.gpsimd.memset(hv[:, :, T + 1:T + 2], 0.0)
                    nc.gpsimd.memset(he[i][:, BC * (T + 2):], 0.0)
                for mi in range(2):
                    for ni in range(2):
                        pe1 = psB.tile([128, H * T], f32, tag='pb', name='pb')
                        for k in range(6):
                            kn = 128 if k < 5 else 5
                            mm(pe1[:], w1s[0:kn, k, mi * 128:(mi + 1) * 128],
                               imt[k][:].rearrange('p b t -> p (b t)')[:, ni * H * T:(ni + 1) * H * T],
                               k == 0, k == 5)
                        nc.scalar.activation(he[mi][:, 0:BC * (T + 2)].rearrange('p (b t) -> p b t', t=T + 2)[:, ni * H:(ni + 1) * H, 1:1 + T],
                                             pe1[:].rearrange('p (b t) -> p b t', t=T),
                                             actf.Relu, bias=C['w1b'][:, mi, e:e + 1])
                w2s = ep.tile([128, 6, 256], bf16, tag='w2s', name='w2s')
                nc.sync.dma_start(w2s[:], d['w2p'][e].rearrange('(k p) m -> p k m', p=128))
                eo = [ep.tile([128, BC, T], f32, tag=f'eo{i}', name=f'eo{i}', bufs=1) for i in range(2)]
                W2 = T + 2
                for mi in range(2):
                    for bi in range(2):
                        pe2 = psB.tile([128, H * W2], f32, tag='pb', name='pb')
                        for k in range(6):
                            dt, ch = divmod(k, 2)
                            mm(pe2[:], w2s[:, k, mi * 128:(mi + 1) * 128],
                               he[ch][:, bi * H * W2 + dt:bi * H * W2 + dt + H * W2],
                               k == 0, k == 5)
                        nc.scalar.activation(eo[mi][:, bi * H:(bi + 1) * H, :],
                                             pe2[:].rearrange('p (b t) -> p b t', t=W2)[:, :, 0:T],
                                             actf.Relu, bias=C['w2b'][:, mi, e:e + 1])
                pwe = psA.tile([128, BC], f32, tag='pa', name='pwe')
                mmf(pwe[:], C['sel8'][:, e * 128:(e + 1) * 128], W8T[:], True, True)
                wE = ep.tile([128, BC], f32, tag='wE', name='wE')
                nc.vector.tensor_copy(wE[:], pwe[:])
                wbc = wE[:].unsqueeze(2).to_broadcast([128, BC, T])
                for mi in range(2):
                    if e == 0:
                        nc.vector.tensor_tensor(accF[mi][:].bitcast(f32r), eo[mi][:], wbc, alu.mult)
                    else:
                        eow = ep.tile([128, BC, T], f32, tag='eow', name='eow')
                        nc.vector.tensor_tensor(eow[:], eo[mi][:], wbc, alu.mult)
                        nc.vector.tensor_add(accF[mi][:].bitcast(f32r), accF[mi][:], eow[:])

            ep_cm.__exit__(None, None, None)

            # ---------------- line branch ----------------

            hp2 = lp.tile([128, BC, 258], bf16, tag='hp2', name='hp2', bufs=1)
            nc.gpsimd.memset(hp2[:, :, 0:1], 0.0)
            nc.gpsimd.memset(hp2[:, :, 257:258], 0.0)
            hp2v = hp2[:, :, 1:257].rearrange('p (g s) t -> p s g t', s=2)
            seacc = lp.tile([128, 2, 8, 2], f32, tag='seacc', name='seacc', bufs=1)
            hs = [lp.tile([128, 8, 1028], bf16, tag=f'hs{s}', name=f'hs{s}', bufs=1)
                  for s in range(2)]
            for s in range(2):
                nc.sync.dma_start(hs[s][0:64], h1[s * 64:(s + 1) * 64])
                nc.sync.dma_start(hs[s][64:128, :, 0:1027], h1[s * 64:(s + 1) * 64, :, 1:1028])
            for s in range(2):
                for gg in range(8):
                    for uh in range(2):
                        pl2 = psB.tile([128, 512], f32, tag='pb', name='pb')
                        base = uh * 512
                        mm(pl2[:], C['w2Lr'][:, 0, :], hs[s][:, gg, base:base + 512],
                           True, False)
                        mm(pl2[:], C['w2Lr'][:, 1, :], hs[s][:, gg, base + 2:base + 514],
                           False, False)
                        mm(pl2[:], C['w2Lr'][0:64, 2, :], hs[s][0:64, gg, base + 4:base + 516],
                           False, True)
                        r2 = lp.tile([128, 512], f32, tag='r2', name='r2')
                        nc.scalar.activation(r2[:], pl2[:], actf.Relu, bias=C['lb2c'][:, 0:1],
                                             accum_out=seacc[:, s, gg, uh:uh + 1])
                        nc.vector.tensor_reduce(hp2v[:, s, gg, uh * 128:uh * 128 + 128],
                                                r2[:].rearrange('p (w q) -> p w q', q=4),
                                                X, alu.max)
            seY = lp.tile([128, 2, 8], f32, tag='seY', name='seY')
            nc.vector.tensor_reduce(seY[:], seacc[:], X, alu.add)
            pse1 = psA.tile([32, 16], f32, tag='pa', name='pa')
            mmf(pse1[:], C['se2w1t'][:], seY[:].rearrange('p s g -> p (s g)'), True, True)
            z2 = lp.tile([32, 16], f32, tag='z2', name='z2')
            nc.scalar.activation(z2[:], pse1[:], actf.Relu)
            pse2 = psA.tile([128, 16], f32, tag='pa', name='pa')
            mmf(pse2[:], C['se2w2t'][:], z2[:], True, True)
            sc2 = lp.tile([128, 2, 8], f32, tag='sc2', name='sc2')
            nc.scalar.activation(sc2[:].rearrange('p s g -> p (s g)'), pse2[:], actf.Sigmoid)
            nc.vector.tensor_tensor(
                hp2[:, :, 1:257].rearrange('p (g s) t -> p g s t', s=2),
                hp2[:, :, 1:257].rearrange('p (g s) t -> p g s t', s=2),
                sc2[:].rearrange('p s g -> p g s').unsqueeze(3).to_broadcast([128, 8, 2, 256]),
                alu.mult)

            # conv3 + SE3 + pool, chunked over sample pairs
            y3 = lp.tile([128, 2, BC], f32, tag='y3', name='y3')
            lf = [lp.tile([128, BC, 64], bf16, tag=f'lf{i}', name=f'lf{i}', bufs=1) for i in range(2)]
            hp2f = hp2[:].rearrange('p b t -> p (b t)')
            for mi in range(2):
                for b0 in range(0, BC, 2):
                    pl3 = psB.tile([128, 2, 256], f32, tag='pb', name='pb')
                    for bi in (b0, b0 + 1):
                        for dt in range(3):
                            mm(pl3[:, bi - b0, :], C['w3L'][:, dt, mi * 128:(mi + 1) * 128],
                               hp2f[:, bi * 258 + dt:bi * 258 + dt + 256], dt == 0, dt == 2)
                    r3 = lp.tile([128, 2, 256], f32, tag='r3', name='r3')
                    nc.scalar.activation(r3[:], pl3[:],
                                         actf.Relu, bias=C['lb3c'][:, mi, 0:1])
                    nc.vector.tensor_reduce(y3[:, mi, b0:b0 + 2], r3[:], X, alu.add)
                    nc.vector.tensor_reduce(lf[mi][:, b0:b0 + 2, :],
                                            r3[:].rearrange('p c (u q) -> p c u q', q=4),
                                            X, alu.max)
            pse3 = psA.tile([64, 16], f32, tag='pa', name='pa')
            for k in range(2):
                mmf(pse3[:], C['se3w1t'][:, k, :], y3[:, k, :], k == 0, k == 1)
            z3 = lp.tile([64, 16], f32, tag='z3', name='z3')
            nc.scalar.activation(z3[:], pse3[:], actf.Relu)
            sc3 = [lp.tile([128, BC], f32, tag=f'sc3_{i}', name=f'sc3_{i}') for i in range(2)]
            for mi in range(2):
                pse4 = psA.tile([128, 16], f32, tag='pa', name='pa')
                mmf(pse4[:], C['se3w2t'][:, mi * 128:(mi + 1) * 128], z3[:], True, True)
                nc.scalar.activation(sc3[mi][:], pse4[:], actf.Sigmoid)
            for mi in range(2):
                nc.vector.tensor_tensor(lf[mi][:], lf[mi][:],
                                        sc3[mi][:].unsqueeze(2).to_broadcast([128, BC, 64]),
                                        alu.mult)
            # interp 64 -> 33
            li = [bp.tile([128, BC, T], f32, tag=f'li{i}', name=f'li{i}') for i in range(2)]
            pwl = psA.tile([128, T], f32, tag='pa', name='pwl')
            mmf(pwl[:], C['ones1'][:], C['wlo'][:], True, True)
            wlo128 = lp.tile([128, T], f32, tag='wlo128', name='wlo128', bufs=1)
            nc.vector.tensor_copy(wlo128[:], pwl[:])
            wbc_all = wlo128[:]
            for mi in range(2):
                for (a, n, lo0, st) in runs:
                    end = lo0 + (n - 1) * st + 1
                    lov = lf[mi][:, :, lo0:end:st]
                    hiv = lf[mi][:, :, lo0 + 1:end + 1:st]
                    dd = lp.tile([128, BC, T], f32, tag='dd', name='dd')
                    nc.vector.tensor_sub(dd[:, :, a:a + n], hiv, lov)
                    nc.vector.tensor_tensor(dd[:, :, a:a + n], dd[:, :, a:a + n],
                                            wbc_all[:, a:a + n].unsqueeze(1).to_broadcast([128, BC, n]),
                                            alu.mult)
                    nc.vector.tensor_add(li[mi][:, :, a:a + n].bitcast(f32r), dd[:, :, a:a + n], lov)

            lp_cm.__exit__(None, None, None)

            # ---------------- LSTM input precompute (transposed) ----------------
            # XsT[p, jj, t, b] = (x_t @ wih^T + bias)[b, jj*128+p], gate order (i,f,o,g)
            ct = [accF[0], accF[1], li[0], li[1]]
            xp_cm = tc.tile_pool(name='xp', bufs=2)
            xpp = xp_cm.__enter__()
            XsT = bp.tile([128, 8, T, 16], f32, tag='XsT', name='XsT')
            wft = [xpp.tile([128, 1024], bf16, tag=f'wft{k}', name=f'wft{k}', bufs=1)
                   for k in range(4)]
            xtT = [xpp.tile([128, T, 16], bf16, tag=f'xtT{k}', name=f'xtT{k}', bufs=1)
                   for k in range(4)]
            for k in range(4):
                nc.sync.dma_start(wft[k][:], d['wihft'][k * 128:(k + 1) * 128, :])
                nc.vector.tensor_copy(xtT[k][:], ct[k][:, :, :].transpose([0, 2, 1]))
            for jj in range(8):
                for (t0, tl) in ((0, 16), (16, 17)):
                    ps = psB.tile([128, tl * 16], f32, tag='pb', name='pb')
                    for k in range(4):
                        mm(ps[:], wft[k][:, jj * 128:(jj + 1) * 128],
                           xtT[k][:, t0:t0 + tl, :].rearrange('p t b -> p (t b)'),
                           k == 0, k == 3)
                    nc.scalar.activation(
                        XsT[:, jj, t0:t0 + tl, :].rearrange('p t b -> p (t b)').bitcast(f32r),
                        ps[:], actf.Identity, bias=C['biasfT'][:, jj:jj + 1])

            # ---------------- backward cell (t=32, transposed) ----------------
            wbt = [xpp.tile([128, 1024], bf16, tag=f'wft{k}', name=f'wbt{k}', bufs=1)
                   for k in range(4)]
            for k in range(4):
                nc.sync.dma_start(wbt[k][:], d['wihbt'][k * 128:(k + 1) * 128, :])
            psb = psA.tile([128, 8, 16], f32, tag='pa', name='psb')
            for jj in range(8):
                for k in range(4):
                    mm(psb[:, jj, :], wbt[k][:, jj * 128:(jj + 1) * 128],
                       xtT[k][:, 32, :], k == 0, k == 3)
            gbT = wp1.tile([128, 8, 16], f32, tag='gbT', name='gbT')
            for jj in range(8):
                nc.scalar.activation(gbT[:, jj, :], psb[:, jj, :], actf.Identity,
                                     bias=C['biasbT'][:, jj:jj + 1])
            sgb = wp1.tile([128, 8, 16], f32, tag='sgb', name='sgb')
            nc.scalar.activation(sgb[:, 0:6, :], gbT[:, 0:6, :], actf.Sigmoid)
            nc.scalar.activation(sgb[:, 6:8, :], gbT[:, 6:8, :], actf.Tanh)
            cbT = wp1.tile([128, 2, 16], f32, tag='cbT', name='cbT')
            nc.vector.tensor_tensor(cbT[:], sgb[:, 0:2, :], sgb[:, 6:8, :], alu.mult)
            tcb = wp1.tile([128, 2, 16], f32, tag='tcb', name='tcb')
            nc.scalar.activation(tcb[:], cbT[:], actf.Tanh)
            hbT = bp.tile([128, 2, 16], f32, tag='hbT', name='hbT')
            nc.vector.tensor_tensor(hbT[:], sgb[:, 4:6, :], tcb[:], alu.mult)
            xp_cm.__exit__(None, None, None)

            # ---------------- forward LSTM (33 steps, transposed) ----------------
            # gates live as [128 = j-chunk, jj, 16 = batch]; no per-step transposes.
            hT = None
            cT = None
            for t in range(T):
                psg = psB.tile([128, 8, 16], f32, tag='pb', name='psg')
                for jj in range(8):
                    mmr(psg[:, jj, :], C['i128'][:], XsT[:, jj, t, :], True, t == 0)
                    if t > 0:
                        for k in range(2):
                            mmr(psg[:, jj, :], C['whhft'][:, k, jj * 128:(jj + 1) * 128],
                                hT[:, k, :], False, k == 1)
                sg = wp.tile([128, 8, 16], f32, tag='lstm_sg', name='lstm_sg')
                nc.scalar.activation(sg[:, 0:6, :], psg[:, 0:6, :], actf.Sigmoid)
                nc.scalar.activation(sg[:, 6:8, :], psg[:, 6:8, :], actf.Tanh)
                cT_new = wp.tile([128, 2, 16], f32, tag='lstm_c', name='lstm_c')
                if t == 0:
                    nc.vector.tensor_tensor(cT_new[:], sg[:, 0:2, :], sg[:, 6:8, :], alu.mult)
                else:
                    t1 = wp.tile([128, 2, 16], f32, tag='lstm_t1', name='lstm_t1')
                    nc.vector.tensor_tensor(t1[:], sg[:, 2:4, :], cT[:], alu.mult)
                    t2 = wp.tile([128, 2, 16], f32, tag='lstm_t2', name='lstm_t2')
                    nc.vector.tensor_tensor(t2[:], sg[:, 0:2, :], sg[:, 6:8, :], alu.mult)
                    nc.vector.tensor_tensor(cT_new[:], t1[:], t2[:], alu.add)
                cT = cT_new
                tct = wp.tile([128, 2, 16], f32, tag='lstm_tc', name='lstm_tc')
                nc.scalar.activation(tct[:], cT[:], actf.Tanh)
                hT_new = wp.tile([128, 2, 16], f32, tag='lstm_h', name='lstm_h')
                nc.vector.tensor_tensor(hT_new[:].bitcast(f32r), sg[:, 4:6, :], tct[:],
                                        alu.mult)
                hT = hT_new

            # ---------------- FFN head ----------------
            lastT = [hT[:, 0, :], hT[:, 1, :], hbT[:, 0, :], hbT[:, 1, :]]
            z = [wp1.tile([128, 16], f32, tag=f'z_{i}', name=f'z_{i}') for i in range(2)]
            for mi in range(2):
                pz = psA.tile([128, 16], f32, tag='pa', name='pa')
                for k in range(4):
                    mmf(pz[:], C['ffn1t'][:, k, mi * 128:(mi + 1) * 128], lastT[k],
                        k == 0, k == 3)
                nc.scalar.activation(z[mi][:], pz[:], actf.Relu,
                                     bias=C['ffnb1'][:, mi, 0:1])
            py = psA.tile([1, 16], f32, tag='pa', name='pa')
            for k in range(2):
                mmf(py[:], C['ffn2t'][:, k, :], z[k][:], k == 0, k == 1)
            yo = wp1.tile([1, 16], f32, tag='yo', name='yo')
            nc.scalar.activation(yo[:], py[:], actf.Copy, bias=float(ffn_b2_val))
            nc.sync.dma_start(yout[:].unsqueeze(0), yo[:])

    _split_tpb_waits(nc)
    return nc


def _split_tpb_waits(nc, max_waits=1):
    """This walrus build caps sync-waits per TPB instruction; hoist extras
    onto same-engine NoOps placed immediately before the instruction."""
    from concourse import mybir
    dma_ops = ('DMACopy', 'DMATranspose', 'TensorLoad', 'TensorSave')
    cnt = 0
    for f in nc.m.functions:
        for bb in f.blocks:
            out = []
            changed = False
            for inst in bb.instructions:
                si = inst.sync_info
                opc = getattr(inst, 'opcode', '') or type(inst).__name__
                if (si is not None and len(si.on_wait) > max_waits
                        and getattr(inst, 'engine', None) is not None):
                    waits = list(si.on_wait)
                    for w in waits[:-max_waits]:
                        nop = mybir.InstNoOp(name=f'{inst.name}-sw{cnt}', ins=[], outs=[])
                        cnt += 1
                        nop.engine = inst.engine
                        nop.sync_info = mybir.SyncInfo(on_wait=[w], on_update=[])
                        out.append(nop)
                    inst.sync_info = mybir.SyncInfo(on_wait=waits[-max_waits:],
                                                    on_update=list(si.on_update))
                    changed = True
                out.append(inst)
            if changed:
                bb.instructions = out
    return nc


def _host_prep(inputs):
    f = lambda x: np.ascontiguousarray(x, dtype=np.float32)
    n = np.arange(NFFT)
    win = 0.5 * (1.0 - np.cos(2.0 * np.pi * n / NFFT))
    k = np.arange(NF)
    ang = 2.0 * np.pi * np.outer(n, k) / NFFT
    gw1t = inputs['gate_w1'].T / T
    runs, lo_t, w_t = _interp_tables()
    w14 = np.zeros((14, 128), np.float32)
    for s in range(2):
        for jj in range(7):
            w14[s * 7 + jj, s * 64:(s + 1) * 64] = inputs['lw1'][:, 0, jj]
    w14r = np.concatenate([np.concatenate([w14, np.zeros((18, 128), np.float32)])] * 3)
    wt = np.transpose(inputs['lw2'], (1, 2, 0))  # [64ch, 5dt, 128oc]
    w2Lr = np.zeros((128, 3, 128), np.float32)
    for c in range(3):
        w2Lr[0:64, c, :] = wt[:, 2 * c, :]
        if c < 2:
            w2Lr[64:128, c, :] = wt[:, 2 * c + 1, :]
    shared = {
        'crw': f(win[:, None] * np.cos(ang)),
        'ciw': f(win[:, None] * np.sin(ang)),
        'gw1ta': f(gw1t[0:128]), 'gw1tb': f(gw1t[128:129]),
        'gb1c': f(inputs['gate_b1'][:, None]),
        'gw2t': f(inputs['gate_w2'].T), 'gb2c': f(inputs['gate_b2'][:, None]),
        'iota8': f(np.tile(np.arange(NE)[None, :], (BC, 1))),
        'ones1': f(np.ones((1, 128))),
        'zer': f(np.zeros((128, 70))),
        'sel8': f(np.concatenate([np.tile(v[:, None], (1, 128)) for v in np.eye(NE)], axis=1)),
        'w1p': np.asarray(np.transpose(inputs['exp_w1'], (0, 3, 2, 1)).reshape(NE, 645, 256), dtype=ml_dtypes.bfloat16),
        'w1b': f(inputs['exp_b1'].T),
        'w2p': np.asarray(np.transpose(inputs['exp_w2'], (0, 3, 2, 1)).reshape(NE, 768, 256), dtype=ml_dtypes.bfloat16),
        'w2b': f(inputs['exp_b2'].T),
        'w14r': np.asarray(w14r, dtype=ml_dtypes.bfloat16), 'lb1c': f(np.tile(inputs['lb1'], 2)[:, None]),
        'w2Lr': np.asarray(w2Lr, dtype=ml_dtypes.bfloat16),
        'lb2c': f(inputs['lb2'][:, None]),
        'se2w1t': f(inputs['se2_w1'].T / 1024.0), 'se2w2t': f(inputs['se2_w2'].T),
        'w3L': np.asarray(np.transpose(inputs['lw3'], (2, 1, 0)), dtype=ml_dtypes.bfloat16),
        'lb3c': f(inputs['lb3'][:, None]),
        'se3w1t': f(inputs['se3_w1'].T / 256.0), 'se3w2t': f(inputs['se3_w2'].T),
        'wlo': f(w_t[None, :]),
        'wihft': np.asarray(inputs['wih_f'].T[:, GPERM], dtype=ml_dtypes.bfloat16),
        'biasfT': f((inputs['bih_f'] + inputs['bhh_f'])[GPERM].reshape(NE, 128).T),
        'whhft': f(inputs['whh_f'].T[:, GPERM]),
        'wihbt': np.asarray(inputs['wih_b'].T[:, GPERM], dtype=ml_dtypes.bfloat16),
        'biasbT': f((inputs['bih_b'] + inputs['bhh_b'])[GPERM].reshape(NE, 128).T),
        'i16': f(np.eye(16)), 'i128': f(np.eye(128)),
        'ffn1t': f(inputs['ffn_w1'].T), 'ffnb1': f(inputs['ffn_b1'][:, None]),
        'ffn2t': f(inputs['ffn_w2'].T),
    }
    xp = np.pad(inputs['x_continuum'], ((0, 0), (NFFT // 2, NFFT // 2)), mode='reflect')
    s0, s1 = xp.strides
    frames = np.lib.stride_tricks.as_strided(xp, (B, T, NFFT), (s0, 128 * s1, s1))
    xnp = np.pad(inputs['x_normalized'], ((0, 0), (3, 3 + 10)))
    in_maps = []
    for c in range(N_CORES):
        m = dict(shared)
        fr = frames[c * BC:(c + 1) * BC]
        m['framesT'] = f(np.transpose(fr, (2, 0, 1)).reshape(NFFT, BC * T))
        xc = xnp[c * BC:(c + 1) * BC]  # [16, 4112]
        x7b = np.zeros((3, 96, 8, 520), np.float32)
        for u in range(8):
            for s in range(2):
                for jj in range(7):
                    r = (u % 3) * 32 + s * 7 + jj
                    for gg in range(8):
                        x7b[u // 3, r, gg, :] = xc[gg * 2 + s, u * 512 + jj:u * 512 + jj + 520]
        m['x7b'] = np.asarray(x7b, dtype=ml_dtypes.bfloat16)
        in_maps.append(m)
    return in_maps


def _apply_tile_patch():
    from concourse import tile, mybir
    from concourse.vector_clock import ScopedClock

    def _drain_split(self, tick_clock, wait_clock):
        nc2 = self.nc
        di = nc2.sync.drain()
        wait_clock.add_sem_waits(di.ins, ScopedClock({None: tick_clock.global_clock}))
        si = di.ins.sync_info
        if si is not None and len(si.on_wait) > 1:
            waits = list(si.on_wait)
            ups = list(si.on_update)
            di.ins.sync_info = mybir.SyncInfo(on_wait=waits[:1], on_update=[])
            for kk, w in enumerate(waits[1:]):
                extra = nc2.sync.drain()
                extra.ins.sync_info = mybir.SyncInfo(
                    on_wait=[w], on_update=ups if kk == len(waits) - 2 else [])
        nc2.all_engine_barrier()
        assert self.sems is not None
        popped = nc2._tile_sem_poison_stack.pop()
        assert popped is self._sem_poison
        nc2.clear_and_free_semaphores(list(self.sems.allocated().values()))
        nc2.all_engine_barrier()

    tile.TileContext._drain_and_barrier = _drain_split


def kernel(**inputs):
    global _cache
    if 'nc' not in _cache:
        _apply_tile_patch()
        _cache['nc'] = _build(float(np.asarray(inputs['ffn_b2']).reshape(-1)[0]))
    from concourse.bass_utils import run_bass_kernel_spmd
    in_maps = _host_prep(inputs)
    res = run_bass_kernel_spmd(_cache['nc'], in_maps, list(range(N_CORES)))
    out = np.concatenate([res.results[c]['yout'] for c in range(N_CORES)])
    return out[:, None].astype(np.float32)



# revision 30
# speedup vs baseline: 1.0390x; 1.0031x over previous
"""DualBranchMoENet on Trainium2 — 8-core data-parallel (16 samples/core).

Channels live on SBUF partitions, (batch, time) on the free dim. Heavy
matmuls run fp32r (1 cyc/row at N>=256). Convolutions contract (cin, tap)
on the PE partition axis via shifted access patterns; only expert conv1
(129 ch x 5 taps) materialises an im2col stack. The LSTM keeps its hidden
state transposed ([256c, 16b]) so h @ whh^T needs no input transpose;
h is re-transposed once per step on the PE. The backward LSTM output
hb[T-1] equals ONE cell evaluated at t=32 from the zero state.
"""
import sys
sys.path.insert(0, '/opt/trn_rl_repo')
import numpy as np
import ml_dtypes

N_CORES = 8
B = 128
BC = B // N_CORES
L = 4096
NFFT = 256
NF = 129
T = 33
NE = 8

_cache = {}

# LSTM gate reorder (torch i,f,g,o) -> (i,f,o,g) so the three sigmoid gates
# are contiguous and fuse into one activation instruction.
GPERM = np.concatenate([np.arange(0, 512), np.arange(768, 1024), np.arange(512, 768)])


def _interp_tables():
    coords = np.clip((np.arange(T) + 0.5) * (64.0 / T) - 0.5, 0.0, 63.0)
    lo = np.floor(coords).astype(np.int64)
    w = coords - lo
    runs = []
    a = 0
    while a < T:
        b = a + 1
        if b < T:
            step = lo[a + 1] - lo[a]
            while b < T and lo[b] - lo[b - 1] == step:
                b += 1
        runs.append((a, b - a, int(lo[a]), int(lo[a + 1] - lo[a]) if b - a >= 2 else 1))
        a = b
    return runs, lo, w


def _build(ffn_b2_val):
    from concourse import bass, tile, mybir
    from concourse.mybir import AluOpType as alu
    from concourse.mybir import ActivationFunctionType as actf

    f32 = mybir.dt.float32
    f32r = mybir.dt.float32r
    bf16 = mybir.dt.bfloat16
    X = mybir.AxisListType.X

    BF16_IN = {'w1p', 'w2p', 'x7b', 'w14r', 'w2Lr', 'w3L', 'wihft', 'wihbt', 'i128'}
    nc = bass.Bass()
    inp = lambda name, shape: nc.declare_dram_parameter(
        name, list(shape), bf16 if name in BF16_IN else f32, isOutput=False)

    d = {}
    for name, shape in [
        ('framesT', [NFFT, BC * T]), ('x7b', [3, 96, 8, 520]),
        ('crw', [NFFT, NF]), ('ciw', [NFFT, NF]),
        ('gw1ta', [128, 128]), ('gw1tb', [1, 128]), ('gb1c', [128, 1]),
        ('gw2t', [128, NE]), ('gb2c', [NE, 1]), ('iota8', [BC, NE]), ('ones1', [1, 128]), ('sel8', [NE, NE * 128]), ('zer', [128, 70]),
        ('w1p', [NE, 645, 256]), ('w1b', [256, NE]),
        ('w2p', [NE, 768, 256]), ('w2b', [256, NE]),
        ('w14r', [96, 128]), ('lb1c', [128, 1]),
        ('w2Lr', [128, 3, 128]), ('lb2c', [128, 1]),
        ('se2w1t', [128, 32]), ('se2w2t', [32, 128]),
        ('w3L', [3, 128, 256]), ('lb3c', [256, 1]),
        ('se3w1t', [256, 64]), ('se3w2t', [64, 256]),
        ('wlo', [1, T]),
        ('wihft', [512, 1024]), ('biasfT', [128, NE]), ('whhft', [256, 1024]),
        ('wihbt', [512, 1024]), ('biasbT', [128, NE]),
        ('i16', [16, 16]), ('i128', [128, 128]),
        ('ffn1t', [512, 256]), ('ffnb1', [256, 1]), ('ffn2t', [256, 1]),
    ]:
        d[name] = inp(name, shape)
    yout = nc.declare_dram_parameter('yout', [BC], f32, isOutput=True)

    runs, lo_t, w_t = _interp_tables()

    def mm(out, lhsT, rhs, start, stop):
        nc.tensor.matmul(out, lhsT, rhs, start=start, stop=stop)

    def mmr(out, lhsT, rhs, start, stop):
        nc.tensor.matmul(out, lhsT.bitcast(f32r), rhs.bitcast(f32r),
                         start=start, stop=stop)

    def mmf(out, lhsT, rhs, start, stop):
        nc.tensor.matmul(out, lhsT, rhs, start=start, stop=stop)

    with tile.TileContext(nc, num_cores=N_CORES) as tc:
        with (
            tc.tile_pool(name='const', bufs=1) as cp,
            tc.tile_pool(name='work', bufs=2) as wp,
            tc.tile_pool(name='one', bufs=1) as wp1,
            tc.tile_pool(name='big', bufs=1) as bp,
            tc.tile_pool(name='psA', bufs=2, space='PSUM') as psA,
            tc.tile_pool(name='psB', bufs=4, space='PSUM') as psB,
            tc.tile_pool(name='psC', bufs=2, space='PSUM') as psC,
        ):
            C = {}
            for name, shape, rr in [
                ('crw', [128, 2, NF], '(k p) m -> p k m'),
                ('ciw', [128, 2, NF], '(k p) m -> p k m'),
                ('gw1ta', [128, 128], None), ('gw1tb', [1, 128], None),
                ('gb1c', [128, 1], None),
                ('gw2t', [128, NE], None), ('gb2c', [NE, 1], None),
                ('iota8', [BC, NE], None),
                ('ones1', [1, 128], None),
                ('sel8', [NE, NE * 128], None),
                ('w1b', [128, 2, NE], '(k p) m -> p k m'),
                ('w2b', [128, 2, NE], '(k p) m -> p k m'),
                ('lb1c', [128, 1], None),
                ('lb2c', [128, 1], None),
                ('se2w1t', [128, 32], None), ('se2w2t', [32, 128], None),
                ('lb3c', [128, 2, 1], '(k p) m -> p k m'),
                ('se3w1t', [128, 2, 64], '(k p) m -> p k m'),
                ('se3w2t', [64, 256], None),
                ('wlo', [1, T], None),
                ('biasfT', [128, NE], None),
                ('whhft', [128, 2, 1024], '(k p) m -> p k m'),
                ('biasbT', [128, NE], None),
                ('i16', [16, 16], None),
                ('ffn1t', [128, 4, 256], '(k p) m -> p k m'),
                ('ffnb1', [128, 2, 1], '(k p) m -> p k m'),
                ('ffn2t', [128, 2, 1], '(k p) m -> p k m'),
            ]:
                t = cp.tile(shape, f32, tag=name)
                src = d[name][:]
                if rr:
                    src = src.rearrange(rr, p=128)
                if name in ('crw', 'ciw', 'whhft'):
                    nc.sync.dma_start(t[:].bitcast(f32r), src.bitcast(f32r))
                else:
                    nc.sync.dma_start(t[:], src)
                C[name] = t
            t = cp.tile([128, 128], bf16, tag='i128')
            nc.sync.dma_start(t[:], d['i128'][:])
            C['i128'] = t
            t = cp.tile([96, 128], bf16, tag='w14r')
            nc.sync.dma_start(t[:], d['w14r'][:])
            C['w14r'] = t
            t = cp.tile([128, 3, 128], bf16, tag='w2Lr')
            nc.sync.dma_start(t[:], d['w2Lr'][:])
            C['w2Lr'] = t
            t = cp.tile([128, 3, 256], bf16, tag='w3L')
            nc.sync.dma_start(t[:], d['w3L'][:].rearrange('d k m -> k d m'))
            C['w3L'] = t

            # ---------------- STFT magnitude ----------------
            lp_cm = tc.tile_pool(name='lp', bufs=2)
            lp = lp_cm.__enter__()
            xp_cm = tc.tile_pool(name='xp', bufs=2)
            xpp = xp_cm.__enter__()
            ep_cm = tc.tile_pool(name='ep', bufs=2)
            ep = ep_cm.__enter__()
            c_fr = ep.tile([128, 2, BC * T], f32, tag='framesT', name='framesT', bufs=1)
            nc.sync.dma_start(c_fr[:].bitcast(f32r), d['framesT'][:].rearrange('(k p) m -> p k m', p=128).bitcast(f32r))
            C['framesT'] = c_fr
            magA = ep.tile([128, BC, T + 4], f32, tag='magA', name='magA', bufs=1)
            magB = ep.tile([1, BC, T + 4], f32, tag='magB', name='magB', bufs=1)
            nc.vector.memset(magA[:], 0.0)
            nc.vector.memset(magB[:], 0.0)
            NB2 = BC * T // 2
            for m0, mn, magX in [(0, 128, magA), (128, 1, magB)]:
                sqr = ep.tile([mn, BC * T], f32, tag=f'sqr{m0}', name=f'sqr{m0}', bufs=1)
                sqi = ep.tile([mn, BC * T], f32, tag=f'sqi{m0}', name=f'sqi{m0}', bufs=1)
                for ni in range(2):
                    pre = psA.tile([mn, NB2], f32, tag='pa', name='pa')
                    pim = psA.tile([mn, NB2], f32, tag='pa', name='pa')
                    for k in range(2):
                        co = slice(ni * NB2, (ni + 1) * NB2)
                        mmr(pre[:], C['crw'][:, k, m0:m0 + mn], C['framesT'][:, k, co], k == 0, k == 1)
                        mmr(pim[:], C['ciw'][:, k, m0:m0 + mn], C['framesT'][:, k, co], k == 0, k == 1)
                    nc.scalar.square(sqr[:, ni * NB2:(ni + 1) * NB2], pre[:])
                    nc.scalar.square(sqi[:, ni * NB2:(ni + 1) * NB2], pim[:])
                nc.vector.tensor_add(sqr[:], sqr[:], sqi[:])
                nc.scalar.sqrt(magX[0:mn, :, 2:2 + T],
                               sqr[:].rearrange('p (b t) -> p b t', b=BC))

            magAb = ep.tile([128, BC, T + 4], bf16, tag='magAb', name='magAb', bufs=1)
            magBb = ep.tile([1, BC, T + 4], bf16, tag='magBb', name='magBb', bufs=1)
            nc.scalar.activation(magAb[:], magA[:], actf.Copy)
            nc.scalar.activation(magBb[:], magB[:], actf.Copy)

            # ---------------- gating (fp32 matmuls) ----------------
            pooledA = ep.tile([128, BC], f32, tag='pooledA', name='pooledA')
            pooledB = ep.tile([1, BC], f32, tag='pooledB', name='pooledB')
            nc.vector.tensor_reduce(pooledA[:], magA[:, :, 2:2 + T], X, alu.add)
            nc.vector.tensor_reduce(pooledB[:], magB[:, :, 2:2 + T], X, alu.add)
            pg1 = psA.tile([128, BC], f32, tag='pa', name='pa')
            mmf(pg1[:], C['gw1ta'][:], pooledA[:], True, False)
            mmf(pg1[:], C['gw1tb'][:], pooledB[:], False, True)
            gh = ep.tile([128, BC], f32, tag='gh', name='gh')
            nc.scalar.activation(gh[:], pg1[:], actf.Relu, bias=C['gb1c'][:, 0:1])
            pg2 = psA.tile([NE, BC], f32, tag='pa', name='pa')
            mmf(pg2[:], C['gw2t'][:], gh[:], True, True)
            logitsT = ep.tile([NE, BC], f32, tag='logitsT', name='logitsT')
            nc.vector.tensor_tensor(logitsT[:], pg2[:],
                                    C['gb2c'][:, 0:1].to_broadcast([NE, BC]), alu.add)
            plg = psA.tile([BC, NE], f32, tag='pa', name='pa')
            nc.tensor.transpose(plg[:], logitsT[:], C['i16'][0:NE, 0:NE])
            lg = ep.tile([BC, NE], f32, tag='lg', name='lg')
            nc.vector.tensor_copy(lg[:], plg[:])
            iob = C['iota8'][:]

            def small(tag, shape=(BC, NE)):
                return ep.tile(list(shape), f32, tag=tag, name=tag)

            m1 = small('m1', (BC, 1))
            nc.vector.tensor_reduce(m1[:], lg[:], X, alu.max)
            eq1 = small('eq1')
            nc.vector.tensor_tensor(eq1[:], lg[:], m1[:].to_broadcast([BC, NE]), alu.is_equal)
            l2 = small('l2')
            nc.vector.scalar_tensor_tensor(l2[:], eq1[:], -1e30, lg[:], alu.mult, alu.add)
            m2 = small('m2', (BC, 1))
            nc.vector.tensor_reduce(m2[:], l2[:], X, alu.max)
            it1 = small('it1')
            nc.vector.tensor_tensor(it1[:], eq1[:], iob, alu.mult)
            idx1 = small('idx1', (BC, 1))
            nc.vector.tensor_reduce(idx1[:], it1[:], X, alu.max)
            eq2 = small('eq2')
            nc.vector.tensor_tensor(eq2[:], l2[:], m2[:].to_broadcast([BC, NE]), alu.is_equal)
            it2 = small('it2')
            nc.vector.tensor_tensor(it2[:], eq2[:], iob, alu.mult)
            idx2 = small('idx2', (BC, 1))
            nc.vector.tensor_reduce(idx2[:], it2[:], X, alu.max)
            dm = small('dm', (BC, 1))
            nc.vector.tensor_sub(dm[:], m1[:], m2[:])
            g1 = small('g1', (BC, 1))
            nc.scalar.activation(g1[:], dm[:], actf.Sigmoid)
            g2 = small('g2', (BC, 1))
            nc.vector.tensor_scalar(g2[:], g1[:], -1.0, 1.0, alu.mult, alu.add)
            eA = small('eA')
            nc.vector.tensor_tensor(eA[:], idx1[:].to_broadcast([BC, NE]), iob, alu.is_equal)
            eB = small('eB')
            nc.vector.tensor_tensor(eB[:], idx2[:].to_broadcast([BC, NE]), iob, alu.is_equal)
            tA = small('tA')
            nc.vector.tensor_tensor(tA[:], eA[:], g1[:].to_broadcast([BC, NE]), alu.mult)
            tB = small('tB')
            nc.vector.tensor_tensor(tB[:], eB[:], g2[:].to_broadcast([BC, NE]), alu.mult)
            W8 = small('W8')
            nc.vector.tensor_add(W8[:], tA[:], tB[:])
            pW8T = psA.tile([NE, BC], f32, tag='pa', name='pa')
            nc.tensor.transpose(pW8T[:], W8[:], C['i16'][:])
            W8T = ep.tile([NE, BC], f32, tag='W8T', name='W8T')
            nc.vector.tensor_copy(W8T[:], pW8T[:])

            # ---------------- line conv1 (emitted early: overlaps expert DMA) ----
            h1 = lp.tile([128, 8, 1028], bf16, tag='h1', name='h1', bufs=1)
            nc.gpsimd.memset(h1[:, :, 0:2], 0.0)
            nc.gpsimd.memset(h1[:, :, 1026:1028], 0.0)
            x7 = [ep.tile([96, 8, 520], bf16, tag=f'x7_{h}', name=f'x7_{h}', bufs=1)
                  for h in range(3)]
            for h in range(3):
                nc.sync.dma_start(x7[h][:], d['x7b'][h])
            for u in range(8):
                ub = (u % 3) * 32
                for gg in range(8):
                    pl1 = psC.tile([128, 512], f32, tag='pc', name='pc')
                    mm(pl1[:], C['w14r'][ub:ub + 32, :],
                       x7[u // 3][ub:ub + 32, gg, 0:512], True, True)
                    o0 = 2 + u * 128
                    nc.vector.tensor_reduce(h1[:, gg, o0:o0 + 128],
                                            pl1[:].rearrange('p (t q) -> p t q', q=4),
                                            X, alu.max)
            nc.scalar.activation(h1[:, :, 2:1026], h1[:, :, 2:1026], actf.Relu,
                                 bias=C['lb1c'][:, 0:1])

            # ---------------- experts (dense, weighted sum) ----------------
            imt = [ep.tile([128 if k < 5 else 5, BC, T], bf16, tag=f'im1_{k}', name=f'im1_{k}', bufs=1)
                   for k in range(6)]
            for dt in range(5):
                pos = dt * NF
                done = 0
                while done < NF:
                    k, r = divmod(pos + done, 128)
                    if done < 128:
                        n = min(128 - r, NF - done, 128 - done)
                        nc.sync.dma_start(imt[k][r:r + n],
                                          magAb[done:done + n, :, dt:dt + T])
                    else:
                        n = 1
                        nc.sync.dma_start(imt[k][r:r + 1], magBb[0:1, :, dt:dt + T])
                    done += n
            accF = [bp.tile([128, BC, T], f32, tag=f'accF{i}', name=f'accF{i}') for i in range(2)]
            H = BC // 2
            for e in range(NE):
                w1s = ep.tile([128, 6, 256], bf16, tag='w1s', name='w1s')
                nc.sync.dma_start(w1s[:, 0:5, :],
                                  d['w1p'][e, 0:640, :].rearrange('(k p) m -> p k m', p=128))
                nc.sync.dma_start(w1s[0:5, 5, :], d['w1p'][e, 640:645, :])
                he = [ep.tile([128, BC * (T + 2) + 2], bf16, tag=f'he{i}', name=f'he{i}') for i in range(2)]
                for i in range(2):
                    hv = he[i][:, 0:BC * (T + 2)].rearrange('p (b t) -> p b t', t=T + 2)
                    nc.gpsimd.memset(hv[:, :, 0:1], 0.0)
                    nc.gpsimd.memset(hv[:, :, T + 1:T + 2], 0.0)
                    nc.gpsimd.memset(he[i][:, BC * (T + 2):], 0.0)
                for mi in range(2):
                    for ni in range(2):
                        pe1 = psB.tile([128, H * T], f32, tag='pb', name='pb')
                        for k in range(6):
                            kn = 128 if k < 5 else 5
                            mm(pe1[:], w1s[0:kn, k, mi * 128:(mi + 1) * 128],
                               imt[k][:].rearrange('p b t -> p (b t)')[:, ni * H * T:(ni + 1) * H * T],
                               k == 0, k == 5)
                        nc.scalar.activation(he[mi][:, 0:BC * (T + 2)].rearrange('p (b t) -> p b t', t=T + 2)[:, ni * H:(ni + 1) * H, 1:1 + T],
                                             pe1[:].rearrange('p (b t) -> p b t', t=T),
                                             actf.Relu, bias=C['w1b'][:, mi, e:e + 1])
                w2s = ep.tile([128, 6, 256], bf16, tag='w2s', name='w2s')
                nc.sync.dma_start(w2s[:], d['w2p'][e].rearrange('(k p) m -> p k m', p=128))
                eo = [ep.tile([128, BC, T], bf16, tag=f'eo{i}', name=f'eo{i}', bufs=1) for i in range(2)]
                W2 = T + 2
                for mi in range(2):
                    for bi in range(2):
                        pe2 = psB.tile([128, H * W2], f32, tag='pb', name='pb')
                        for k in range(6):
                            dt, ch = divmod(k, 2)
                            mm(pe2[:], w2s[:, k, mi * 128:(mi + 1) * 128],
                               he[ch][:, bi * H * W2 + dt:bi * H * W2 + dt + H * W2],
                               k == 0, k == 5)
                        nc.scalar.activation(eo[mi][:, bi * H:(bi + 1) * H, :],
                                             pe2[:].rearrange('p (b t) -> p b t', t=W2)[:, :, 0:T],
                                             actf.Relu, bias=C['w2b'][:, mi, e:e + 1])
                pwe = psA.tile([128, BC], f32, tag='pa', name='pwe')
                mmf(pwe[:], C['sel8'][:, e * 128:(e + 1) * 128], W8T[:], True, True)
                wE = ep.tile([128, BC], f32, tag='wE', name='wE')
                nc.vector.tensor_copy(wE[:], pwe[:])
                wbc = wE[:].unsqueeze(2).to_broadcast([128, BC, T])
                for mi in range(2):
                    if e == 0:
                        nc.vector.tensor_tensor(accF[mi][:].bitcast(f32r), eo[mi][:], wbc, alu.mult)
                    else:
                        eow = ep.tile([128, BC, T], f32, tag='eow', name='eow')
                        nc.vector.tensor_tensor(eow[:], eo[mi][:], wbc, alu.mult)
                        nc.vector.tensor_add(accF[mi][:].bitcast(f32r), accF[mi][:], eow[:])

            # ---- LSTM input precompute, part A (accF-dependent k=0,1) ----
            XsT = bp.tile([128, 8, T, 16], bf16, tag='XsT', name='XsT')
            wft = [xpp.tile([128, 1024], bf16, tag=f'wft{k}', name=f'wft{k}', bufs=1)
                   for k in range(4)]
            xtT = [xpp.tile([128, T, 16], bf16, tag=f'xtT{k}', name=f'xtT{k}', bufs=1)
                   for k in range(4)]
            for k in range(4):
                nc.sync.dma_start(wft[k][:], d['wihft'][k * 128:(k + 1) * 128, :])
            for k in range(2):
                nc.vector.tensor_copy(xtT[k][:], accF[k][:, :, :].transpose([0, 2, 1]))
            for jj in range(8):
                for (t0, tl) in ((0, 16), (16, 17)):
                    ps = psB.tile([128, tl * 16], f32, tag='pb', name='pb')
                    for k in range(2):
                        mm(ps[:], wft[k][:, jj * 128:(jj + 1) * 128],
                           xtT[k][:, t0:t0 + tl, :].rearrange('p t b -> p (t b)'),
                           k == 0, k == 1)
                    nc.scalar.activation(
                        XsT[:, jj, t0:t0 + tl, :].rearrange('p t b -> p (t b)'),
                        ps[:], actf.Identity, bias=C['biasfT'][:, jj:jj + 1])

            ep_cm.__exit__(None, None, None)

            # ---------------- line branch ----------------

            hp2 = lp.tile([128, BC, 258], bf16, tag='hp2', name='hp2', bufs=1)
            nc.gpsimd.memset(hp2[:, :, 0:1], 0.0)
            nc.gpsimd.memset(hp2[:, :, 257:258], 0.0)
            hp2v = hp2[:, :, 1:257].rearrange('p (g s) t -> p s g t', s=2)
            seacc = lp.tile([128, 2, 8, 2], f32, tag='seacc', name='seacc', bufs=1)
            hs = [lp.tile([128, 8, 1028], bf16, tag=f'hs{s}', name=f'hs{s}', bufs=1)
                  for s in range(2)]
            for s in range(2):
                nc.sync.dma_start(hs[s][0:64], h1[s * 64:(s + 1) * 64])
                nc.sync.dma_start(hs[s][64:128, :, 0:1027], h1[s * 64:(s + 1) * 64, :, 1:1028])
            for s in range(2):
                for gg in range(8):
                    for uh in range(2):
                        pl2 = psB.tile([128, 512], f32, tag='pb', name='pb')
                        base = uh * 512
                        mm(pl2[:], C['w2Lr'][:, 0, :], hs[s][:, gg, base:base + 512],
                           True, False)
                        mm(pl2[:], C['w2Lr'][:, 1, :], hs[s][:, gg, base + 2:base + 514],
                           False, False)
                        mm(pl2[:], C['w2Lr'][0:64, 2, :], hs[s][0:64, gg, base + 4:base + 516],
                           False, True)
                        r2 = lp.tile([128, 512], f32, tag='r2', name='r2')
                        nc.scalar.activation(r2[:], pl2[:], actf.Relu, bias=C['lb2c'][:, 0:1],
                                             accum_out=seacc[:, s, gg, uh:uh + 1])
                        nc.vector.tensor_reduce(hp2v[:, s, gg, uh * 128:uh * 128 + 128],
                                                r2[:].rearrange('p (w q) -> p w q', q=4),
                                                X, alu.max)
            seY = lp.tile([128, 2, 8], f32, tag='seY', name='seY')
            nc.vector.tensor_reduce(seY[:], seacc[:], X, alu.add)
            pse1 = psA.tile([32, 16], f32, tag='pa', name='pa')
            mmf(pse1[:], C['se2w1t'][:], seY[:].rearrange('p s g -> p (s g)'), True, True)
            z2 = lp.tile([32, 16], f32, tag='z2', name='z2')
            nc.scalar.activation(z2[:], pse1[:], actf.Relu)
            pse2 = psA.tile([128, 16], f32, tag='pa', name='pa')
            mmf(pse2[:], C['se2w2t'][:], z2[:], True, True)
            sc2 = lp.tile([128, 2, 8], f32, tag='sc2', name='sc2')
            nc.scalar.activation(sc2[:].rearrange('p s g -> p (s g)'), pse2[:], actf.Sigmoid)
            nc.vector.tensor_tensor(
                hp2[:, :, 1:257].rearrange('p (g s) t -> p g s t', s=2),
                hp2[:, :, 1:257].rearrange('p (g s) t -> p g s t', s=2),
                sc2[:].rearrange('p s g -> p g s').unsqueeze(3).to_broadcast([128, 8, 2, 256]),
                alu.mult)

            # conv3 + SE3 + pool, chunked over sample pairs
            y3 = lp.tile([128, 2, BC], f32, tag='y3', name='y3')
            lf = [lp.tile([128, BC, 64], bf16, tag=f'lf{i}', name=f'lf{i}', bufs=1) for i in range(2)]
            hp2f = hp2[:].rearrange('p b t -> p (b t)')
            for mi in range(2):
                for b0 in range(0, BC, 2):
                    pl3 = psB.tile([128, 2, 256], f32, tag='pb', name='pb')
                    for bi in (b0, b0 + 1):
                        for dt in range(3):
                            mm(pl3[:, bi - b0, :], C['w3L'][:, dt, mi * 128:(mi + 1) * 128],
                               hp2f[:, bi * 258 + dt:bi * 258 + dt + 256], dt == 0, dt == 2)
                    r3 = lp.tile([128, 2, 256], f32, tag='r3', name='r3')
                    nc.scalar.activation(r3[:], pl3[:],
                                         actf.Relu, bias=C['lb3c'][:, mi, 0:1])
                    nc.vector.tensor_reduce(y3[:, mi, b0:b0 + 2], r3[:], X, alu.add)
                    nc.vector.tensor_reduce(lf[mi][:, b0:b0 + 2, :],
                                            r3[:].rearrange('p c (u q) -> p c u q', q=4),
                                            X, alu.max)
            pse3 = psA.tile([64, 16], f32, tag='pa', name='pa')
            for k in range(2):
                mmf(pse3[:], C['se3w1t'][:, k, :], y3[:, k, :], k == 0, k == 1)
            z3 = lp.tile([64, 16], f32, tag='z3', name='z3')
            nc.scalar.activation(z3[:], pse3[:], actf.Relu)
            sc3 = [lp.tile([128, BC], f32, tag=f'sc3_{i}', name=f'sc3_{i}') for i in range(2)]
            for mi in range(2):
                pse4 = psA.tile([128, 16], f32, tag='pa', name='pa')
                mmf(pse4[:], C['se3w2t'][:, mi * 128:(mi + 1) * 128], z3[:], True, True)
                nc.scalar.activation(sc3[mi][:], pse4[:], actf.Sigmoid)
            for mi in range(2):
                nc.vector.tensor_tensor(lf[mi][:], lf[mi][:],
                                        sc3[mi][:].unsqueeze(2).to_broadcast([128, BC, 64]),
                                        alu.mult)
            # interp 64 -> 33
            li = [bp.tile([128, BC, T], f32, tag=f'li{i}', name=f'li{i}') for i in range(2)]
            pwl = psA.tile([128, T], f32, tag='pa', name='pwl')
            mmf(pwl[:], C['ones1'][:], C['wlo'][:], True, True)
            wlo128 = lp.tile([128, T], f32, tag='wlo128', name='wlo128', bufs=1)
            nc.vector.tensor_copy(wlo128[:], pwl[:])
            wbc_all = wlo128[:]
            for mi in range(2):
                for (a, n, lo0, st) in runs:
                    end = lo0 + (n - 1) * st + 1
                    lov = lf[mi][:, :, lo0:end:st]
                    hiv = lf[mi][:, :, lo0 + 1:end + 1:st]
                    dd = lp.tile([128, BC, T], f32, tag='dd', name='dd')
                    nc.vector.tensor_sub(dd[:, :, a:a + n], hiv, lov)
                    nc.vector.tensor_tensor(dd[:, :, a:a + n], dd[:, :, a:a + n],
                                            wbc_all[:, a:a + n].unsqueeze(1).to_broadcast([128, BC, n]),
                                            alu.mult)
                    nc.vector.tensor_add(li[mi][:, :, a:a + n].bitcast(f32r), dd[:, :, a:a + n], lov)

            # ---------------- LSTM input precompute, part B (li-dependent k=2,3) --
            lct = [li[0], li[1]]
            for k in (2, 3):
                nc.vector.tensor_copy(xtT[k][:], lct[k - 2][:, :, :].transpose([0, 2, 1]))
            for jj in range(8):
                for (t0, tl) in ((0, 16), (16, 17)):
                    ps = psB.tile([128, tl * 16], f32, tag='pb', name='pb')
                    for k in (2, 3):
                        mm(ps[:], wft[k][:, jj * 128:(jj + 1) * 128],
                           xtT[k][:, t0:t0 + tl, :].rearrange('p t b -> p (t b)'),
                           k == 2, k == 3)
                    xv = XsT[:, jj, t0:t0 + tl, :].rearrange('p t b -> p (t b)')
                    nc.vector.tensor_tensor(xv, ps[:], xv, alu.add)

            # ---------------- backward cell (t=32, transposed) ----------------
            wbt = [xpp.tile([128, 1024], bf16, tag=f'wft{k}', name=f'wbt{k}', bufs=1)
                   for k in range(4)]
            for k in range(4):
                nc.sync.dma_start(wbt[k][:], d['wihbt'][k * 128:(k + 1) * 128, :])
            psb = psA.tile([128, 8, 16], f32, tag='pa', name='psb')
            for jj in range(8):
                for k in range(4):
                    mm(psb[:, jj, :], wbt[k][:, jj * 128:(jj + 1) * 128],
                       xtT[k][:, 32, :], k == 0, k == 3)
            gbT = wp1.tile([128, 8, 16], f32, tag='gbT', name='gbT')
            for jj in range(8):
                nc.scalar.activation(gbT[:, jj, :], psb[:, jj, :], actf.Identity,
                                     bias=C['biasbT'][:, jj:jj + 1])
            sgb = wp1.tile([128, 8, 16], f32, tag='sgb', name='sgb')
            nc.scalar.activation(sgb[:, 0:6, :], gbT[:, 0:6, :], actf.Sigmoid)
            nc.scalar.activation(sgb[:, 6:8, :], gbT[:, 6:8, :], actf.Tanh)
            cbT = wp1.tile([128, 2, 16], f32, tag='cbT', name='cbT')
            nc.vector.tensor_tensor(cbT[:], sgb[:, 0:2, :], sgb[:, 6:8, :], alu.mult)
            tcb = wp1.tile([128, 2, 16], f32, tag='tcb', name='tcb')
            nc.scalar.activation(tcb[:], cbT[:], actf.Tanh)
            hbT = bp.tile([128, 2, 16], f32, tag='hbT', name='hbT')
            nc.vector.tensor_tensor(hbT[:], sgb[:, 4:6, :], tcb[:], alu.mult)
            xp_cm.__exit__(None, None, None)
            lp_cm.__exit__(None, None, None)

            # ---------------- forward LSTM (33 steps, transposed) ----------------
            # gates live as [128 = j-chunk, jj, 16 = batch]; no per-step transposes.
            hT = None
            cT = None
            for t in range(T):
                psg = psB.tile([128, 8, 16], f32, tag='pb', name='psg')
                for jj in range(8):
                    mm(psg[:, jj, :], C['i128'][:], XsT[:, jj, t, :], True, t == 0)
                    if t > 0:
                        for k in range(2):
                            mmr(psg[:, jj, :], C['whhft'][:, k, jj * 128:(jj + 1) * 128],
                                hT[:, k, :], False, k == 1)
                sg = wp.tile([128, 8, 16], f32, tag='lstm_sg', name='lstm_sg')
                nc.scalar.activation(sg[:, 0:6, :], psg[:, 0:6, :], actf.Sigmoid)
                nc.scalar.activation(sg[:, 6:8, :], psg[:, 6:8, :], actf.Tanh)
                cT_new = wp.tile([128, 2, 16], f32, tag='lstm_c', name='lstm_c')
                if t == 0:
                    nc.vector.tensor_tensor(cT_new[:], sg[:, 0:2, :], sg[:, 6:8, :], alu.mult)
                else:
                    t1 = wp.tile([128, 2, 16], f32, tag='lstm_t1', name='lstm_t1')
                    nc.vector.tensor_tensor(t1[:], sg[:, 2:4, :], cT[:], alu.mult)
                    t2 = wp.tile([128, 2, 16], f32, tag='lstm_t2', name='lstm_t2')
                    nc.vector.tensor_tensor(t2[:], sg[:, 0:2, :], sg[:, 6:8, :], alu.mult)
                    nc.vector.tensor_tensor(cT_new[:], t1[:], t2[:], alu.add)
                cT = cT_new
                tct = wp.tile([128, 2, 16], f32, tag='lstm_tc', name='lstm_tc')
                nc.scalar.activation(tct[:], cT[:], actf.Tanh)
                hT_new = wp.tile([128, 2, 16], f32, tag='lstm_h', name='lstm_h')
                nc.vector.tensor_tensor(hT_new[:].bitcast(f32r), sg[:, 4:6, :], tct[:],
                                        alu.mult)
                hT = hT_new

            # ---------------- FFN head ----------------
            lastT = [hT[:, 0, :], hT[:, 1, :], hbT[:, 0, :], hbT[:, 1, :]]
            z = [wp1.tile([128, 16], f32, tag=f'z_{i}', name=f'z_{i}') for i in range(2)]
            for mi in range(2):
                pz = psA.tile([128, 16], f32, tag='pa', name='pa')
                for k in range(4):
                    mmf(pz[:], C['ffn1t'][:, k, mi * 128:(mi + 1) * 128], lastT[k],
                        k == 0, k == 3)
                nc.scalar.activation(z[mi][:], pz[:], actf.Relu,
                                     bias=C['ffnb1'][:, mi, 0:1])
            py = psA.tile([1, 16], f32, tag='pa', name='pa')
            for k in range(2):
                mmf(py[:], C['ffn2t'][:, k, :], z[k][:], k == 0, k == 1)
            yo = wp1.tile([1, 16], f32, tag='yo', name='yo')
            nc.scalar.activation(yo[:], py[:], actf.Copy, bias=float(ffn_b2_val))
            nc.sync.dma_start(yout[:].unsqueeze(0), yo[:])

    _split_tpb_waits(nc)
    return nc


def _split_tpb_waits(nc, max_waits=1):
    """This walrus build caps sync-waits per TPB instruction; hoist extras
    onto same-engine NoOps placed immediately before the instruction."""
    from concourse import mybir
    dma_ops = ('DMACopy', 'DMATranspose', 'TensorLoad', 'TensorSave')
    cnt = 0
    for f in nc.m.functions:
        for bb in f.blocks:
            out = []
            changed = False
            for inst in bb.instructions:
                si = inst.sync_info
                opc = getattr(inst, 'opcode', '') or type(inst).__name__
                if (si is not None and len(si.on_wait) > max_waits
                        and getattr(inst, 'engine', None) is not None):
                    waits = list(si.on_wait)
                    for w in waits[:-max_waits]:
                        nop = mybir.InstNoOp(name=f'{inst.name}-sw{cnt}', ins=[], outs=[])
                        cnt += 1
                        nop.engine = inst.engine
                        nop.sync_info = mybir.SyncInfo(on_wait=[w], on_update=[])
                        out.append(nop)
                    inst.sync_info = mybir.SyncInfo(on_wait=waits[-max_waits:],
                                                    on_update=list(si.on_update))
                    changed = True
                out.append(inst)
            if changed:
                bb.instructions = out
    return nc


def _host_prep(inputs):
    f = lambda x: np.ascontiguousarray(x, dtype=np.float32)
    n = np.arange(NFFT)
    win = 0.5 * (1.0 - np.cos(2.0 * np.pi * n / NFFT))
    k = np.arange(NF)
    ang = 2.0 * np.pi * np.outer(n, k) / NFFT
    gw1t = inputs['gate_w1'].T / T
    runs, lo_t, w_t = _interp_tables()
    w14 = np.zeros((14, 128), np.float32)
    for s in range(2):
        for jj in range(7):
            w14[s * 7 + jj, s * 64:(s + 1) * 64] = inputs['lw1'][:, 0, jj]
    w14r = np.concatenate([np.concatenate([w14, np.zeros((18, 128), np.float32)])] * 3)
    wt = np.transpose(inputs['lw2'], (1, 2, 0))  # [64ch, 5dt, 128oc]
    w2Lr = np.zeros((128, 3, 128), np.float32)
    for c in range(3):
        w2Lr[0:64, c, :] = wt[:, 2 * c, :]
        if c < 2:
            w2Lr[64:128, c, :] = wt[:, 2 * c + 1, :]
    shared = {
        'crw': f(win[:, None] * np.cos(ang)),
        'ciw': f(win[:, None] * np.sin(ang)),
        'gw1ta': f(gw1t[0:128]), 'gw1tb': f(gw1t[128:129]),
        'gb1c': f(inputs['gate_b1'][:, None]),
        'gw2t': f(inputs['gate_w2'].T), 'gb2c': f(inputs['gate_b2'][:, None]),
        'iota8': f(np.tile(np.arange(NE)[None, :], (BC, 1))),
        'ones1': f(np.ones((1, 128))),
        'zer': f(np.zeros((128, 70))),
        'sel8': f(np.concatenate([np.tile(v[:, None], (1, 128)) for v in np.eye(NE)], axis=1)),
        'w1p': np.asarray(np.transpose(inputs['exp_w1'], (0, 3, 2, 1)).reshape(NE, 645, 256), dtype=ml_dtypes.bfloat16),
        'w1b': f(inputs['exp_b1'].T),
        'w2p': np.asarray(np.transpose(inputs['exp_w2'], (0, 3, 2, 1)).reshape(NE, 768, 256), dtype=ml_dtypes.bfloat16),
        'w2b': f(inputs['exp_b2'].T),
        'w14r': np.asarray(w14r, dtype=ml_dtypes.bfloat16), 'lb1c': f(np.tile(inputs['lb1'], 2)[:, None]),
        'w2Lr': np.asarray(w2Lr, dtype=ml_dtypes.bfloat16),
        'lb2c': f(inputs['lb2'][:, None]),
        'se2w1t': f(inputs['se2_w1'].T / 1024.0), 'se2w2t': f(inputs['se2_w2'].T),
        'w3L': np.asarray(np.transpose(inputs['lw3'], (2, 1, 0)), dtype=ml_dtypes.bfloat16),
        'lb3c': f(inputs['lb3'][:, None]),
        'se3w1t': f(inputs['se3_w1'].T / 256.0), 'se3w2t': f(inputs['se3_w2'].T),
        'wlo': f(w_t[None, :]),
        'wihft': np.asarray(inputs['wih_f'].T[:, GPERM], dtype=ml_dtypes.bfloat16),
        'biasfT': f((inputs['bih_f'] + inputs['bhh_f'])[GPERM].reshape(NE, 128).T),
        'whhft': f(inputs['whh_f'].T[:, GPERM]),
        'wihbt': np.asarray(inputs['wih_b'].T[:, GPERM], dtype=ml_dtypes.bfloat16),
        'biasbT': f((inputs['bih_b'] + inputs['bhh_b'])[GPERM].reshape(NE, 128).T),
        'i16': f(np.eye(16)), 'i128': np.asarray(np.eye(128), dtype=ml_dtypes.bfloat16),
        'ffn1t': f(inputs['ffn_w1'].T), 'ffnb1': f(inputs['ffn_b1'][:, None]),
        'ffn2t': f(inputs['ffn_w2'].T),
    }
    xp = np.pad(inputs['x_continuum'], ((0, 0), (NFFT // 2, NFFT // 2)), mode='reflect')
    s0, s1 = xp.strides
    frames = np.lib.stride_tricks.as_strided(xp, (B, T, NFFT), (s0, 128 * s1, s1))
    xnp = np.pad(inputs['x_normalized'], ((0, 0), (3, 3 + 10)))
    in_maps = []
    for c in range(N_CORES):
        m = dict(shared)
        fr = frames[c * BC:(c + 1) * BC]
        m['framesT'] = f(np.transpose(fr, (2, 0, 1)).reshape(NFFT, BC * T))
        xc = xnp[c * BC:(c + 1) * BC]  # [16, 4112]
        x7b = np.zeros((3, 96, 8, 520), np.float32)
        for u in range(8):
            for s in range(2):
                for jj in range(7):
                    r = (u % 3) * 32 + s * 7 + jj
                    for gg in range(8):
                        x7b[u // 3, r, gg, :] = xc[gg * 2 + s, u * 512 + jj:u * 512 + jj + 520]
        m['x7b'] = np.asarray(x7b, dtype=ml_dtypes.bfloat16)
        in_maps.append(m)
    return in_maps


def _apply_tile_patch():
    from concourse import tile, mybir
    from concourse.vector_clock import ScopedClock

    def _drain_split(self, tick_clock, wait_clock):
        nc2 = self.nc
        di = nc2.sync.drain()
        wait_clock.add_sem_waits(di.ins, ScopedClock({None: tick_clock.global_clock}))
        si = di.ins.sync_info
        if si is not None and len(si.on_wait) > 1:
            waits = list(si.on_wait)
            ups = list(si.on_update)
            di.ins.sync_info = mybir.SyncInfo(on_wait=waits[:1], on_update=[])
            for kk, w in enumerate(waits[1:]):
                extra = nc2.sync.drain()
                extra.ins.sync_info = mybir.SyncInfo(
                    on_wait=[w], on_update=ups if kk == len(waits) - 2 else [])
        nc2.all_engine_barrier()
        assert self.sems is not None
        popped = nc2._tile_sem_poison_stack.pop()
        assert popped is self._sem_poison
        nc2.clear_and_free_semaphores(list(self.sems.allocated().values()))
        nc2.all_engine_barrier()

    tile.TileContext._drain_and_barrier = _drain_split


def kernel(**inputs):
    global _cache
    if 'nc' not in _cache:
        _apply_tile_patch()
        _cache['nc'] = _build(float(np.asarray(inputs['ffn_b2']).reshape(-1)[0]))
    from concourse.bass_utils import run_bass_kernel_spmd
    in_maps = _host_prep(inputs)
    res = run_bass_kernel_spmd(_cache['nc'], in_maps, list(range(N_CORES)))
    out = np.concatenate([res.results[c]['yout'] for c in range(N_CORES)])
    return out[:, None].astype(np.float32)

